# revision 1
# baseline (speedup 1.0000x reference)
"""GTLayer (relational graph transformer layer) on 8 Trainium2 NeuronCores.

Strategy
--------
Nodes are partitioned across 8 cores in graph-aligned contiguous slices
(graphNorm stays core-local). Edges live with the core that owns dst.
Per core, dst nodes are processed in 256-node windows.

- graphNorm1: slice-local stats (one-pass sum/sumsq via one-hot matmuls),
  normalize, then AllGather hn -> global gather table.
- RelConv (Q|K|V fused, 384 cols): edges sorted by (window, src-half,
  relation), each (w,half,r) run padded to 128-slot chunks (>=1 pad slot per
  chunk, index 0, key -1). hn[src] rows fetched with the ext-isa dma_gather
  (int16 indices, signed, two base offsets 32768/98304 cover the global
  table). Aggregation is one-hot matmuls: S_r^T[D,256] += G[e,D]^T-free
  matmul rhs one-hot A[e,256]; transform S_r @ W_r accumulated in PSUM,
  plus self-loop + bias + relu. K|V written to a kv table, AllGathered.
- Attention: same chunk machinery per (window, half). Per chunk: one-hot A,
  PE-transposed for Qdst = A @ Q_win; scores = rowdot(K, Qdst) per head;
  exp(clip(s/4)); wV,z aggregated by matmuls with lhsT=A.
- Epilogue per 128 rows: attn = wV/(z+eps), hO = attn@o_w+o_b, LN1,
  graphNorm2 (stats pass over SBUF-resident h1), FFN, LN2 -> output slice.

All per-core variation is in input data (indices/keys); the SPMD program is
identical across cores (chunk counts are max'ed over cores).
"""
import os
import sys
import types
import numpy as np

NCORES = 8
N_NODES = 100000
N_EDGES = 600000
D = 128
REL = 9
NG = 64
HEADS = 8
DH = 16
WIN = 256          # dst window (2 x 128 subwindows)
GMAX = 16          # max graphs per core
BASE_LO = 32768    # gather base offsets (int16 signed reach +-32767)
BASE_HI = 98304
HALF_SPLIT = 65536
CALL_MAX = 8       # max chunks (of 128 slots) per dma_gather call


def _ensure_hooks():
    if "antenv.axon_hooks" not in sys.modules:
        hooks = types.ModuleType("antenv.axon_hooks")
        h = [None]
        hooks.set_axon_ntff_profile_hook = lambda v: h.__setitem__(0, v)
        hooks.get_axon_ntff_profile_hook = lambda: h[0]
        sys.modules["antenv.axon_hooks"] = hooks
        try:
            from trn_agent_boot.trn_boot import _ntff_profile_via_ctypes
            hooks.set_axon_ntff_profile_hook(
                _ntff_profile_via_ctypes("/opt/axon/libaxon_pjrt.so"))
        except Exception:
            pass


# ----------------------------------------------------------------------------
# Host preprocessing
# ----------------------------------------------------------------------------

def _pack_idx16(idx):
    """int16 index array -> [128, n/16] wrapped+replicated layout."""
    n = len(idx)
    assert n % 16 == 0
    blk = idx.reshape(n // 16, 16).T
    return np.tile(blk, (8, 1)).astype(np.int16)


def _layout_slots(order_edges, idx_vals, key_vals, n_chunks):
    """Place edges into n_chunks*128 slots, <=127 real per chunk, last slot of
    each chunk is a pad (idx 0, key -1). Returns (idx int32, key f32)."""
    tot = n_chunks * 128
    idx = np.zeros(tot, np.int32)
    key = np.full(tot, -1.0, np.float32)
    ne = len(order_edges)
    pos = 0
    ei = 0
    for c in range(n_chunks):
        take = min(127, ne - ei)
        if take > 0:
            sl = slice(c * 128, c * 128 + take)
            idx[sl] = idx_vals[order_edges[ei:ei + take]]
            key[sl] = key_vals[order_edges[ei:ei + take]]
            ei += take
    assert ei == ne, (ei, ne, n_chunks)
    return idx, key


def preprocess(inputs):
    h = np.asarray(inputs['h'], np.float32)
    src = np.asarray(inputs['src']).astype(np.int64)
    dst = np.asarray(inputs['dst']).astype(np.int64)
    et = np.asarray(inputs['etypes']).astype(np.int64)
    seg = np.asarray(inputs['seg']).astype(np.int64)

    # --- graph-aligned node partition ---
    gstart = np.searchsorted(seg, np.arange(NG + 1))  # graph g: [gstart[g], gstart[g+1])
    bounds = [0]
    for c in range(1, NCORES):
        target = c * N_NODES / NCORES
        g = int(np.argmin(np.abs(gstart - target)))
        bounds.append(int(gstart[g]))
    bounds.append(N_NODES)
    n0 = np.array(bounds[:-1]); n1 = np.array(bounds[1:])
    sizes = n1 - n0
    P_NODES = int(np.ceil(sizes.max() / WIN) * WIN)
    NW = P_NODES // WIN
    NTAB = NCORES * P_NODES
    assert NTAB <= BASE_HI + 32767

    owner = np.searchsorted(n1, np.arange(N_NODES), side='right')
    pos = owner * P_NODES + (np.arange(N_NODES) - n0[owner])

    # --- per-core graph info ---
    g0 = np.searchsorted(gstart, n0, side='right') - 1  # first graph on core
    counts_g = np.diff(gstart).astype(np.float32)

    srcp = pos[src]
    half = (srcp >= HALF_SPLIT).astype(np.int64)
    ecore = owner[dst]
    dst_off = dst - n0[ecore]
    w_e = dst_off // WIN
    dl_e = (dst_off % WIN).astype(np.float32)
    idx_rel = np.where(half == 0, srcp - BASE_LO, srcp - BASE_HI).astype(np.int32)

    # --- relconv structure: runs (w, rgroup, half, r); rgroup = 0 for r<5 ---
    rgrp = (et >= 5).astype(np.int64)
    rkey = ((w_e * 2 + rgrp) * 2 + half) * REL + et
    rc_counts = np.zeros((NCORES, NW * 4 * REL), np.int64)
    for c in range(NCORES):
        m = ecore == c
        rc_counts[c] = np.bincount(rkey[m], minlength=NW * 4 * REL)
    rc_chunks = np.maximum(np.ceil(rc_counts / 127.0).max(0), 1).astype(np.int64)
    # zero out slots for relations not in their group: key (w,g,hh,r) valid iff (r<5)==(g==0)
    kgrid = np.arange(NW * 4 * REL)
    r_of = kgrid % REL
    g_of = (kgrid // (2 * REL)) % 2
    rc_chunks[np.where((r_of < 5) != (g_of == 0))] = 0

    # --- attention structure: segments (sw128, half) ---
    sw_e = dst_off // 128
    dl128 = (dst_off % 128).astype(np.float32)
    NSW = NW * 2
    akey = sw_e * 2 + half
    at_counts = np.zeros((NCORES, NSW * 2), np.int64)
    for c in range(NCORES):
        m = ecore == c
        at_counts[c] = np.bincount(akey[m], minlength=NSW * 2)
    at_chunks = np.maximum(np.ceil(at_counts / 127.0).max(0), 1).astype(np.int64)

    RC_CHUNKS = int(rc_chunks.sum())
    AT_CHUNKS = int(at_chunks.sum())

    # static gather call lists: (nchunks) per call, chopping each same-base
    # contiguous region into <= CALL_MAX chunk pieces.
    def calls_for(region_chunk_counts):
        calls = []
        for n in region_chunk_counts:
            n = int(n)
            while n > 0:
                take = min(n, CALL_MAX)
                calls.append(take)
                n -= take
        return calls


    # --- per-core data arrays ---
    in_maps = []
    for c in range(NCORES):
        m = np.nonzero(ecore == c)[0]
        rk = rkey[m]
        order = np.argsort(rk, kind='stable')
        edges_sorted = m[order]
        rk_sorted = rk[order]
        run_start = np.searchsorted(rk_sorted, np.arange(NW * 4 * REL))
        run_end = np.searchsorted(rk_sorted, np.arange(NW * 4 * REL) + 1)

        rc_idx = np.zeros(RC_CHUNKS * 128, np.int32)
        rc_key = np.full(RC_CHUNKS * 128, -1.0, np.float32)
        coff = 0
        for q in range(NW * 4 * REL):
            nch = int(rc_chunks[q])
            eidx = edges_sorted[run_start[q]:run_end[q]]
            ii, kk = _layout_slots(eidx, idx_rel, dl_e, nch)
            rc_idx[coff * 128:(coff + nch) * 128] = ii
            rc_key[coff * 128:(coff + nch) * 128] = kk
            coff += nch
        assert coff == RC_CHUNKS

        ak = akey[m]
        aorder = np.argsort(ak, kind='stable')
        aedges = m[aorder]
        ak_sorted = ak[aorder]
        astart = np.searchsorted(ak_sorted, np.arange(NSW * 2))
        aend = np.searchsorted(ak_sorted, np.arange(NSW * 2) + 1)
        at_idx = np.zeros(AT_CHUNKS * 128, np.int32)
        at_key = np.full(AT_CHUNKS * 128, -1.0, np.float32)
        coff = 0
        for q in range(NSW * 2):
            nch = int(at_chunks[q])
            eidx = aedges[astart[q]:aend[q]]
            ii, kk = _layout_slots(eidx, idx_rel, dl128, nch)
            at_idx[coff * 128:(coff + nch) * 128] = ii
            at_key[coff * 128:(coff + nch) * 128] = kk
            coff += nch
        assert coff == AT_CHUNKS

        hs = np.zeros((P_NODES, D), np.float32)
        hs[:sizes[c]] = h[n0[c]:n1[c]]
        segl = np.full(P_NODES, -1.0, np.float32)
        segl[:sizes[c]] = (seg[n0[c]:n1[c]] - g0[c]).astype(np.float32)
        ginc = np.zeros((GMAX, 1), np.float32)
        ng_c = int(seg[n1[c] - 1] - g0[c]) + 1
        assert ng_c <= GMAX
        ginc[:ng_c, 0] = 1.0 / counts_g[g0[c]:g0[c] + ng_c]

        im = {
            'h_slice': hs,
            'seg_col': segl.reshape(NW * 2, 128).T.copy(),   # [128, NW*2]
            'inv_cnt': ginc,
            'rc_idx': _pack_idx16(rc_idx.astype(np.int16)),
            'rc_key': rc_key.reshape(RC_CHUNKS, 128).T.copy(),  # [128, RC_CHUNKS]
            'at_idx': _pack_idx16(at_idx.astype(np.int16)),
            'at_key': at_key.reshape(AT_CHUNKS, 128).T.copy(),
        }
        in_maps.append(im)

    # --- shared weights (same for all cores) ---
    def A(x):
        return np.ascontiguousarray(np.asarray(x, np.float32))
    Wrel = np.concatenate([
        np.einsum('rb,bio->rio', A(inputs[f'{nm}_coeff']), A(inputs[f'{nm}_basis']))
        for nm in ('q', 'k', 'v')], axis=2)            # [9, 128, 384]
    w_shared = {
        'w_rel': A(Wrel.reshape(REL * D, 3 * D)),
        'w_loop': np.concatenate([A(inputs[f'{nm}_loop']) for nm in ('q', 'k', 'v')], 1),
        'b_qkv': np.tile(np.concatenate([A(inputs[f'{nm}_bias']) for nm in ('q', 'k', 'v')])[None, :], (128, 1)),
        'o_w': A(inputs['o_w']), 'o_b': np.tile(A(inputs['o_b'])[None, :], (128, 1)),
        'ffn1': A(inputs['ffn1_w']), 'ffn1_b': np.tile(A(inputs['ffn1_b'])[None, :], (128, 1)),
        'ffn2': A(inputs['ffn2_w']), 'ffn2_b': np.tile(A(inputs['ffn2_b'])[None, :], (128, 1)),
        'ln1_g': np.tile(A(inputs['ln1_g'])[None, :], (128, 1)),
        'ln1_b': np.tile(A(inputs['ln1_b'])[None, :], (128, 1)),
        'ln2_g': np.tile(A(inputs['ln2_g'])[None, :], (128, 1)),
        'ln2_b': np.tile(A(inputs['ln2_b'])[None, :], (128, 1)),
    }
    for nm in ('gn1', 'gn2'):
        w = A(inputs[f'{nm}_w']); b = A(inputs[f'{nm}_b']); ms = A(inputs[f'{nm}_ms'])
        w_shared[f'{nm}_w16'] = np.tile(w[None, :], (GMAX, 1))
        w_shared[f'{nm}_b16'] = np.tile(b[None, :], (GMAX, 1))
        w_shared[f'{nm}_ms16'] = np.tile(ms[None, :], (GMAX, 1))
        w_shared[f'{nm}_msfac16'] = np.tile((ms * (2 - ms))[None, :], (GMAX, 1))
    for im in in_maps:
        im.update(w_shared)

    static = dict(P_NODES=P_NODES, NW=NW, NTAB=NTAB,
                  rc_chunks=tuple(int(x) for x in rc_chunks),
                  at_chunks=tuple(int(x) for x in at_chunks),
                  RC_CHUNKS=RC_CHUNKS, AT_CHUNKS=AT_CHUNKS)
    meta = dict(n0=n0, n1=n1, sizes=sizes)
    return static, in_maps, meta


# ----------------------------------------------------------------------------
# Bass program
# ----------------------------------------------------------------------------

_PROGRAM_CACHE = {}


def build_program(st):
    import concourse.bass as bass
    import concourse.bacc as bacc
    import concourse.mybir as mybir
    import concourse.tile as tile
    from concourse.tile import TileContext
    from concourse.masks import make_identity
    from bass_rust import add_dep_helper

    P_NODES = st['P_NODES']; NW = st['NW']; NTAB = st['NTAB']
    rc_chunks = st['rc_chunks']; at_chunks = st['at_chunks']
    RC_CHUNKS = st['RC_CHUNKS']; AT_CHUNKS = st['AT_CHUNKS']
    f32 = mybir.dt.float32
    i16 = mybir.dt.int16
    AO = mybir.AluOpType
    AF = mybir.ActivationFunctionType

    nc = bacc.Bacc()

    # --- I/O ---
    h_slice = nc.declare_dram_parameter('h_slice', [P_NODES, D], f32, isOutput=False)
    seg_col = nc.declare_dram_parameter('seg_col', [128, NW * 2], f32, isOutput=False)
    inv_cnt = nc.declare_dram_parameter('inv_cnt', [GMAX, 1], f32, isOutput=False)
    rc_idx = nc.declare_dram_parameter('rc_idx', [128, RC_CHUNKS * 8], i16, isOutput=False)
    rc_keyd = nc.declare_dram_parameter('rc_key', [128, RC_CHUNKS], f32, isOutput=False)
    at_idx = nc.declare_dram_parameter('at_idx', [128, AT_CHUNKS * 8], i16, isOutput=False)
    at_keyd = nc.declare_dram_parameter('at_key', [128, AT_CHUNKS], f32, isOutput=False)
    w_rel = nc.declare_dram_parameter('w_rel', [REL * D, 3 * D], f32, isOutput=False)
    w_loop = nc.declare_dram_parameter('w_loop', [D, 3 * D], f32, isOutput=False)
    b_qkv = nc.declare_dram_parameter('b_qkv', [128, 3 * D], f32, isOutput=False)
    o_w = nc.declare_dram_parameter('o_w', [D, D], f32, isOutput=False)
    o_b = nc.declare_dram_parameter('o_b', [128, D], f32, isOutput=False)
    ffn1 = nc.declare_dram_parameter('ffn1', [D, 2 * D], f32, isOutput=False)
    ffn1_b = nc.declare_dram_parameter('ffn1_b', [128, 2 * D], f32, isOutput=False)
    ffn2 = nc.declare_dram_parameter('ffn2', [2 * D, D], f32, isOutput=False)
    ffn2_b = nc.declare_dram_parameter('ffn2_b', [128, D], f32, isOutput=False)
    cdecl = {}
    for nm in ('ln1_g', 'ln1_b', 'ln2_g', 'ln2_b'):
        cdecl[nm] = nc.declare_dram_parameter(nm, [128, D], f32, isOutput=False)
    for nm in ('gn1', 'gn2'):
        for sfx in ('w16', 'b16', 'ms16', 'msfac16'):
            cdecl[f'{nm}_{sfx}'] = nc.declare_dram_parameter(
                f'{nm}_{sfx}', [GMAX, D], f32, isOutput=False)
    out_sl = nc.declare_dram_parameter('out_slice', [P_NODES, D], f32, isOutput=True)

    # --- internal DRAM ---
    hn_local = nc.dram_tensor('hn_local', [P_NODES, D], f32)
    q_local = nc.dram_tensor('q_local', [P_NODES, D], f32)
    kv_local = nc.dram_tensor('kv_local', [P_NODES, 2 * D], f32)
    debug = os.environ.get('KERNEL_DEBUG') == '1'
    if debug:
        hn_dbg = nc.declare_dram_parameter('hn_dbg', [P_NODES, D], f32, isOutput=True)
        hnf_dbg = nc.declare_dram_parameter('hnf_dbg', [NCORES * P_NODES, D], f32, isOutput=True)
        q_dbg = nc.declare_dram_parameter('q_dbg', [P_NODES, D], f32, isOutput=True)
        kv_dbg = nc.declare_dram_parameter('kv_dbg', [P_NODES, 2 * D], f32, isOutput=True)
    hn_full = nc.dram_tensor('hn_full', [NCORES, P_NODES, D], f32, addr_space='Shared')
    kv_full = nc.dram_tensor('kv_full', [NCORES, P_NODES, 2 * D], f32, addr_space='Shared')
    hn_flat = hn_full[:].rearrange('c p d -> (c p) d')
    kv_flat = kv_full[:].rearrange('c p d -> (c p) d')

    with TileContext(nc) as tc:
        with tc.tile_pool(name='const', bufs=1) as cpool:
            # constants
            iota = cpool.tile([128, WIN], f32)
            nc.gpsimd.iota(iota[:], pattern=[[1, WIN]], base=0,
                           channel_multiplier=0, allow_small_or_imprecise_dtypes=True)
            ident = cpool.tile([128, 128], f32)
            make_identity(nc, ident[:])
            ones1 = cpool.tile([1, 128], f32)
            nc.gpsimd.memset(ones1[:], 1.0)

            segs = cpool.tile([128, NW * 2], f32)
            nc.sync.dma_start(out=segs[:], in_=seg_col[:])
            rck = cpool.tile([128, RC_CHUNKS], f32)
            nc.sync.dma_start(out=rck[:], in_=rc_keyd[:])
            atk = cpool.tile([128, AT_CHUNKS], f32)
            nc.sync.dma_start(out=atk[:], in_=at_keyd[:])
            rci = cpool.tile([128, RC_CHUNKS * 8], i16)
            nc.sync.dma_start(out=rci[:], in_=rc_idx[:])
            ati = cpool.tile([128, AT_CHUNKS * 8], i16)
            nc.sync.dma_start(out=ati[:], in_=at_idx[:])

            wrel_sb = cpool.tile([128, REL * 3 * D], f32)  # r-th block at [:, r*384:(r+1)*384]
            for r in range(REL):
                nc.sync.dma_start(out=wrel_sb[:, r * 3 * D:(r + 1) * 3 * D],
                                  in_=w_rel[r * D:(r + 1) * D, :])
            wloop_sb = cpool.tile([128, 3 * D], f32)
            nc.sync.dma_start(out=wloop_sb[:], in_=w_loop[:])
            bqkv_sb = cpool.tile([128, 3 * D], f32)
            nc.sync.dma_start(out=bqkv_sb[:], in_=b_qkv[:])
            ow_sb = cpool.tile([D, D], f32)
            nc.sync.dma_start(out=ow_sb[:], in_=o_w[:])
            ob_sb = cpool.tile([128, D], f32)
            nc.sync.dma_start(out=ob_sb[:], in_=o_b[:])
            ffn1_sb = cpool.tile([D, 2 * D], f32)
            nc.sync.dma_start(out=ffn1_sb[:], in_=ffn1[:])
            ffn1b_sb = cpool.tile([128, 2 * D], f32)
            nc.sync.dma_start(out=ffn1b_sb[:], in_=ffn1_b[:])
            ffn2_sb = cpool.tile([128, 2 * D], f32)  # two K-chunks side by side
            nc.sync.dma_start(out=ffn2_sb[:, :D], in_=ffn2[:D, :])
            nc.sync.dma_start(out=ffn2_sb[:, D:], in_=ffn2[D:, :])
            ffn2b_sb = cpool.tile([128, D], f32)
            nc.sync.dma_start(out=ffn2b_sb[:], in_=ffn2_b[:])
            csb = {}
            for nm, dd in cdecl.items():
                t = cpool.tile(list(dd.shape), f32, tag=f'c_{nm}')
                nc.sync.dma_start(out=t[:], in_=dd[:])
                csb[nm] = t
            invc_sb = cpool.tile([GMAX, 1], f32)
            nc.sync.dma_start(out=invc_sb[:], in_=inv_cnt[:])

            NSW = NW * 2  # number of 128-row subwindows

            # =========== phase 1: graphNorm1 ===========
            sc1 = nc.enter_named_scope('ph1_gn1', False)
            with (
                tc.tile_pool(name='p1keep', bufs=1) as keep1,
                tc.tile_pool(name='p1sb', bufs=3) as sb1,
                tc.tile_pool(name='p1ps', bufs=1, space='PSUM') as ps1,
                tc.tile_pool(name='p1ps2', bufs=1, space='PSUM') as ps1b,
            ):
                sum_ps = ps1.tile([GMAX, D], f32, tag='sums')
                sq_ps = ps1.tile([GMAX, D], f32, tag='sqs')
                hwins = []
                for s in range(NSW):
                    hw = keep1.tile([128, D], f32, tag=f'h_{s}')
                    nc.sync.dma_start(out=hw[:], in_=h_slice[s * 128:(s + 1) * 128, :])
                    B = sb1.tile([128, GMAX], f32, tag='B1')
                    nc.vector.tensor_tensor(out=B[:], in0=segs[:, s:s + 1].to_broadcast([128, GMAX]),
                                            in1=iota[:, :GMAX], op=AO.is_equal)
                    hsq = sb1.tile([128, D], f32, tag='hsq')
                    nc.vector.tensor_tensor(out=hsq[:], in0=hw[:], in1=hw[:], op=AO.mult)
                    nc.tensor.matmul(out=sum_ps[:], lhsT=B[:], rhs=hw[:],
                                     start=(s == 0), stop=(s == NSW - 1))
                    nc.tensor.matmul(out=sq_ps[:], lhsT=B[:], rhs=hsq[:],
                                     start=(s == 0), stop=(s == NSW - 1))
                    hwins.append(hw)
                # finalize: alpha/beta [GMAX, D]
                mean = keep1.tile([GMAX, D], f32)
                nc.vector.tensor_tensor(out=mean[:], in0=sum_ps[:],
                                        in1=invc_sb[:].to_broadcast([GMAX, D]), op=AO.mult)
                ex2 = keep1.tile([GMAX, D], f32)
                nc.vector.tensor_tensor(out=ex2[:], in0=sq_ps[:],
                                        in1=invc_sb[:].to_broadcast([GMAX, D]), op=AO.mult)
                msq = keep1.tile([GMAX, D], f32)
                nc.vector.tensor_tensor(out=msq[:], in0=mean[:], in1=mean[:], op=AO.mult)
                nc.vector.tensor_tensor(out=msq[:], in0=msq[:], in1=csb['gn1_msfac16'][:], op=AO.mult)
                var = keep1.tile([GMAX, D], f32)
                nc.vector.tensor_tensor(out=var[:], in0=ex2[:], in1=msq[:], op=AO.subtract)
                nc.vector.tensor_scalar_add(out=var[:], in0=var[:], scalar1=1e-6)
                std = keep1.tile([GMAX, D], f32)
                nc.scalar.activation(out=std[:], in_=var[:], func=AF.Sqrt)
                rstd = keep1.tile([GMAX, D], f32)
                nc.vector.reciprocal(out=rstd[:], in_=std[:])
                alpha1 = keep1.tile([GMAX, D], f32)
                nc.vector.tensor_tensor(out=alpha1[:], in0=rstd[:], in1=csb['gn1_w16'][:], op=AO.mult)
                beta1 = keep1.tile([GMAX, D], f32)
                nc.vector.tensor_tensor(out=beta1[:], in0=mean[:], in1=csb['gn1_ms16'][:], op=AO.mult)
                nc.vector.tensor_tensor(out=beta1[:], in0=beta1[:], in1=alpha1[:], op=AO.mult)
                nc.vector.tensor_tensor(out=beta1[:], in0=csb['gn1_b16'][:], in1=beta1[:], op=AO.subtract)
                # apply
                for s in range(NSW):
                    B = sb1.tile([128, GMAX], f32, tag='B1b')
                    nc.vector.tensor_tensor(out=B[:], in0=segs[:, s:s + 1].to_broadcast([128, GMAX]),
                                            in1=iota[:, :GMAX], op=AO.is_equal)
                    bt_ps = ps1b.tile([GMAX, 128], f32, tag='bt')
                    nc.tensor.transpose(out=bt_ps[:], in_=B[:], identity=ident[:])
                    bt = sb1.tile([GMAX, 128], f32, tag='btsb')
                    nc.vector.tensor_copy(out=bt[:], in_=bt_ps[:])
                    ab_ps = ps1b.tile([128, 2 * D], f32, tag='ab')
                    nc.tensor.matmul(out=ab_ps[:, :D], lhsT=bt[:], rhs=alpha1[:], start=True, stop=False)
                    nc.tensor.matmul(out=ab_ps[:, D:], lhsT=bt[:], rhs=beta1[:], start=True, stop=True)
                    hnw = sb1.tile([128, D], f32, tag='hnw')
                    nc.vector.tensor_tensor(out=hnw[:], in0=hwins[s][:], in1=ab_ps[:, :D], op=AO.mult)
                    nc.vector.tensor_tensor(out=hnw[:], in0=hnw[:], in1=ab_ps[:, D:], op=AO.add)
                    nc.sync.dma_start(out=hn_local[s * 128:(s + 1) * 128, :], in_=hnw[:])

            nc.leave_named_scope('ph1_gn1', sc1[0], False)
            # allgather hn
            cc1 = nc.gpsimd.collective_compute(
                'AllGather', AO.bypass, replica_groups=[list(range(NCORES))],
                ins=[hn_local[:]], outs=[hn_full[:]])

            # =========== phase 2: relconv (fused QKV) ===========
            sc2 = nc.enter_named_scope('ph2_relconv', False)
            RGROUPS = [(0, [0, 1, 2, 3, 4]), (1, [5, 6, 7, 8])]
            with (
                tc.tile_pool(name='p2g', bufs=3) as gp2,
                tc.tile_pool(name='p2sb', bufs=3) as sb2,
                tc.tile_pool(name='p2S', bufs=1, space='PSUM') as psS,
                tc.tile_pool(name='p2qkv', bufs=1, space='PSUM') as psQ,
                tc.tile_pool(name='p2tr', bufs=1, space='PSUM') as psT,
            ):
                rc_off = 0   # chunk offset
                for w in range(NW):
                    qkv_ps = [psQ.tile([128, 3 * D], f32, tag=f'qkv{i}', name=f'qkv{i}') for i in range(2)]
                    for g, rels in RGROUPS:
                        Sg5 = [psS.tile([128, WIN], f32, tag=f'S{i}', name=f'S{i}') for i in range(len(rels))]
                        for hh in range(2):
                            seg_chunks = sum(
                                rc_chunks[((w * 2 + g) * 2 + hh) * REL + r] for r in rels)
                            co = rc_off
                            remaining = seg_chunks
                            base = BASE_LO if hh == 0 else BASE_HI
                            gtiles = {}
                            while remaining > 0:
                                take = min(remaining, CALL_MAX)
                                gt = gp2.tile([128, CALL_MAX * D], f32, tag='g')
                                gi = nc.gpsimd.dma_gather(
                                    out_ap=gt[:, :take * D].rearrange('p (c e) -> p c e', e=D),
                                    in_ap=hn_flat[base:NTAB, :],
                                    idxs_ap=rci[:, co * 8:(co + take) * 8],
                                    num_idxs=take * 128, num_idxs_reg=take * 128,
                                    elem_size=D)
                                add_dep_helper(gi.ins, cc1.ins, True,
                                               'gather reads allgathered hn')
                                for j in range(take):
                                    gtiles[co + j] = (gt, j)
                                co += take
                                remaining -= take
                            for ri, r in enumerate(rels):
                                nch = rc_chunks[((w * 2 + g) * 2 + hh) * REL + r]
                                for k in range(nch):
                                    ck = rc_off
                                    gt, j = gtiles[ck]
                                    A = sb2.tile([128, WIN], f32, tag='A')
                                    nc.vector.tensor_tensor(
                                        out=A[:], in0=rck[:, ck:ck + 1].to_broadcast([128, WIN]),
                                        in1=iota[:], op=AO.is_equal)
                                    nc.tensor.matmul(
                                        out=Sg5[ri][:], lhsT=gt[:, j * D:(j + 1) * D],
                                        rhs=A[:], start=(hh == 0 and k == 0),
                                        stop=(hh == 1 and k == nch - 1))
                                    rc_off += 1
                        # transforms for this relation group
                        for sub in range(2):
                            ssub = slice(sub * 128, (sub + 1) * 128)
                            for ri, r in enumerate(rels):
                                st = sb2.tile([128, 128], f32, tag='St')
                                nc.vector.tensor_copy(out=st[:], in_=Sg5[ri][:, ssub])
                                nc.tensor.matmul(out=qkv_ps[sub][:], lhsT=st[:],
                                                 rhs=wrel_sb[:, r * 3 * D:(r + 1) * 3 * D],
                                                 start=(g == 0 and ri == 0), stop=False)
                    # self-loop + bias + relu + store
                    for sub in range(2):
                        row0 = w * WIN + sub * 128
                        hnw = sb2.tile([128, D], f32, tag='hnl')
                        nc.sync.dma_start(out=hnw[:], in_=hn_local[row0:row0 + 128, :])
                        ht_ps = psT.tile([128, 128], f32, tag='ht')
                        nc.tensor.transpose(out=ht_ps[:], in_=hnw[:], identity=ident[:])
                        ht = sb2.tile([128, 128], f32, tag='htsb')
                        nc.vector.tensor_copy(out=ht[:], in_=ht_ps[:])
                        nc.tensor.matmul(out=qkv_ps[sub][:], lhsT=ht[:], rhs=wloop_sb[:],
                                         start=False, stop=True)
                        qkv_sb = sb2.tile([128, 3 * D], f32, tag='qkvsb')
                        nc.vector.tensor_tensor(out=qkv_sb[:], in0=qkv_ps[sub][:],
                                                in1=bqkv_sb[:], op=AO.add)
                        nc.scalar.activation(out=qkv_sb[:], in_=qkv_sb[:], func=AF.Relu)
                        nc.sync.dma_start(out=q_local[row0:row0 + 128, :], in_=qkv_sb[:, :D])
                        nc.sync.dma_start(out=kv_local[row0:row0 + 128, :], in_=qkv_sb[:, D:])

            nc.leave_named_scope('ph2_relconv', sc2[0], False)
            cc2 = nc.gpsimd.collective_compute(
                'AllGather', AO.bypass, replica_groups=[list(range(NCORES))],
                ins=[kv_local[:]], outs=[kv_full[:]])

            # =========== phase 3: attention + epilogue ===========
            sc3 = nc.enter_named_scope('ph3_attn', False)
            h1_cm = tc.tile_pool(name='h1', bufs=1)
            h1_pool = h1_cm.__enter__()
            h1t = []
            with (
                tc.tile_pool(name='p3g', bufs=3) as gp3,
                tc.tile_pool(name='p3sb', bufs=3) as sb3,
                tc.tile_pool(name='p3at', bufs=2, space='PSUM') as psA,
                tc.tile_pool(name='p3wv', bufs=1, space='PSUM') as psW,
                tc.tile_pool(name='p3ep', bufs=1, space='PSUM') as psE,
            ):
                at_off = 0
                for sw in range(NSW):
                    qwin = sb3.tile([128, D], f32, tag='qwin')
                    nc.sync.dma_start(out=qwin[:], in_=q_local[sw * 128:(sw + 1) * 128, :])
                    wvz = psW.tile([128, D + HEADS], f32, tag='wvz')
                    first = True
                    for hh in range(2):
                        nch = at_chunks[sw * 2 + hh]
                        base = BASE_LO if hh == 0 else BASE_HI
                        co = at_off
                        remaining = nch
                        gtiles = {}
                        while remaining > 0:
                            take = min(remaining, CALL_MAX)
                            gt = gp3.tile([128, CALL_MAX * 2 * D], f32, tag='ag')
                            gi = nc.gpsimd.dma_gather(
                                out_ap=gt[:, :take * 2 * D].rearrange('p (c e) -> p c e', e=2 * D),
                                in_ap=kv_flat[base:NTAB, :],
                                idxs_ap=ati[:, co * 8:(co + take) * 8],
                                num_idxs=take * 128, num_idxs_reg=take * 128,
                                elem_size=2 * D)
                            add_dep_helper(gi.ins, cc2.ins, True,
                                           'gather reads allgathered kv')
                            for j in range(take):
                                gtiles[co + j] = (gt, j)
                            co += take
                            remaining -= take
                        for k in range(nch):
                            ck = at_off
                            gt, j = gtiles[ck]
                            kv_c = gt[:, j * 2 * D:(j + 1) * 2 * D]
                            A = sb3.tile([128, 128], f32, tag='aA')
                            nc.vector.tensor_tensor(
                                out=A[:], in0=atk[:, ck:ck + 1].to_broadcast([128, 128]),
                                in1=iota[:, :128], op=AO.is_equal)
                            at_ps = psA.tile([128, 128], f32, tag='atp')
                            nc.tensor.transpose(out=at_ps[:], in_=A[:], identity=ident[:])
                            att = sb3.tile([128, 128], f32, tag='att')
                            nc.vector.tensor_copy(out=att[:], in_=at_ps[:])
                            qd_ps = psA.tile([128, D], f32, tag='qd')
                            nc.tensor.matmul(out=qd_ps[:], lhsT=att[:], rhs=qwin[:],
                                             start=True, stop=True)
                            kq = sb3.tile([128, D], f32, tag='kq')
                            nc.vector.tensor_tensor(out=kq[:], in0=kv_c[:, :D], in1=qd_ps[:], op=AO.mult)
                            sc = sb3.tile([128, HEADS], f32, tag='sc')
                            nc.vector.reduce_sum(
                                out=sc[:].rearrange('p (h o) -> p h o', o=1),
                                in_=kq[:].rearrange('p (h e) -> p h e', e=DH),
                                axis=mybir.AxisListType.X)
                            nc.vector.tensor_scalar(out=sc[:], in0=sc[:], scalar1=0.25,
                                                    scalar2=10.0, op0=AO.mult, op1=AO.min)
                            nc.vector.tensor_scalar_max(out=sc[:], in0=sc[:], scalar1=-10.0)
                            vse = sb3.tile([128, D + HEADS], f32, tag='vse')
                            nc.scalar.activation(out=vse[:, D:], in_=sc[:], func=AF.Exp)
                            nc.vector.tensor_tensor(
                                out=vse[:, :D].rearrange('p (h e) -> p h e', e=DH),
                                in0=kv_c[:].rearrange('p (x e) -> p x e', e=DH)[:, HEADS:, :],
                                in1=vse[:, D:].rearrange('p (h o) -> p h o', o=1).to_broadcast([128, HEADS, DH]),
                                op=AO.mult)
                            last = (hh == 1 and k == nch - 1)
                            nc.tensor.matmul(out=wvz[:], lhsT=A[:], rhs=vse[:],
                                             start=first, stop=last)
                            first = False
                            at_off += 1
                    # epilogue for this subwindow
                    zr = sb3.tile([128, HEADS], f32, tag='zr')
                    nc.vector.tensor_scalar_add(out=zr[:], in0=wvz[:, D:], scalar1=1e-6)
                    zrec = sb3.tile([128, HEADS], f32, tag='zrec')
                    nc.vector.reciprocal(out=zrec[:], in_=zr[:])
                    attn = sb3.tile([128, D], f32, tag='attn')
                    nc.vector.tensor_tensor(
                        out=attn[:].rearrange('p (h e) -> p h e', e=DH),
                        in0=wvz[:, :D].rearrange('p (h e) -> p h e', e=DH),
                        in1=zrec[:].rearrange('p (h o) -> p h o', o=1).to_broadcast([128, HEADS, DH]),
                        op=AO.mult)
                    atr_ps = psE.tile([128, D], f32, tag='atr')
                    nc.tensor.transpose(out=atr_ps[:], in_=attn[:], identity=ident[:])
                    atr = sb3.tile([128, D], f32, tag='atrsb')
                    nc.vector.tensor_copy(out=atr[:], in_=atr_ps[:])
                    ho_ps = psE.tile([128, D], f32, tag='ho')
                    nc.tensor.matmul(out=ho_ps[:], lhsT=atr[:], rhs=ow_sb[:], start=True, stop=True)
                    hob = sb3.tile([128, D], f32, tag='hob')
                    nc.vector.tensor_tensor(out=hob[:], in0=ho_ps[:], in1=ob_sb[:], op=AO.add)
                    # LN1
                    mu = sb3.tile([128, 1], f32, tag='mu')
                    nc.vector.reduce_sum(out=mu[:], in_=hob[:], axis=mybir.AxisListType.X)
                    nc.vector.tensor_scalar_mul(out=mu[:], in0=mu[:], scalar1=1.0 / D)
                    xc = sb3.tile([128, D], f32, tag='xc')
                    nc.vector.tensor_tensor(out=xc[:], in0=hob[:],
                                            in1=mu[:].to_broadcast([128, D]), op=AO.subtract)
                    sq = sb3.tile([128, D], f32, tag='sq')
                    nc.vector.tensor_tensor(out=sq[:], in0=xc[:], in1=xc[:], op=AO.mult)
                    vr = sb3.tile([128, 1], f32, tag='vr')
                    nc.vector.reduce_sum(out=vr[:], in_=sq[:], axis=mybir.AxisListType.X)
                    nc.vector.tensor_scalar(out=vr[:], in0=vr[:], scalar1=1.0 / D,
                                            scalar2=1e-5, op0=AO.mult, op1=AO.add)
                    sd = sb3.tile([128, 1], f32, tag='sd')
                    nc.scalar.activation(out=sd[:], in_=vr[:], func=AF.Sqrt)
                    rsd = sb3.tile([128, 1], f32, tag='rsd')
                    nc.vector.reciprocal(out=rsd[:], in_=sd[:])
                    h1 = h1_pool.tile([128, D], f32, tag=f'h1_{sw}', name=f'h1_{sw}')
                    nc.vector.tensor_tensor(out=h1[:], in0=xc[:],
                                            in1=rsd[:].to_broadcast([128, D]), op=AO.mult)
                    nc.vector.tensor_tensor(out=h1[:], in0=h1[:], in1=csb['ln1_g'][:], op=AO.mult)
                    nc.vector.tensor_tensor(out=h1[:], in0=h1[:], in1=csb['ln1_b'][:], op=AO.add)
                    h1t.append(h1)

            nc.leave_named_scope('ph3_attn', sc3[0], False)
            # =========== phase 4: graphNorm2 stats + finalize ===========
            sc4 = nc.enter_named_scope('ph45_tail', False)
            with (
                tc.tile_pool(name='p4keep', bufs=1) as keep4,
                tc.tile_pool(name='p4sb', bufs=3) as sb4,
                tc.tile_pool(name='p4ps', bufs=1, space='PSUM') as ps4,
                tc.tile_pool(name='p4ps2', bufs=1, space='PSUM') as ps4b,
            ):
                sum2 = ps4.tile([GMAX, D], f32, tag='sum2')
                sq2 = ps4.tile([GMAX, D], f32, tag='sq2')
                for s in range(NSW):
                    B = sb4.tile([128, GMAX], f32, tag='B2')
                    nc.vector.tensor_tensor(out=B[:], in0=segs[:, s:s + 1].to_broadcast([128, GMAX]),
                                            in1=iota[:, :GMAX], op=AO.is_equal)
                    hsq = sb4.tile([128, D], f32, tag='h2sq')
                    nc.vector.tensor_tensor(out=hsq[:], in0=h1t[s][:], in1=h1t[s][:], op=AO.mult)
                    nc.tensor.matmul(out=sum2[:], lhsT=B[:], rhs=h1t[s][:],
                                     start=(s == 0), stop=(s == NSW - 1))
                    nc.tensor.matmul(out=sq2[:], lhsT=B[:], rhs=hsq[:],
                                     start=(s == 0), stop=(s == NSW - 1))
                mean2 = keep4.tile([GMAX, D], f32)
                nc.vector.tensor_tensor(out=mean2[:], in0=sum2[:],
                                        in1=invc_sb[:].to_broadcast([GMAX, D]), op=AO.mult)
                ex22 = keep4.tile([GMAX, D], f32)
                nc.vector.tensor_tensor(out=ex22[:], in0=sq2[:],
                                        in1=invc_sb[:].to_broadcast([GMAX, D]), op=AO.mult)
                msq2 = keep4.tile([GMAX, D], f32)
                nc.vector.tensor_tensor(out=msq2[:], in0=mean2[:], in1=mean2[:], op=AO.mult)
                nc.vector.tensor_tensor(out=msq2[:], in0=msq2[:], in1=csb['gn2_msfac16'][:], op=AO.mult)
                var2 = keep4.tile([GMAX, D], f32)
                nc.vector.tensor_tensor(out=var2[:], in0=ex22[:], in1=msq2[:], op=AO.subtract)
                nc.vector.tensor_scalar_add(out=var2[:], in0=var2[:], scalar1=1e-6)
                std2 = keep4.tile([GMAX, D], f32)
                nc.scalar.activation(out=std2[:], in_=var2[:], func=AF.Sqrt)
                rstd2 = keep4.tile([GMAX, D], f32)
                nc.vector.reciprocal(out=rstd2[:], in_=std2[:])
                alpha2 = keep4.tile([GMAX, D], f32)
                nc.vector.tensor_tensor(out=alpha2[:], in0=rstd2[:], in1=csb['gn2_w16'][:], op=AO.mult)
                beta2 = keep4.tile([GMAX, D], f32)
                nc.vector.tensor_tensor(out=beta2[:], in0=mean2[:], in1=csb['gn2_ms16'][:], op=AO.mult)
                nc.vector.tensor_tensor(out=beta2[:], in0=beta2[:], in1=alpha2[:], op=AO.mult)
                nc.vector.tensor_tensor(out=beta2[:], in0=csb['gn2_b16'][:], in1=beta2[:], op=AO.subtract)

                # =========== phase 5: gn2 apply + FFN + LN2 + out ===========
                for s in range(NSW):
                    B = sb4.tile([128, GMAX], f32, tag='B3')
                    nc.vector.tensor_tensor(out=B[:], in0=segs[:, s:s + 1].to_broadcast([128, GMAX]),
                                            in1=iota[:, :GMAX], op=AO.is_equal)
                    bt_ps = ps4b.tile([GMAX, 128], f32, tag='bt2')
                    nc.tensor.transpose(out=bt_ps[:], in_=B[:], identity=ident[:])
                    bt = sb4.tile([GMAX, 128], f32, tag='bt2sb')
                    nc.vector.tensor_copy(out=bt[:], in_=bt_ps[:])
                    ab_ps = ps4b.tile([128, 2 * D], f32, tag='ab2')
                    nc.tensor.matmul(out=ab_ps[:, :D], lhsT=bt[:], rhs=alpha2[:], start=True, stop=False)
                    nc.tensor.matmul(out=ab_ps[:, D:], lhsT=bt[:], rhs=beta2[:], start=True, stop=True)
                    h2 = sb4.tile([128, D], f32, tag='h2')
                    nc.vector.tensor_tensor(out=h2[:], in0=h1t[s][:], in1=ab_ps[:, :D], op=AO.mult)
                    nc.vector.tensor_tensor(out=h2[:], in0=h2[:], in1=ab_ps[:, D:], op=AO.add)
                    h2t_ps = ps4b.tile([128, D], f32, tag='h2t')
                    nc.tensor.transpose(out=h2t_ps[:], in_=h2[:], identity=ident[:])
                    h2tt = sb4.tile([128, D], f32, tag='h2tsb')
                    nc.vector.tensor_copy(out=h2tt[:], in_=h2t_ps[:])
                    f1_ps = ps4b.tile([128, 2 * D], f32, tag='f1')
                    nc.tensor.matmul(out=f1_ps[:], lhsT=h2tt[:], rhs=ffn1_sb[:], start=True, stop=True)
                    fr = sb4.tile([128, 2 * D], f32, tag='fr')
                    nc.vector.tensor_tensor(out=fr[:], in0=f1_ps[:], in1=ffn1b_sb[:], op=AO.add)
                    nc.scalar.activation(out=fr[:], in_=fr[:], func=AF.Relu)
                    frt_ps = ps4b.tile([128, 2 * D], f32, tag='frt')
                    nc.tensor.transpose(out=frt_ps[:, :D], in_=fr[:, :D], identity=ident[:])
                    nc.tensor.transpose(out=frt_ps[:, D:], in_=fr[:, D:], identity=ident[:])
                    frt = sb4.tile([128, 2 * D], f32, tag='frtsb')
                    nc.vector.tensor_copy(out=frt[:], in_=frt_ps[:])
                    h3_ps = ps4b.tile([128, D], f32, tag='h3')
                    nc.tensor.matmul(out=h3_ps[:], lhsT=frt[:, :D], rhs=ffn2_sb[:, :D], start=True, stop=False)
                    nc.tensor.matmul(out=h3_ps[:], lhsT=frt[:, D:], rhs=ffn2_sb[:, D:], start=False, stop=True)
                    h3b = sb4.tile([128, D], f32, tag='h3b')
                    nc.vector.tensor_tensor(out=h3b[:], in0=h3_ps[:], in1=ffn2b_sb[:], op=AO.add)
                    # LN2
                    mu = sb4.tile([128, 1], f32, tag='mu2')
                    nc.vector.reduce_sum(out=mu[:], in_=h3b[:], axis=mybir.AxisListType.X)
                    nc.vector.tensor_scalar_mul(out=mu[:], in0=mu[:], scalar1=1.0 / D)
                    xc = sb4.tile([128, D], f32, tag='xc2')
                    nc.vector.tensor_tensor(out=xc[:], in0=h3b[:],
                                            in1=mu[:].to_broadcast([128, D]), op=AO.subtract)
                    sq = sb4.tile([128, D], f32, tag='sq2w')
                    nc.vector.tensor_tensor(out=sq[:], in0=xc[:], in1=xc[:], op=AO.mult)
                    vr = sb4.tile([128, 1], f32, tag='vr2')
                    nc.vector.reduce_sum(out=vr[:], in_=sq[:], axis=mybir.AxisListType.X)
                    nc.vector.tensor_scalar(out=vr[:], in0=vr[:], scalar1=1.0 / D,
                                            scalar2=1e-5, op0=AO.mult, op1=AO.add)
                    sd = sb4.tile([128, 1], f32, tag='sd2')
                    nc.scalar.activation(out=sd[:], in_=vr[:], func=AF.Sqrt)
                    rsd = sb4.tile([128, 1], f32, tag='rsd2')
                    nc.vector.reciprocal(out=rsd[:], in_=sd[:])
                    ov = sb4.tile([128, D], f32, tag='ov')
                    nc.vector.tensor_tensor(out=ov[:], in0=xc[:],
                                            in1=rsd[:].to_broadcast([128, D]), op=AO.mult)
                    nc.vector.tensor_tensor(out=ov[:], in0=ov[:], in1=csb['ln2_g'][:], op=AO.mult)
                    nc.vector.tensor_tensor(out=ov[:], in0=ov[:], in1=csb['ln2_b'][:], op=AO.add)
                    nc.sync.dma_start(out=out_sl[s * 128:(s + 1) * 128, :], in_=ov[:])
            nc.leave_named_scope('ph45_tail', sc4[0], False)
            h1_cm.__exit__(None, None, None)
            if debug:
                nc.sync.dma_start(out=hn_dbg[:], in_=hn_local[:])
                nc.sync.dma_start(out=hnf_dbg[:], in_=hn_flat[:])
                nc.sync.dma_start(out=q_dbg[:], in_=q_local[:])
                nc.sync.dma_start(out=kv_dbg[:], in_=kv_local[:])

    nc.finalize()
    return nc


def kernel(**inputs) -> np.ndarray:
    _ensure_hooks()
    from concourse.bass_utils import run_bass_kernel_spmd

    static, in_maps, meta = preprocess(inputs)
    key = tuple(sorted((k, v) for k, v in static.items()))
    if key not in _PROGRAM_CACHE:
        _PROGRAM_CACHE[key] = build_program(static)
    nc = _PROGRAM_CACHE[key]

    trace = os.environ.get("KERNEL_TRACE") == "1"
    res = run_bass_kernel_spmd(nc, in_maps, list(range(NCORES)), trace=trace)
    if trace and res.exec_time_ns:
        print("HW exec time:", res.exec_time_ns, "ns")
    out = np.zeros((N_NODES, D), np.float32)
    for c in range(NCORES):
        n0, n1 = int(meta['n0'][c]), int(meta['n1'][c])
        out[n0:n1] = res.results[c]['out_slice'][:n1 - n0]
    return out



# revision 10
# speedup vs baseline: 1.1389x; 1.1389x over previous
"""GTLayer (relational graph transformer layer) on 8 Trainium2 NeuronCores.

v2 strategy (see kernel_v1_baseline.py for the original):
- Nodes partitioned across 8 cores in graph-aligned contiguous slices;
  edges live with the core owning dst. Global gather tables laid out in 4
  row-pieces so allgathers can be fired piecewise and overlapped.
- fp16 data lane for relconv (hn table, gathered rows, W_rel/W_loop, S
  sums), bf16 lane for attention (kv table, one-hots, V*exp) — exp(score)
  products can exceed fp16 range. PSUM accumulation is always f32.
- RelConv: edges per (dst-window 256, src-half, rel-PAIR); S psum tiles are
  [128, 512] covering two relations (one-hot column = dl + 256*parity), so
  all 9 relations fit one PSUM residency and gather call regions merge to
  (window, half).
- One-hots built with tensor_scalar(is_equal, scalar1=key column) against
  an iota ramp (2x DVE mode), not tensor_tensor broadcasts.
- Attention: sw-blocks of 4 subwindows share gather calls; chunks
  processed in batches of <=4 with batched vector ops.
- Scalar engine does psum->sbuf copies (Copy), Relu/Exp/Square, arranged
  so activation tables almost never reload. LN1/LN2 sqrt is batched
  (columns collected across subwindows, one Sqrt instruction each).
- Biases applied as K=1 matmuls (ones-row lhsT) accumulating into PSUM.
- LN1 affine folded into graphNorm2's alpha/beta algebra.
"""
import os
import sys
import types
import numpy as np

NCORES = 8
N_NODES = 100000
N_EDGES = 600000
D = 128
REL = 9
NPAIR = 5
NG = 64
HEADS = 8
DH = 16
WIN = 256          # relconv dst window
GMAX = 16          # max graphs per core
CALL_MAX = 8       # max chunks (of 128 slots) per dma_gather call
PIECES = 4         # allgather pieces; also gather base regions (idx >= 0 always)
NBATCH = 4         # attention chunk batch


def _ensure_hooks():
    if "antenv.axon_hooks" not in sys.modules:
        hooks = types.ModuleType("antenv.axon_hooks")
        h = [None]
        hooks.set_axon_ntff_profile_hook = lambda v: h.__setitem__(0, v)
        hooks.get_axon_ntff_profile_hook = lambda: h[0]
        sys.modules["antenv.axon_hooks"] = hooks
        try:
            from trn_agent_boot.trn_boot import _ntff_profile_via_ctypes
            hooks.set_axon_ntff_profile_hook(
                _ntff_profile_via_ctypes("/opt/axon/libaxon_pjrt.so"))
        except Exception:
            pass


# ----------------------------------------------------------------------------
# Host preprocessing
# ----------------------------------------------------------------------------

def _pack_idx16(idx):
    """int16 index array -> [128, n/16] wrapped+replicated layout."""
    n = len(idx)
    assert n % 16 == 0
    blk = idx.reshape(n // 16, 16).T
    return np.tile(blk, (8, 1)).astype(np.int16)


def _layout_slots(order_edges, idx_vals, key_vals, n_chunks):
    """Place edges into n_chunks*128 slots (full 128 per chunk), pads get
    idx 0, key -1. Returns (idx int32, key f32)."""
    tot = n_chunks * 128
    idx = np.zeros(tot, np.int32)
    key = np.full(tot, -1.0, np.float32)
    ne = len(order_edges)
    assert ne <= tot, (ne, n_chunks)
    idx[:ne] = idx_vals[order_edges]
    key[:ne] = key_vals[order_edges]
    return idx, key


def _calls_for(n, cap):
    calls = []
    n = int(n)
    while n > 0:
        take = min(n, cap)
        calls.append(take)
        n -= take
    return calls


def preprocess(inputs):
    import ml_dtypes
    bf16 = ml_dtypes.bfloat16
    h = np.asarray(inputs['h'], np.float32)
    src = np.asarray(inputs['src']).astype(np.int64)
    dst = np.asarray(inputs['dst']).astype(np.int64)
    et = np.asarray(inputs['etypes']).astype(np.int64)
    seg = np.asarray(inputs['seg']).astype(np.int64)

    # --- graph-aligned node partition ---
    gstart = np.searchsorted(seg, np.arange(NG + 1))
    bounds = [0]
    for c in range(1, NCORES):
        target = c * N_NODES / NCORES
        g = int(np.argmin(np.abs(gstart - target)))
        bounds.append(int(gstart[g]))
    bounds.append(N_NODES)
    n0 = np.array(bounds[:-1]); n1 = np.array(bounds[1:])
    sizes = n1 - n0
    ROUND = max(512, WIN * PIECES)
    P_NODES = int(np.ceil(sizes.max() / ROUND) * ROUND)
    RPP = P_NODES // PIECES
    NW = P_NODES // WIN
    NSW = NW * 2
    assert NSW % 4 == 0
    NSWB = NSW // 4
    NTAB = NCORES * P_NODES
    assert NCORES * RPP <= 32768  # per-piece base region fits int16 idx

    owner = np.searchsorted(n1, np.arange(N_NODES), side='right')
    rloc = np.arange(N_NODES) - n0[owner]
    piece = rloc // RPP
    gpos = piece * (NCORES * RPP) + owner * RPP + (rloc - piece * RPP)

    g0 = np.searchsorted(gstart, n0, side='right') - 1
    counts_g = np.diff(gstart).astype(np.float32)

    BASEQ = NCORES * RPP
    srcp = gpos[src]
    half = srcp // BASEQ          # source piece = gather base region (0..3)
    ecore = owner[dst]
    dst_off = dst - n0[ecore]
    w_e = dst_off // WIN
    pair_e = et // 2
    par_e = et % 2
    key512 = (dst_off % WIN + 256 * par_e).astype(np.float32)
    sw_e = dst_off // 128
    dl128 = (dst_off % 128).astype(np.float32)
    swb_e = sw_e // 4
    swin_e = sw_e % 4
    idx_rel = (srcp - half * BASEQ).astype(np.int32)
    assert idx_rel.min() >= 0 and idx_rel.max() < 32768

    # --- relconv chunk structure: groups q = (w, pc, pair) ---
    NQ = NW * PIECES * NPAIR
    rkey = (w_e * PIECES + half) * NPAIR + pair_e
    rc_counts = np.zeros((NCORES, NQ), np.int64)
    for c in range(NCORES):
        rc_counts[c] = np.bincount(rkey[ecore == c], minlength=NQ)
    rc_chunks = np.ceil(rc_counts / 128.0).max(0).astype(np.int64)
    # ensure >=1 chunk per (w, pair) so S psum gets initialized
    for w in range(NW):
        for p in range(NPAIR):
            qs = [(w * PIECES + pc) * NPAIR + p for pc in range(PIECES)]
            if sum(rc_chunks[q] for q in qs) == 0:
                rc_chunks[qs[0]] = 1
    RC_CHUNKS = int(rc_chunks.sum())
    # call list per (w, pc)
    rc_calls = []
    for w in range(NW):
        for pc in range(PIECES):
            tot = int(sum(rc_chunks[(w * PIECES + pc) * NPAIR + p] for p in range(NPAIR)))
            rc_calls.append(tuple(_calls_for(tot, CALL_MAX)))

    # --- attention chunk structure: groups aq = (swb, pc, swin) ---
    NAQ = NSWB * PIECES * 4
    akey = (swb_e * PIECES + half) * 4 + swin_e
    at_counts = np.zeros((NCORES, NAQ), np.int64)
    for c in range(NCORES):
        at_counts[c] = np.bincount(akey[ecore == c], minlength=NAQ)
    at_chunks = np.ceil(at_counts / 128.0).max(0).astype(np.int64)
    # ensure every sw has >=1 chunk overall (wvz psum init)
    for sw in range(NSW):
        swb, swin = sw // 4, sw % 4
        aqs = [(swb * PIECES + pc) * 4 + swin for pc in range(PIECES)]
        if sum(at_chunks[a] for a in aqs) == 0:
            at_chunks[aqs[0]] = 1
    AT_CHUNKS = int(at_chunks.sum())
    at_calls = []
    for swb in range(NSWB):
        for pc in range(PIECES):
            tot = int(sum(at_chunks[(swb * PIECES + pc) * 4 + s] for s in range(4)))
            at_calls.append(tuple(_calls_for(tot, CALL_MAX)))

    # --- per-core data arrays ---
    in_maps = []
    for c in range(NCORES):
        m = np.nonzero(ecore == c)[0]
        # order by (group, srcp) for gather locality
        order = np.lexsort((srcp[m], rkey[m]))
        es = m[order]
        rk = rkey[m][order]
        run_s = np.searchsorted(rk, np.arange(NQ))
        run_e = np.searchsorted(rk, np.arange(NQ) + 1)
        rc_idx = np.zeros(RC_CHUNKS * 128, np.int32)
        rc_key = np.full(RC_CHUNKS * 128, -1.0, np.float32)
        coff = 0
        for q in range(NQ):
            nch = int(rc_chunks[q])
            if nch == 0:
                assert run_e[q] == run_s[q]
                continue
            ii, kk = _layout_slots(es[run_s[q]:run_e[q]], idx_rel, key512, nch)
            rc_idx[coff * 128:(coff + nch) * 128] = ii
            rc_key[coff * 128:(coff + nch) * 128] = kk
            coff += nch
        assert coff == RC_CHUNKS

        aorder = np.lexsort((srcp[m], akey[m]))
        aes = m[aorder]
        ak = akey[m][aorder]
        arun_s = np.searchsorted(ak, np.arange(NAQ))
        arun_e = np.searchsorted(ak, np.arange(NAQ) + 1)
        at_idx = np.zeros(AT_CHUNKS * 128, np.int32)
        at_key = np.full(AT_CHUNKS * 128, -1.0, np.float32)
        coff = 0
        for q in range(NAQ):
            nch = int(at_chunks[q])
            if nch == 0:
                assert arun_e[q] == arun_s[q]
                continue
            ii, kk = _layout_slots(aes[arun_s[q]:arun_e[q]], idx_rel, dl128, nch)
            at_idx[coff * 128:(coff + nch) * 128] = ii
            at_key[coff * 128:(coff + nch) * 128] = kk
            coff += nch
        assert coff == AT_CHUNKS

        hs = np.zeros((P_NODES, D), np.float32)
        hs[:sizes[c]] = h[n0[c]:n1[c]]
        segl = np.full(P_NODES, -1.0, np.float32)
        segl[:sizes[c]] = (seg[n0[c]:n1[c]] - g0[c]).astype(np.float32)
        ginc = np.zeros((GMAX, 1), np.float32)
        ng_c = int(seg[n1[c] - 1] - g0[c]) + 1
        assert ng_c <= GMAX
        ginc[:ng_c, 0] = 1.0 / counts_g[g0[c]:g0[c] + ng_c]

        im = {
            'h_slice': hs.astype(np.float16),
            'seg_col': segl.reshape(NSW, 128).T.copy(),
            'inv_cnt': ginc,
            'rc_idx': _pack_idx16(rc_idx.astype(np.int16)),
            'rc_key': rc_key.reshape(RC_CHUNKS, 128).T.copy(),
            'at_idx': _pack_idx16(at_idx.astype(np.int16)),
            'at_key': at_key.reshape(AT_CHUNKS, 128).T.copy(),
        }
        in_maps.append(im)

    # --- shared weights ---
    def A(x):
        return np.ascontiguousarray(np.asarray(x, np.float32))
    Wrel = np.concatenate([
        np.einsum('rb,bio->rio', A(inputs[f'{nm}_coeff']), A(inputs[f'{nm}_basis']))
        for nm in ('q', 'k', 'v')], axis=2)            # [9, 128, 384]
    Wrel[:, :, :D] *= 0.25  # fold score/sqrt(dh) into Q
    wloop = np.concatenate([A(inputs[f'{nm}_loop']) for nm in ('q', 'k', 'v')], 1)
    wloop[:, :D] *= 0.25
    bqkv = np.concatenate([A(inputs[f'{nm}_bias']) for nm in ('q', 'k', 'v')])
    bqkv[:D] *= 0.25
    ffn2p = np.zeros((D, 2 * D), np.float32)           # two K-chunks side by side
    ffn2p[:, :D] = A(inputs['ffn2_w'])[:D, :]
    ffn2p[:, D:] = A(inputs['ffn2_w'])[D:, :]
    w_shared = {
        'w_rel': A(Wrel.reshape(REL * D, 3 * D)).astype(np.float16),
        'w_loop': wloop.astype(np.float16),
        'brow_qkv': bqkv.reshape(1, 3 * D).astype(np.float16),
        'o_w': A(inputs['o_w']).astype(bf16),
        'brow_o': A(inputs['o_b']).reshape(1, D).astype(bf16),
        'ffn1': A(inputs['ffn1_w']).astype(np.float16),
        'brow_f1': A(inputs['ffn1_b']).reshape(1, 2 * D).astype(np.float16),
        'ffn2': ffn2p.astype(np.float16),
        'brow_f2': A(inputs['ffn2_b']).reshape(1, D).astype(np.float16),
        'ln2_g': np.tile(A(inputs['ln2_g'])[None, :], (128, 1)),
        'ln2_b': np.tile(A(inputs['ln2_b'])[None, :], (128, 1)),
    }
    for nm in ('gn1', 'gn2'):
        w = A(inputs[f'{nm}_w']); b = A(inputs[f'{nm}_b']); ms = A(inputs[f'{nm}_ms'])
        w_shared[f'{nm}_w16'] = np.tile(w[None, :], (GMAX, 1))
        w_shared[f'{nm}_b16'] = np.tile(b[None, :], (GMAX, 1))
        w_shared[f'{nm}_ms16'] = np.tile(ms[None, :], (GMAX, 1))
        w_shared[f'{nm}_msfac16'] = np.tile((ms * (2 - ms))[None, :], (GMAX, 1))
    for nm in ('ln1_g', 'ln1_b'):
        w_shared[f'{nm}16'] = np.tile(A(inputs[nm])[None, :], (GMAX, 1))
    for im in in_maps:
        im.update(w_shared)

    static = dict(P_NODES=P_NODES, NW=NW, NSW=NSW, NSWB=NSWB, RPP=RPP,
                  NTAB=NTAB,
                  rc_chunks=tuple(int(x) for x in rc_chunks),
                  at_chunks=tuple(int(x) for x in at_chunks),
                  rc_calls=tuple(rc_calls), at_calls=tuple(at_calls),
                  RC_CHUNKS=RC_CHUNKS, AT_CHUNKS=AT_CHUNKS)
    meta = dict(n0=n0, n1=n1, sizes=sizes)
    return static, in_maps, meta


# ----------------------------------------------------------------------------
# Bass program
# ----------------------------------------------------------------------------

_PROGRAM_CACHE = {}


def build_program(st):
    import concourse.bass as bass
    import concourse.bacc as bacc
    import concourse.mybir as mybir
    import concourse.tile as tile
    from concourse.tile import TileContext
    from concourse.masks import make_identity
    from bass_rust import add_dep_helper

    P_NODES = st['P_NODES']; NW = st['NW']; NSW = st['NSW']
    NSWB = st['NSWB']; RPP = st['RPP']; NTAB = st['NTAB']
    BASEQ = NCORES * RPP
    rc_chunks = st['rc_chunks']; at_chunks = st['at_chunks']
    rc_calls = st['rc_calls']; at_calls = st['at_calls']
    RC_CHUNKS = st['RC_CHUNKS']; AT_CHUNKS = st['AT_CHUNKS']
    f32 = mybir.dt.float32
    f16 = mybir.dt.float16
    b16 = mybir.dt.bfloat16
    i16 = mybir.dt.int16
    AO = mybir.AluOpType
    AF = mybir.ActivationFunctionType

    nc = bacc.Bacc()

    # --- I/O ---
    h_slice = nc.declare_dram_parameter('h_slice', [P_NODES, D], f16, isOutput=False)
    seg_col = nc.declare_dram_parameter('seg_col', [128, NSW], f32, isOutput=False)
    inv_cnt = nc.declare_dram_parameter('inv_cnt', [GMAX, 1], f32, isOutput=False)
    rc_idx = nc.declare_dram_parameter('rc_idx', [128, RC_CHUNKS * 8], i16, isOutput=False)
    rc_keyd = nc.declare_dram_parameter('rc_key', [128, RC_CHUNKS], f32, isOutput=False)
    at_idx = nc.declare_dram_parameter('at_idx', [128, AT_CHUNKS * 8], i16, isOutput=False)
    at_keyd = nc.declare_dram_parameter('at_key', [128, AT_CHUNKS], f32, isOutput=False)
    w_rel = nc.declare_dram_parameter('w_rel', [REL * D, 3 * D], f16, isOutput=False)
    w_loop = nc.declare_dram_parameter('w_loop', [D, 3 * D], f16, isOutput=False)
    brow_qkv = nc.declare_dram_parameter('brow_qkv', [1, 3 * D], f16, isOutput=False)
    o_w = nc.declare_dram_parameter('o_w', [D, D], b16, isOutput=False)
    brow_o = nc.declare_dram_parameter('brow_o', [1, D], b16, isOutput=False)
    ffn1 = nc.declare_dram_parameter('ffn1', [D, 2 * D], f16, isOutput=False)
    brow_f1 = nc.declare_dram_parameter('brow_f1', [1, 2 * D], f16, isOutput=False)
    ffn2 = nc.declare_dram_parameter('ffn2', [D, 2 * D], f16, isOutput=False)
    brow_f2 = nc.declare_dram_parameter('brow_f2', [1, D], f16, isOutput=False)
    cdecl = {}
    for nm in ('ln2_g', 'ln2_b'):
        cdecl[nm] = nc.declare_dram_parameter(nm, [128, D], f32, isOutput=False)
    for nm in ('gn1_w16', 'gn1_b16', 'gn1_ms16', 'gn1_msfac16',
               'gn2_w16', 'gn2_b16', 'gn2_ms16', 'gn2_msfac16',
               'ln1_g16', 'ln1_b16'):
        cdecl[nm] = nc.declare_dram_parameter(nm, [GMAX, D], f32, isOutput=False)
    out_sl = nc.declare_dram_parameter('out_slice', [P_NODES, D], f32, isOutput=True)

    # --- internal DRAM ---
    hn_local = nc.dram_tensor('hn_local', [P_NODES, D], f16)
    q_local = nc.dram_tensor('q_local', [P_NODES, D], b16)
    kv_local = nc.dram_tensor('kv_local', [P_NODES, 2 * D], b16)
    hn_full = nc.dram_tensor('hn_full', [NTAB, D], f16, addr_space='Shared')
    kv_full = nc.dram_tensor('kv_full', [NTAB, 2 * D], b16, addr_space='Shared')
    debug = os.environ.get('KERNEL_DEBUG') == '1'
    if debug:
        hn_dbg = nc.declare_dram_parameter('hn_dbg', [P_NODES, D], f16, isOutput=True)
        kv_dbg = nc.declare_dram_parameter('kv_dbg', [P_NODES, 2 * D], b16, isOutput=True)

    with TileContext(nc) as tc:
        with tc.tile_pool(name='const', bufs=1) as cpool:
            iota = cpool.tile([128, 2 * WIN], f16)
            nc.gpsimd.iota(iota[:], pattern=[[1, 2 * WIN]], base=0,
                           channel_multiplier=0, allow_small_or_imprecise_dtypes=True)
            iota_b = cpool.tile([128, 128], b16)
            nc.gpsimd.iota(iota_b[:], pattern=[[1, 128]], base=0,
                           channel_multiplier=0, allow_small_or_imprecise_dtypes=True)
            ident_f = cpool.tile([128, 128], f16)
            make_identity(nc, ident_f[:])
            ident_b = cpool.tile([128, 128], b16)
            make_identity(nc, ident_b[:])
            ones_f = cpool.tile([1, 128], f16)
            nc.gpsimd.memset(ones_f[:], 1.0)
            ones_b = cpool.tile([1, 128], b16)
            nc.gpsimd.memset(ones_b[:], 1.0)

            segs = cpool.tile([128, NSW], f32)
            nc.sync.dma_start(out=segs[:], in_=seg_col[:])
            rck = cpool.tile([128, RC_CHUNKS], f32)
            nc.sync.dma_start(out=rck[:], in_=rc_keyd[:])
            atk = cpool.tile([128, AT_CHUNKS], f32)
            nc.sync.dma_start(out=atk[:], in_=at_keyd[:])

            wrel_sb = cpool.tile([128, REL * 3 * D], f16)
            for r in range(REL):
                nc.sync.dma_start(out=wrel_sb[:, r * 3 * D:(r + 1) * 3 * D],
                                  in_=w_rel[r * D:(r + 1) * D, :])
            wloop_sb = cpool.tile([128, 3 * D], f16)
            nc.sync.dma_start(out=wloop_sb[:], in_=w_loop[:])
            brqkv_sb = cpool.tile([1, 3 * D], f16)
            nc.sync.dma_start(out=brqkv_sb[:], in_=brow_qkv[:])
            ow_sb = cpool.tile([D, D], b16)
            nc.sync.dma_start(out=ow_sb[:], in_=o_w[:])
            bro_sb = cpool.tile([1, D], b16)
            nc.sync.dma_start(out=bro_sb[:], in_=brow_o[:])
            ffn1_sb = cpool.tile([D, 2 * D], f16)
            nc.sync.dma_start(out=ffn1_sb[:], in_=ffn1[:])
            brf1_sb = cpool.tile([1, 2 * D], f16)
            nc.sync.dma_start(out=brf1_sb[:], in_=brow_f1[:])
            ffn2_sb = cpool.tile([D, 2 * D], f16)
            nc.sync.dma_start(out=ffn2_sb[:], in_=ffn2[:])
            brf2_sb = cpool.tile([1, D], f16)
            nc.sync.dma_start(out=brf2_sb[:], in_=brow_f2[:])
            csb = {}
            for nm, dd in cdecl.items():
                t = cpool.tile(list(dd.shape), f32, tag=f'c_{nm}')
                nc.sync.dma_start(out=t[:], in_=dd[:])
                csb[nm] = t
            invc_sb = cpool.tile([GMAX, 1], f32)
            nc.sync.dma_start(out=invc_sb[:], in_=inv_cnt[:])

            # batched LN stat columns
            ln1_ns = cpool.tile([128, NSW], f32, tag='ln1_ns')
            ln1_sq = cpool.tile([128, NSW], f32, tag='ln1_sq')
            ln1_rsd = cpool.tile([128, NSW], f32, tag='ln1_rsd')
            ln1_nmr = cpool.tile([128, NSW], f32, tag='ln1_nmr')
            ln2_vr = cpool.tile([128, NSW], f32, tag='ln2_vr')
            ln2_rsd = cpool.tile([128, NSW], f32, tag='ln2_rsd')

            # persistent pools
            bpool_cm = tc.tile_pool(name='bpool', bufs=1)
            bpool = bpool_cm.__enter__()
            hobp_cm = tc.tile_pool(name='hobp', bufs=1)
            hobp = hobp_cm.__enter__()
            xc2p_cm = tc.tile_pool(name='xc2p', bufs=1)
            xc2p = xc2p_cm.__enter__()
            k4 = tc.tile_pool(name='p4keep', bufs=1)
            keep4 = k4.__enter__()
            Bt = []      # [128, GMAX] f16 one-hot per subwindow
            hobt = []    # [128, D] f16 attn-out per subwindow
            xc2t = []    # [128, D] f16 LN2-centered per subwindow

            # =========== phase 1: graphNorm1 ===========
            sc1 = nc.enter_named_scope('ph1_gn1', False)
            cc_hn = []
            with (
                tc.tile_pool(name='p1keep', bufs=1) as keep1,
                tc.tile_pool(name='p1sb', bufs=3) as sb1,
                tc.tile_pool(name='p1ps', bufs=1, space='PSUM') as ps1,
                tc.tile_pool(name='p1ps2', bufs=2, space='PSUM') as ps1b,
            ):
                sum_ps = ps1.tile([GMAX, D], f32, tag='sums')
                sq_ps = ps1.tile([GMAX, D], f32, tag='sqs')
                for s in range(NSW):
                    hw = sb1.tile([128, D], f16, tag='h_in')
                    nc.sync.dma_start(out=hw[:], in_=h_slice[s * 128:(s + 1) * 128, :])
                    B = bpool.tile([128, GMAX], f16, tag=f'B_{s}', name=f'B_{s}')
                    nc.vector.tensor_scalar(out=B[:], in0=iota[:, :GMAX],
                                            scalar1=segs[:, s:s + 1], scalar2=None,
                                            op0=AO.is_equal)
                    hsq = sb1.tile([128, D], f16, tag='hsq')
                    nc.scalar.activation(out=hsq[:], in_=hw[:], func=AF.Square)
                    nc.tensor.matmul(out=sum_ps[:], lhsT=B[:], rhs=hw[:],
                                     start=(s == 0), stop=(s == NSW - 1))
                    nc.tensor.matmul(out=sq_ps[:], lhsT=B[:], rhs=hsq[:],
                                     start=(s == 0), stop=(s == NSW - 1))
                    Bt.append(B)
                # finalize -> alpha/beta [GMAX, D] f32
                mean = keep1.tile([GMAX, D], f32)
                nc.vector.tensor_tensor(out=mean[:], in0=sum_ps[:],
                                        in1=invc_sb[:].to_broadcast([GMAX, D]), op=AO.mult)
                ex2 = keep1.tile([GMAX, D], f32)
                nc.vector.tensor_tensor(out=ex2[:], in0=sq_ps[:],
                                        in1=invc_sb[:].to_broadcast([GMAX, D]), op=AO.mult)
                msq = keep1.tile([GMAX, D], f32)
                nc.vector.tensor_tensor(out=msq[:], in0=mean[:], in1=mean[:], op=AO.mult)
                nc.vector.tensor_tensor(out=msq[:], in0=msq[:], in1=csb['gn1_msfac16'][:], op=AO.mult)
                var = keep1.tile([GMAX, D], f32)
                nc.vector.tensor_tensor(out=var[:], in0=ex2[:], in1=msq[:], op=AO.subtract)
                nc.vector.tensor_scalar_add(out=var[:], in0=var[:], scalar1=1e-6)
                std = keep1.tile([GMAX, D], f32)
                nc.scalar.activation(out=std[:], in_=var[:], func=AF.Sqrt)
                rstd = keep1.tile([GMAX, D], f32)
                nc.vector.reciprocal(out=rstd[:], in_=std[:])
                alpha1 = keep1.tile([GMAX, D], f32)
                nc.vector.tensor_tensor(out=alpha1[:], in0=rstd[:], in1=csb['gn1_w16'][:], op=AO.mult)
                beta1 = keep1.tile([GMAX, D], f32)
                nc.vector.tensor_tensor(out=beta1[:], in0=mean[:], in1=csb['gn1_ms16'][:], op=AO.mult)
                nc.vector.tensor_tensor(out=beta1[:], in0=beta1[:], in1=alpha1[:], op=AO.mult)
                nc.vector.tensor_tensor(out=beta1[:], in0=csb['gn1_b16'][:], in1=beta1[:], op=AO.subtract)
                a1b1 = keep1.tile([GMAX, 2 * D], f16)
                nc.vector.tensor_copy(out=a1b1[:, :D], in_=alpha1[:])
                nc.vector.tensor_copy(out=a1b1[:, D:], in_=beta1[:])
                # apply
                piece_stores = []
                for s in range(NSW):
                    btp = ps1b.tile([GMAX, 128], f16, tag='btp')
                    nc.tensor.transpose(out=btp[:], in_=Bt[s][:], identity=ident_f[:])
                    bts = sb1.tile([GMAX, 128], f16, tag='bts')
                    nc.scalar.copy(out=bts[:], in_=btp[:])
                    ab_ps = ps1b.tile([128, 2 * D], f32, tag='ab')
                    nc.tensor.matmul(out=ab_ps[:], lhsT=bts[:], rhs=a1b1[:],
                                     start=True, stop=True)
                    hw2 = sb1.tile([128, D], f16, tag='h_in2')
                    nc.sync.dma_start(out=hw2[:], in_=h_slice[s * 128:(s + 1) * 128, :])
                    hnw = sb1.tile([128, D], f16, tag='hnw')
                    nc.vector.tensor_tensor(out=hnw[:], in0=hw2[:], in1=ab_ps[:, :D], op=AO.mult)
                    nc.vector.tensor_tensor(out=hnw[:], in0=hnw[:], in1=ab_ps[:, D:], op=AO.add)
                    stin = nc.sync.dma_start(out=hn_local[s * 128:(s + 1) * 128, :], in_=hnw[:])
                    piece_stores.append(stin)
                    if (s + 1) % (NSW // PIECES) == 0:
                        p = (s + 1) // (NSW // PIECES) - 1
                        cc = nc.gpsimd.collective_compute(
                            'AllGather', AO.bypass,
                            replica_groups=[list(range(NCORES))],
                            ins=[hn_local[p * RPP:(p + 1) * RPP, :]],
                            outs=[hn_full[p * NCORES * RPP:(p + 1) * NCORES * RPP, :]
                                  .rearrange('(c r) d -> c r d', c=NCORES)])
                        for stx in piece_stores:
                            add_dep_helper(cc.ins, stx.ins, True, 'allgather reads hn piece')
                        piece_stores = []
                        cc_hn.append(cc)
            nc.leave_named_scope('ph1_gn1', sc1[0], False)

            # =========== phase 2: relconv (fused QKV) ===========
            sc2 = nc.enter_named_scope('ph2_relconv', False)
            cc_kv = []
            RC_SLOTS = max(len(c) for c in rc_calls)
            with (
                tc.tile_pool(name='p2i', bufs=1) as ip2,
                tc.tile_pool(name='p2g', bufs=2) as gp2,
                tc.tile_pool(name='p2sb', bufs=3) as sb2,
                tc.tile_pool(name='p2S', bufs=1, space='PSUM') as psS,
                tc.tile_pool(name='p2qkv', bufs=1, space='PSUM') as psQ,
                tc.tile_pool(name='p2tr', bufs=1, space='PSUM') as psT,
            ):
                rci = ip2.tile([128, RC_CHUNKS * 8], i16)
                nc.sync.dma_start(out=rci[:], in_=rc_idx[:])
                rc_off = 0
                piece_stores = []
                for w in range(NW):
                    qkv_ps = [psQ.tile([128, 3 * D], f32, tag=f'qkv{i}', name=f'qkv{i}')
                              for i in range(2)]
                    S_ps = [psS.tile([128, 512 if p < 4 else 256], f32,
                                     tag=f'S{p}', name=f'S{p}') for p in range(NPAIR)]
                    # first/last chunk flags per pair across base regions
                    nch_wp = [sum(rc_chunks[(w * PIECES + pc) * NPAIR + p]
                                  for pc in range(PIECES)) for p in range(NPAIR)]
                    done_wp = [0] * NPAIR
                    for hh in range(PIECES):
                        base = hh * BASEQ
                        calls = rc_calls[w * PIECES + hh]
                        gtiles = {}
                        co = rc_off
                        for slot, take in enumerate(calls):
                            gt = gp2.tile([128, CALL_MAX * D], f16, tag=f'g{slot}')
                            gi = nc.gpsimd.dma_gather(
                                out_ap=gt[:, :take * D].rearrange('p (c e) -> p c e', e=D),
                                in_ap=hn_full[base:NTAB, :],
                                idxs_ap=rci[:, co * 8:(co + take) * 8],
                                num_idxs=take * 128, num_idxs_reg=take * 128,
                                elem_size=D)
                            for cc in cc_hn:
                                add_dep_helper(gi.ins, cc.ins, True, 'gather reads hn')
                            for j in range(take):
                                gtiles[co + j] = (gt, j)
                            co += take
                        for p in range(NPAIR):
                            nch = rc_chunks[(w * PIECES + hh) * NPAIR + p]
                            wd = 512 if p < 4 else 256
                            tot_p = nch_wp[p]
                            for k in range(nch):
                                ck = rc_off
                                gt, j = gtiles[ck]
                                Ax = sb2.tile([128, 512], f16, tag='A')
                                nc.vector.tensor_scalar(
                                    out=Ax[:, :wd], in0=iota[:, :wd],
                                    scalar1=rck[:, ck:ck + 1], scalar2=None,
                                    op0=AO.is_equal)
                                nc.tensor.matmul(
                                    out=S_ps[p][:], lhsT=gt[:, j * D:(j + 1) * D],
                                    rhs=Ax[:, :wd], start=(done_wp[p] == 0),
                                    stop=(done_wp[p] == tot_p - 1))
                                done_wp[p] += 1
                                rc_off += 1
                    # copy S to sbuf (fp16)
                    stp = []
                    for p in range(NPAIR):
                        wd = 512 if p < 4 else 256
                        stx = sb2.tile([128, 512], f16, tag=f'St{p}')
                        nc.scalar.copy(out=stx[:, :wd], in_=S_ps[p][:])
                        stp.append(stx)
                    # transforms + self-loop + bias + relu
                    for sub in range(2):
                        row0 = w * WIN + sub * 128
                        for r in range(REL):
                            p, par = r // 2, r % 2
                            lhs = stp[p][:, par * 256 + sub * 128: par * 256 + sub * 128 + 128]
                            nc.tensor.matmul(out=qkv_ps[sub][:], lhsT=lhs,
                                             rhs=wrel_sb[:, r * 3 * D:(r + 1) * 3 * D],
                                             start=(r == 0), stop=False)
                        hnw2 = sb2.tile([128, D], f16, tag='hnl')
                        nc.sync.dma_start(out=hnw2[:], in_=hn_local[row0:row0 + 128, :])
                        ht_ps = psT.tile([128, 128], f16, tag='ht')
                        nc.tensor.transpose(out=ht_ps[:], in_=hnw2[:], identity=ident_f[:])
                        ht = sb2.tile([128, 128], f16, tag='htsb')
                        nc.scalar.copy(out=ht[:], in_=ht_ps[:])
                        nc.tensor.matmul(out=qkv_ps[sub][:], lhsT=ht[:], rhs=wloop_sb[:],
                                         start=False, stop=False)
                        nc.tensor.matmul(out=qkv_ps[sub][:], lhsT=ones_f[:], rhs=brqkv_sb[:],
                                         start=False, stop=True)
                        qk = sb2.tile([128, D], b16, tag='qsb')
                        nc.scalar.activation(out=qk[:], in_=qkv_ps[sub][:, :D], func=AF.Relu)
                        nc.sync.dma_start(out=q_local[row0:row0 + 128, :], in_=qk[:])
                        kvsb = sb2.tile([128, 2 * D], b16, tag='kvsb')
                        nc.scalar.activation(out=kvsb[:], in_=qkv_ps[sub][:, D:], func=AF.Relu)
                        stin = nc.sync.dma_start(out=kv_local[row0:row0 + 128, :], in_=kvsb[:])
                        piece_stores.append(stin)
                    if (w + 1) % (NW // PIECES) == 0:
                        p = (w + 1) // (NW // PIECES) - 1
                        cc = nc.gpsimd.collective_compute(
                            'AllGather', AO.bypass,
                            replica_groups=[list(range(NCORES))],
                            ins=[kv_local[p * RPP:(p + 1) * RPP, :]],
                            outs=[kv_full[p * NCORES * RPP:(p + 1) * NCORES * RPP, :]
                                  .rearrange('(c r) d -> c r d', c=NCORES)])
                        for stx in piece_stores:
                            add_dep_helper(cc.ins, stx.ins, True, 'allgather reads kv piece')
                        piece_stores = []
                        cc_kv.append(cc)
            nc.leave_named_scope('ph2_relconv', sc2[0], False)

            # =========== phase 3: attention ===========
            sc3 = nc.enter_named_scope('ph3_attn', False)
            AT_SLOTS = max(len(c) for c in at_calls)
            with (
                tc.tile_pool(name='p3i', bufs=1) as ip3,
                tc.tile_pool(name='p3g', bufs=1) as gp3,
                tc.tile_pool(name='p3sb', bufs=3) as sb3,
                tc.tile_pool(name='p3at', bufs=2, space='PSUM') as psA,
                tc.tile_pool(name='p3wv', bufs=2, space='PSUM') as psW,
                tc.tile_pool(name='p3ep', bufs=1, space='PSUM') as psE,
            ):
                ati = ip3.tile([128, AT_CHUNKS * 8], i16)
                nc.sync.dma_start(out=ati[:], in_=at_idx[:])
                at_off = 0
                for swb in range(NSWB):
                    # gather calls for both halves of this block
                    gtiles = {}
                    co = at_off
                    for hh in range(PIECES):
                        base = hh * BASEQ
                        calls = at_calls[swb * PIECES + hh]
                        for slot, take in enumerate(calls):
                            gt = gp3.tile([128, CALL_MAX * 2 * D], b16,
                                          tag=f'ag{hh}_{slot}')
                            gi = nc.gpsimd.dma_gather(
                                out_ap=gt[:, :take * 2 * D].rearrange('p (c e) -> p c e', e=2 * D),
                                in_ap=kv_full[base:NTAB, :],
                                idxs_ap=ati[:, co * 8:(co + take) * 8],
                                num_idxs=take * 128, num_idxs_reg=take * 128,
                                elem_size=2 * D)
                            for cc in cc_kv:
                                add_dep_helper(gi.ins, cc.ins, True, 'gather reads kv')
                            for j in range(take):
                                gtiles[co + j] = (gt, j)
                            co += take
                    # chunk offsets per (hh, swin)
                    offs = {}
                    o = at_off
                    for hh in range(PIECES):
                        for swin in range(4):
                            n = at_chunks[(swb * PIECES + hh) * 4 + swin]
                            offs[(hh, swin)] = (o, n)
                            o += n
                    at_off = o
                    for swin in range(4):
                        sw = swb * 4 + swin
                        tot_sw = sum(offs[(hh, swin)][1] for hh in range(PIECES))
                        done = 0
                        qwin = sb3.tile([128, D], b16, tag='qwin')
                        nc.sync.dma_start(out=qwin[:], in_=q_local[sw * 128:(sw + 1) * 128, :])
                        wvz = psW.tile([128, D + HEADS], f32, tag='wvz')
                        for hh in range(PIECES):
                            c0, nch = offs[(hh, swin)]
                            k = 0
                            while k < nch:
                                # batch within one gather tile
                                gt0, j0 = gtiles[c0 + k]
                                nb = 1
                                while (nb < NBATCH and k + nb < nch
                                       and gtiles[c0 + k + nb][0] is gt0):
                                    nb += 1
                                A4 = sb3.tile([128, NBATCH * 128], b16, tag='A4')
                                for i in range(nb):
                                    ck = c0 + k + i
                                    nc.vector.tensor_scalar(
                                        out=A4[:, i * 128:(i + 1) * 128],
                                        in0=iota_b[:], scalar1=atk[:, ck:ck + 1],
                                        scalar2=None, op0=AO.is_equal)
                                at4 = psA.tile([128, NBATCH * 128], b16, tag='at4')
                                for i in range(nb):
                                    nc.tensor.transpose(
                                        out=at4[:, i * 128:(i + 1) * 128],
                                        in_=A4[:, i * 128:(i + 1) * 128],
                                        identity=ident_b[:])
                                at4s = sb3.tile([128, NBATCH * 128], b16, tag='at4s')
                                nc.scalar.copy(out=at4s[:, :nb * 128], in_=at4[:, :nb * 128])
                                qd4 = psA.tile([128, NBATCH * 128], f32, tag='qd4')
                                for i in range(nb):
                                    nc.tensor.matmul(
                                        out=qd4[:, i * 128:(i + 1) * 128],
                                        lhsT=at4s[:, i * 128:(i + 1) * 128],
                                        rhs=qwin[:], start=True, stop=True)
                                gtv = gt0[:].rearrange('p (c e) -> p c e', e=2 * D)
                                kq4 = sb3.tile([128, NBATCH * 128], f32, tag='kq4')
                                nc.vector.tensor_tensor(
                                    out=kq4[:, :nb * 128].rearrange('p (c e) -> p c e', e=D),
                                    in0=gtv[:, j0:j0 + nb, :D],
                                    in1=qd4[:, :nb * 128].rearrange('p (c e) -> p c e', e=D),
                                    op=AO.mult)
                                sc4 = sb3.tile([128, NBATCH * HEADS], f32, tag='sc4')
                                nc.vector.reduce_sum(
                                    out=sc4[:, :nb * HEADS],
                                    in_=kq4[:, :nb * 128].rearrange('p (h e) -> p h e', e=DH),
                                    axis=mybir.AxisListType.X)
                                nc.vector.tensor_scalar_min(
                                    out=sc4[:, :nb * HEADS], in0=sc4[:, :nb * HEADS],
                                    scalar1=10.0)
                                vse4 = sb3.tile([128, NBATCH * 136], b16, tag='vse4')
                                vsev = vse4[:].rearrange('p (c e) -> p c e', e=136)
                                nc.scalar.activation(
                                    out=vsev[:, :nb, D:],
                                    in_=sc4[:, :nb * HEADS].rearrange('p (c h) -> p c h', h=HEADS),
                                    func=AF.Exp)
                                nc.vector.tensor_tensor(
                                    out=vsev[:, :nb, :D].rearrange('p c (h e) -> p c h e', e=DH),
                                    in0=gtv[:, j0:j0 + nb, D:].rearrange('p c (h e) -> p c h e', e=DH),
                                    in1=vsev[:, :nb, D:].rearrange('p c (h o) -> p c h o', o=1)
                                        .to_broadcast([128, nb, HEADS, DH]),
                                    op=AO.mult)
                                for i in range(nb):
                                    nc.tensor.matmul(
                                        out=wvz[:], lhsT=A4[:, i * 128:(i + 1) * 128],
                                        rhs=vse4[:, i * 136:(i + 1) * 136],
                                        start=(done == 0), stop=(done == tot_sw - 1))
                                    done += 1
                                k += nb
                        # epilogue for this subwindow
                        zr = sb3.tile([128, HEADS], f32, tag='zr')
                        nc.vector.tensor_scalar_add(out=zr[:], in0=wvz[:, D:], scalar1=1e-6)
                        zrec = sb3.tile([128, HEADS], f32, tag='zrec')
                        nc.vector.reciprocal(out=zrec[:], in_=zr[:])
                        attn = sb3.tile([128, D], b16, tag='attn')
                        nc.vector.tensor_tensor(
                            out=attn[:].rearrange('p (h e) -> p h e', e=DH),
                            in0=wvz[:, :D].rearrange('p (h e) -> p h e', e=DH),
                            in1=zrec[:].rearrange('p (h o) -> p h o', o=1)
                                .to_broadcast([128, HEADS, DH]),
                            op=AO.mult)
                        atr_ps = psE.tile([128, D], b16, tag='atr')
                        nc.tensor.transpose(out=atr_ps[:], in_=attn[:], identity=ident_b[:])
                        atr = sb3.tile([128, D], b16, tag='atrsb')
                        nc.scalar.copy(out=atr[:], in_=atr_ps[:])
                        ho_ps = psE.tile([128, D], f32, tag='ho')
                        nc.tensor.matmul(out=ho_ps[:], lhsT=atr[:], rhs=ow_sb[:],
                                         start=True, stop=False)
                        nc.tensor.matmul(out=ho_ps[:], lhsT=ones_b[:], rhs=bro_sb[:],
                                         start=False, stop=True)
                        hob = hobp.tile([128, D], f16, tag=f'hob_{sw}', name=f'hob_{sw}')
                        nc.scalar.copy(out=hob[:], in_=ho_ps[:])
                        hobt.append(hob)
                        nc.vector.tensor_reduce(out=ln1_ns[:, sw:sw + 1], in_=hob[:],
                                                axis=mybir.AxisListType.X, op=AO.add,
                                                negate=True)
                        hsq2 = sb3.tile([128, D], f32, tag='hsq2')
                        nc.vector.tensor_tensor(out=hsq2[:], in0=hob[:], in1=hob[:],
                                                op=AO.mult)
                        nc.vector.tensor_reduce(out=ln1_sq[:, sw:sw + 1], in_=hsq2[:],
                                                axis=mybir.AxisListType.X, op=AO.add)
            nc.leave_named_scope('ph3_attn', sc3[0], False)

            # =========== phase 4: LN1 finalize + gn2 stats ===========
            sc4 = nc.enter_named_scope('ph4_stats', False)
            with (
                tc.tile_pool(name='p4sb', bufs=3) as sb4,
                tc.tile_pool(name='p4ps', bufs=1, space='PSUM') as ps4,
            ):
                nmu = sb4.tile([128, NSW], f32, tag='nmu')
                nc.vector.tensor_scalar_mul(out=nmu[:], in0=ln1_ns[:], scalar1=1.0 / D)
                e2 = sb4.tile([128, NSW], f32, tag='e2')
                nc.vector.tensor_scalar_mul(out=e2[:], in0=ln1_sq[:], scalar1=1.0 / D)
                msq1 = sb4.tile([128, NSW], f32, tag='msq1')
                nc.vector.tensor_tensor(out=msq1[:], in0=nmu[:], in1=nmu[:], op=AO.mult)
                nc.vector.tensor_tensor(out=msq1[:], in0=e2[:], in1=msq1[:], op=AO.subtract)
                nc.vector.tensor_scalar_add(out=msq1[:], in0=msq1[:], scalar1=1e-5)
                rv1 = sb4.tile([128, NSW], f32, tag='rv1')
                nc.vector.reciprocal(out=rv1[:], in_=msq1[:])
                nc.scalar.activation(out=ln1_rsd[:], in_=rv1[:], func=AF.Sqrt)
                nc.vector.tensor_tensor(out=ln1_nmr[:], in0=nmu[:], in1=ln1_rsd[:], op=AO.mult)
                sum2 = ps4.tile([GMAX, D], f32, tag='sum2')
                sq2 = ps4.tile([GMAX, D], f32, tag='sq2')
                for s in range(NSW):
                    y = sb4.tile([128, D], f16, tag='y4')
                    nc.vector.tensor_scalar(out=y[:], in0=hobt[s][:],
                                            scalar1=ln1_rsd[:, s:s + 1],
                                            scalar2=ln1_nmr[:, s:s + 1],
                                            op0=AO.mult, op1=AO.add)
                    ysq = sb4.tile([128, D], f16, tag='ysq')
                    nc.scalar.activation(out=ysq[:], in_=y[:], func=AF.Square)
                    nc.tensor.matmul(out=sum2[:], lhsT=Bt[s][:], rhs=y[:],
                                     start=(s == 0), stop=(s == NSW - 1))
                    nc.tensor.matmul(out=sq2[:], lhsT=Bt[s][:], rhs=ysq[:],
                                     start=(s == 0), stop=(s == NSW - 1))

                # gn2 finalize with LN1 affine folded in
                S1 = keep4.tile([GMAX, D], f32)
                nc.vector.tensor_tensor(out=S1[:], in0=sum2[:],
                                        in1=invc_sb[:].to_broadcast([GMAX, D]), op=AO.mult)
                S2 = keep4.tile([GMAX, D], f32)
                nc.vector.tensor_tensor(out=S2[:], in0=sq2[:],
                                        in1=invc_sb[:].to_broadcast([GMAX, D]), op=AO.mult)
                g1 = csb['ln1_g16']; b1 = csb['ln1_b16']
                mh = keep4.tile([GMAX, D], f32)
                nc.vector.tensor_tensor(out=mh[:], in0=S1[:], in1=g1[:], op=AO.mult)
                nc.vector.tensor_tensor(out=mh[:], in0=mh[:], in1=b1[:], op=AO.add)
                t1 = keep4.tile([GMAX, D], f32)
                nc.vector.tensor_tensor(out=t1[:], in0=S2[:], in1=g1[:], op=AO.mult)
                nc.vector.tensor_tensor(out=t1[:], in0=t1[:], in1=g1[:], op=AO.mult)
                t2 = keep4.tile([GMAX, D], f32)
                nc.vector.tensor_tensor(out=t2[:], in0=S1[:], in1=g1[:], op=AO.mult)
                nc.vector.tensor_tensor(out=t2[:], in0=t2[:], in1=b1[:], op=AO.mult)
                nc.vector.tensor_scalar_mul(out=t2[:], in0=t2[:], scalar1=2.0)
                nc.vector.tensor_tensor(out=t1[:], in0=t1[:], in1=t2[:], op=AO.add)
                nc.vector.tensor_tensor(out=t2[:], in0=b1[:], in1=b1[:], op=AO.mult)
                nc.vector.tensor_tensor(out=t1[:], in0=t1[:], in1=t2[:], op=AO.add)
                # t1 = E[h1^2]
                msq2 = keep4.tile([GMAX, D], f32)
                nc.vector.tensor_tensor(out=msq2[:], in0=mh[:], in1=mh[:], op=AO.mult)
                nc.vector.tensor_tensor(out=msq2[:], in0=msq2[:], in1=csb['gn2_msfac16'][:], op=AO.mult)
                nc.vector.tensor_tensor(out=t1[:], in0=t1[:], in1=msq2[:], op=AO.subtract)
                nc.vector.tensor_scalar_add(out=t1[:], in0=t1[:], scalar1=1e-6)
                std2 = keep4.tile([GMAX, D], f32)
                nc.scalar.activation(out=std2[:], in_=t1[:], func=AF.Sqrt)
                rstd2 = keep4.tile([GMAX, D], f32)
                nc.vector.reciprocal(out=rstd2[:], in_=std2[:])
                alpha2 = keep4.tile([GMAX, D], f32)
                nc.vector.tensor_tensor(out=alpha2[:], in0=rstd2[:], in1=csb['gn2_w16'][:], op=AO.mult)
                A2 = keep4.tile([GMAX, D], f32)
                nc.vector.tensor_tensor(out=A2[:], in0=alpha2[:], in1=g1[:], op=AO.mult)
                B2 = keep4.tile([GMAX, D], f32)
                nc.vector.tensor_tensor(out=B2[:], in0=mh[:], in1=csb['gn2_ms16'][:], op=AO.mult)
                nc.vector.tensor_tensor(out=B2[:], in0=B2[:], in1=alpha2[:], op=AO.mult)
                nc.vector.tensor_tensor(out=B2[:], in0=csb['gn2_b16'][:], in1=B2[:], op=AO.subtract)
                t3 = keep4.tile([GMAX, D], f32)
                nc.vector.tensor_tensor(out=t3[:], in0=b1[:], in1=alpha2[:], op=AO.mult)
                nc.vector.tensor_tensor(out=B2[:], in0=B2[:], in1=t3[:], op=AO.add)
                a2b2 = keep4.tile([GMAX, 2 * D], f16)
                nc.vector.tensor_copy(out=a2b2[:, :D], in_=A2[:])
                nc.vector.tensor_copy(out=a2b2[:, D:], in_=B2[:])
            nc.leave_named_scope('ph4_stats', sc4[0], False)

            # =========== phase 5: gn2 apply + FFN + LN2 center ===========
            sc5 = nc.enter_named_scope('ph5_ffn', False)
            with (
                tc.tile_pool(name='p5sb', bufs=3) as sb5,
                tc.tile_pool(name='p5ps', bufs=1, space='PSUM') as ps5,
                tc.tile_pool(name='p5ps2', bufs=2, space='PSUM') as ps5b,
            ):
                for s in range(NSW):
                    btp = ps5b.tile([GMAX, 128], f16, tag='btp2')
                    nc.tensor.transpose(out=btp[:], in_=Bt[s][:], identity=ident_f[:])
                    bts = sb5.tile([GMAX, 128], f16, tag='bts2')
                    nc.scalar.copy(out=bts[:], in_=btp[:])
                    ab_ps = ps5b.tile([128, 2 * D], f32, tag='ab2')
                    nc.tensor.matmul(out=ab_ps[:], lhsT=bts[:], rhs=a2b2[:],
                                     start=True, stop=True)
                    y5 = sb5.tile([128, D], f16, tag='y5')
                    nc.vector.tensor_scalar(out=y5[:], in0=hobt[s][:],
                                            scalar1=ln1_rsd[:, s:s + 1],
                                            scalar2=ln1_nmr[:, s:s + 1],
                                            op0=AO.mult, op1=AO.add)
                    h2 = sb5.tile([128, D], f16, tag='h2')
                    nc.vector.tensor_tensor(out=h2[:], in0=y5[:], in1=ab_ps[:, :D], op=AO.mult)
                    nc.vector.tensor_tensor(out=h2[:], in0=h2[:], in1=ab_ps[:, D:], op=AO.add)
                    h2t_ps = ps5.tile([128, D], f16, tag='h2t')
                    nc.tensor.transpose(out=h2t_ps[:], in_=h2[:], identity=ident_f[:])
                    h2tt = sb5.tile([128, D], f16, tag='h2tsb')
                    nc.scalar.copy(out=h2tt[:], in_=h2t_ps[:])
                    f1_ps = ps5.tile([128, 2 * D], f32, tag='f1')
                    nc.tensor.matmul(out=f1_ps[:], lhsT=h2tt[:], rhs=ffn1_sb[:],
                                     start=True, stop=False)
                    nc.tensor.matmul(out=f1_ps[:], lhsT=ones_f[:], rhs=brf1_sb[:],
                                     start=False, stop=True)
                    fr = sb5.tile([128, 2 * D], f16, tag='fr')
                    nc.scalar.activation(out=fr[:], in_=f1_ps[:], func=AF.Relu)
                    frt_ps = ps5.tile([128, 2 * D], f16, tag='frt')
                    nc.tensor.transpose(out=frt_ps[:, :D], in_=fr[:, :D], identity=ident_f[:])
                    nc.tensor.transpose(out=frt_ps[:, D:], in_=fr[:, D:], identity=ident_f[:])
                    frt = sb5.tile([128, 2 * D], f16, tag='frtsb')
                    nc.scalar.copy(out=frt[:], in_=frt_ps[:])
                    h3_ps = ps5.tile([128, D], f32, tag='h3')
                    nc.tensor.matmul(out=h3_ps[:], lhsT=frt[:, :D], rhs=ffn2_sb[:, :D],
                                     start=True, stop=False)
                    nc.tensor.matmul(out=h3_ps[:], lhsT=frt[:, D:], rhs=ffn2_sb[:, D:],
                                     start=False, stop=False)
                    nc.tensor.matmul(out=h3_ps[:], lhsT=ones_f[:], rhs=brf2_sb[:],
                                     start=False, stop=True)
                    # LN2 center + var column
                    nscol = sb5.tile([128, 1], f32, tag='nscol')
                    nc.vector.tensor_reduce(out=nscol[:], in_=h3_ps[:],
                                            axis=mybir.AxisListType.X, op=AO.add,
                                            negate=True)
                    nc.vector.tensor_scalar_mul(out=nscol[:], in0=nscol[:], scalar1=1.0 / D)
                    xc2 = xc2p.tile([128, D], f16, tag=f'xc2_{s}', name=f'xc2_{s}')
                    nc.vector.tensor_scalar(out=xc2[:], in0=h3_ps[:], scalar1=nscol[:],
                                            scalar2=None, op0=AO.add)
                    xc2t.append(xc2)
                    sqt = sb5.tile([128, D], f16, tag='sqt')
                    nc.vector.tensor_tensor(out=sqt[:], in0=xc2[:], in1=xc2[:], op=AO.mult)
                    nc.vector.tensor_reduce(out=ln2_vr[:, s:s + 1], in_=sqt[:],
                                            axis=mybir.AxisListType.X, op=AO.add)
            nc.leave_named_scope('ph5_ffn', sc5[0], False)

            # =========== phase 6: LN2 finalize + output ===========
            sc6 = nc.enter_named_scope('ph6_out', False)
            with tc.tile_pool(name='p6sb', bufs=3) as sb6:
                vr2 = sb6.tile([128, NSW], f32, tag='vr2')
                nc.vector.tensor_scalar(out=vr2[:], in0=ln2_vr[:], scalar1=1.0 / D,
                                        scalar2=1e-5, op0=AO.mult, op1=AO.add)
                rv2 = sb6.tile([128, NSW], f32, tag='rv2')
                nc.vector.reciprocal(out=rv2[:], in_=vr2[:])
                nc.scalar.activation(out=ln2_rsd[:], in_=rv2[:], func=AF.Sqrt)
                for s in range(NSW):
                    ov = sb6.tile([128, D], f32, tag='ov')
                    nc.vector.tensor_scalar(out=ov[:], in0=xc2t[s][:],
                                            scalar1=ln2_rsd[:, s:s + 1],
                                            scalar2=None, op0=AO.mult)
                    nc.vector.tensor_tensor(out=ov[:], in0=ov[:], in1=csb['ln2_g'][:], op=AO.mult)
                    nc.vector.tensor_tensor(out=ov[:], in0=ov[:], in1=csb['ln2_b'][:], op=AO.add)
                    nc.sync.dma_start(out=out_sl[s * 128:(s + 1) * 128, :], in_=ov[:])
            nc.leave_named_scope('ph6_out', sc6[0], False)

            if debug:
                nc.sync.dma_start(out=hn_dbg[:], in_=hn_local[:])
                nc.sync.dma_start(out=kv_dbg[:], in_=kv_local[:])

            k4.__exit__(None, None, None)
            xc2p_cm.__exit__(None, None, None)
            hobp_cm.__exit__(None, None, None)
            bpool_cm.__exit__(None, None, None)

    nc.finalize()
    return nc


def kernel(**inputs) -> np.ndarray:
    _ensure_hooks()
    from concourse.bass_utils import run_bass_kernel_spmd

    static, in_maps, meta = preprocess(inputs)
    key = tuple(sorted((k, str(v)) for k, v in static.items()))
    if key not in _PROGRAM_CACHE:
        _PROGRAM_CACHE[key] = build_program(static)
    nc = _PROGRAM_CACHE[key]

    trace = os.environ.get("KERNEL_TRACE") == "1"
    res = run_bass_kernel_spmd(nc, in_maps, list(range(NCORES)), trace=trace)
    if trace and res.exec_time_ns:
        print("HW exec time:", res.exec_time_ns, "ns")
    out = np.zeros((N_NODES, D), np.float32)
    for c in range(NCORES):
        n0, n1 = int(meta['n0'][c]), int(meta['n1'][c])
        out[n0:n1] = res.results[c]['out_slice'][:n1 - n0]
    return out


# revision 11
# speedup vs baseline: 1.3063x; 1.1469x over previous
"""GTLayer (relational graph transformer layer) on 8 Trainium2 NeuronCores.

v2 strategy (see kernel_v1_baseline.py for the original):
- Nodes partitioned across 8 cores in graph-aligned contiguous slices;
  edges live with the core owning dst. Global gather tables laid out in 4
  row-pieces so allgathers can be fired piecewise and overlapped.
- fp16 data lane for relconv (hn table, gathered rows, W_rel/W_loop, S
  sums), bf16 lane for attention (kv table, one-hots, V*exp) — exp(score)
  products can exceed fp16 range. PSUM accumulation is always f32.
- RelConv: edges per (dst-window 256, src-half, rel-PAIR); S psum tiles are
  [128, 512] covering two relations (one-hot column = dl + 256*parity), so
  all 9 relations fit one PSUM residency and gather call regions merge to
  (window, half).
- One-hots built with tensor_scalar(is_equal, scalar1=key column) against
  an iota ramp (2x DVE mode), not tensor_tensor broadcasts.
- Attention: sw-blocks of 4 subwindows share gather calls; chunks
  processed in batches of <=4 with batched vector ops.
- Scalar engine does psum->sbuf copies (Copy), Relu/Exp/Square, arranged
  so activation tables almost never reload. LN1/LN2 sqrt is batched
  (columns collected across subwindows, one Sqrt instruction each).
- Biases applied as K=1 matmuls (ones-row lhsT) accumulating into PSUM.
- LN1 affine folded into graphNorm2's alpha/beta algebra.
"""
import os
import sys
import types
import numpy as np

NCORES = 8
N_NODES = 100000
N_EDGES = 600000
D = 128
REL = 9
NPAIR = 5
NG = 64
HEADS = 8
DH = 16
WIN = 256          # relconv dst window
GMAX = 16          # max graphs per core
CALL_MAX = 8       # max chunks (of 128 slots) per dma_gather call
PIECES = 4         # allgather pieces; also gather base regions (idx >= 0 always)
NBATCH = 4         # attention chunk batch


def _ensure_hooks():
    if "antenv.axon_hooks" not in sys.modules:
        hooks = types.ModuleType("antenv.axon_hooks")
        h = [None]
        hooks.set_axon_ntff_profile_hook = lambda v: h.__setitem__(0, v)
        hooks.get_axon_ntff_profile_hook = lambda: h[0]
        sys.modules["antenv.axon_hooks"] = hooks
        try:
            from trn_agent_boot.trn_boot import _ntff_profile_via_ctypes
            hooks.set_axon_ntff_profile_hook(
                _ntff_profile_via_ctypes("/opt/axon/libaxon_pjrt.so"))
        except Exception:
            pass


# ----------------------------------------------------------------------------
# Host preprocessing
# ----------------------------------------------------------------------------

def _pack_idx16(idx):
    """int16 index array -> [128, n/16] wrapped+replicated layout."""
    n = len(idx)
    assert n % 16 == 0
    blk = idx.reshape(n // 16, 16).T
    return np.tile(blk, (8, 1)).astype(np.int16)


def _layout_slots(order_edges, idx_vals, key_vals, n_chunks):
    """Place edges into n_chunks*128 slots (full 128 per chunk), pads get
    idx 0, key -1. Returns (idx int32, key f32)."""
    tot = n_chunks * 128
    idx = np.zeros(tot, np.int32)
    key = np.full(tot, -1.0, np.float32)
    ne = len(order_edges)
    assert ne <= tot, (ne, n_chunks)
    idx[:ne] = idx_vals[order_edges]
    key[:ne] = key_vals[order_edges]
    return idx, key


def _calls_for(n, cap):
    calls = []
    n = int(n)
    while n > 0:
        take = min(n, cap)
        calls.append(take)
        n -= take
    return calls


def preprocess(inputs):
    import ml_dtypes
    bf16 = ml_dtypes.bfloat16
    h = np.asarray(inputs['h'], np.float32)
    src = np.asarray(inputs['src']).astype(np.int64)
    dst = np.asarray(inputs['dst']).astype(np.int64)
    et = np.asarray(inputs['etypes']).astype(np.int64)
    seg = np.asarray(inputs['seg']).astype(np.int64)

    # --- graph-aligned node partition ---
    gstart = np.searchsorted(seg, np.arange(NG + 1))
    bounds = [0]
    for c in range(1, NCORES):
        target = c * N_NODES / NCORES
        g = int(np.argmin(np.abs(gstart - target)))
        bounds.append(int(gstart[g]))
    bounds.append(N_NODES)
    n0 = np.array(bounds[:-1]); n1 = np.array(bounds[1:])
    sizes = n1 - n0
    ROUND = max(512, WIN * PIECES)
    P_NODES = int(np.ceil(sizes.max() / ROUND) * ROUND)
    RPP = P_NODES // PIECES
    NW = P_NODES // WIN
    NSW = NW * 2
    assert NSW % 4 == 0
    NSWB = NSW // 4
    NTAB = NCORES * P_NODES
    assert NCORES * RPP <= 32768  # per-piece base region fits int16 idx

    owner = np.searchsorted(n1, np.arange(N_NODES), side='right')
    rloc = np.arange(N_NODES) - n0[owner]
    piece = rloc // RPP
    gpos = piece * (NCORES * RPP) + owner * RPP + (rloc - piece * RPP)

    g0 = np.searchsorted(gstart, n0, side='right') - 1
    counts_g = np.diff(gstart).astype(np.float32)

    BASEQ = NCORES * RPP
    srcp = gpos[src]
    half = srcp // BASEQ          # source piece = gather base region (0..3)
    ecore = owner[dst]
    dst_off = dst - n0[ecore]
    w_e = dst_off // WIN
    pair_e = et // 2
    par_e = et % 2
    key512 = (dst_off % WIN + 256 * par_e).astype(np.float32)
    sw_e = dst_off // 128
    dl128 = (dst_off % 128).astype(np.float32)
    swb_e = sw_e // 4
    swin_e = sw_e % 4
    idx_rel = (srcp - half * BASEQ).astype(np.int32)
    assert idx_rel.min() >= 0 and idx_rel.max() < 32768

    # --- relconv chunk structure: groups q = (w, pc, pair) ---
    NQ = NW * PIECES * NPAIR
    rkey = (w_e * PIECES + half) * NPAIR + pair_e
    rc_counts = np.zeros((NCORES, NQ), np.int64)
    for c in range(NCORES):
        rc_counts[c] = np.bincount(rkey[ecore == c], minlength=NQ)
    rc_chunks = np.ceil(rc_counts / 128.0).max(0).astype(np.int64)
    # ensure >=1 chunk per (w, pair) so S psum gets initialized
    for w in range(NW):
        for p in range(NPAIR):
            qs = [(w * PIECES + pc) * NPAIR + p for pc in range(PIECES)]
            if sum(rc_chunks[q] for q in qs) == 0:
                rc_chunks[qs[0]] = 1
    RC_CHUNKS = int(rc_chunks.sum())
    # call list per (w, pc)
    rc_calls = []
    for w in range(NW):
        for pc in range(PIECES):
            tot = int(sum(rc_chunks[(w * PIECES + pc) * NPAIR + p] for p in range(NPAIR)))
            rc_calls.append(tuple(_calls_for(tot, 2 * CALL_MAX)))

    # --- attention chunk structure: groups aq = (swb, pc, swin) ---
    NAQ = NSWB * PIECES * 4
    akey = (swb_e * PIECES + half) * 4 + swin_e
    at_counts = np.zeros((NCORES, NAQ), np.int64)
    for c in range(NCORES):
        at_counts[c] = np.bincount(akey[ecore == c], minlength=NAQ)
    at_chunks = np.ceil(at_counts / 128.0).max(0).astype(np.int64)
    # ensure every sw has >=1 chunk overall (wvz psum init)
    for sw in range(NSW):
        swb, swin = sw // 4, sw % 4
        aqs = [(swb * PIECES + pc) * 4 + swin for pc in range(PIECES)]
        if sum(at_chunks[a] for a in aqs) == 0:
            at_chunks[aqs[0]] = 1
    AT_CHUNKS = int(at_chunks.sum())
    at_calls = []
    for swb in range(NSWB):
        for pc in range(PIECES):
            tot = int(sum(at_chunks[(swb * PIECES + pc) * 4 + s] for s in range(4)))
            at_calls.append(tuple(_calls_for(tot, CALL_MAX)))

    # --- per-core data arrays ---
    in_maps = []
    for c in range(NCORES):
        m = np.nonzero(ecore == c)[0]
        # order by (group, srcp) for gather locality
        order = np.lexsort((srcp[m], rkey[m]))
        es = m[order]
        rk = rkey[m][order]
        run_s = np.searchsorted(rk, np.arange(NQ))
        run_e = np.searchsorted(rk, np.arange(NQ) + 1)
        rc_idx = np.zeros(RC_CHUNKS * 128, np.int32)
        rc_key = np.full(RC_CHUNKS * 128, -1.0, np.float32)
        coff = 0
        for q in range(NQ):
            nch = int(rc_chunks[q])
            if nch == 0:
                assert run_e[q] == run_s[q]
                continue
            ii, kk = _layout_slots(es[run_s[q]:run_e[q]], idx_rel, key512, nch)
            rc_idx[coff * 128:(coff + nch) * 128] = ii
            rc_key[coff * 128:(coff + nch) * 128] = kk
            coff += nch
        assert coff == RC_CHUNKS

        aorder = np.lexsort((srcp[m], akey[m]))
        aes = m[aorder]
        ak = akey[m][aorder]
        arun_s = np.searchsorted(ak, np.arange(NAQ))
        arun_e = np.searchsorted(ak, np.arange(NAQ) + 1)
        at_idx = np.zeros(AT_CHUNKS * 128, np.int32)
        at_key = np.full(AT_CHUNKS * 128, -1.0, np.float32)
        coff = 0
        for q in range(NAQ):
            nch = int(at_chunks[q])
            if nch == 0:
                assert arun_e[q] == arun_s[q]
                continue
            ii, kk = _layout_slots(aes[arun_s[q]:arun_e[q]], idx_rel, dl128, nch)
            at_idx[coff * 128:(coff + nch) * 128] = ii
            at_key[coff * 128:(coff + nch) * 128] = kk
            coff += nch
        assert coff == AT_CHUNKS

        hs = np.zeros((P_NODES, D), np.float32)
        hs[:sizes[c]] = h[n0[c]:n1[c]]
        segl = np.full(P_NODES, -1.0, np.float32)
        segl[:sizes[c]] = (seg[n0[c]:n1[c]] - g0[c]).astype(np.float32)
        ginc = np.zeros((GMAX, 1), np.float32)
        ng_c = int(seg[n1[c] - 1] - g0[c]) + 1
        assert ng_c <= GMAX
        ginc[:ng_c, 0] = 1.0 / counts_g[g0[c]:g0[c] + ng_c]

        im = {
            'h_slice': hs.astype(np.float16),
            'seg_col': segl.reshape(NSW, 128).T.copy(),
            'inv_cnt': ginc,
            'rc_idx': _pack_idx16(rc_idx.astype(np.int16)),
            'rc_key': rc_key.reshape(RC_CHUNKS, 128).T.copy(),
            'at_idx': _pack_idx16(at_idx.astype(np.int16)),
            'at_key': at_key.reshape(AT_CHUNKS, 128).T.copy(),
        }
        in_maps.append(im)

    # --- shared weights ---
    def A(x):
        return np.ascontiguousarray(np.asarray(x, np.float32))
    Wrel = np.concatenate([
        np.einsum('rb,bio->rio', A(inputs[f'{nm}_coeff']), A(inputs[f'{nm}_basis']))
        for nm in ('q', 'k', 'v')], axis=2)            # [9, 128, 384]
    Wrel[:, :, :D] *= 0.25  # fold score/sqrt(dh) into Q
    wloop = np.concatenate([A(inputs[f'{nm}_loop']) for nm in ('q', 'k', 'v')], 1)
    wloop[:, :D] *= 0.25
    bqkv = np.concatenate([A(inputs[f'{nm}_bias']) for nm in ('q', 'k', 'v')])
    bqkv[:D] *= 0.25
    ffn2p = np.zeros((D, 2 * D), np.float32)           # two K-chunks side by side
    ffn2p[:, :D] = A(inputs['ffn2_w'])[:D, :]
    ffn2p[:, D:] = A(inputs['ffn2_w'])[D:, :]
    w_shared = {
        'w_rel': A(Wrel.reshape(REL * D, 3 * D)).astype(np.float16),
        'w_loop': wloop.astype(np.float16),
        'brow_qkv': bqkv.reshape(1, 3 * D).astype(np.float16),
        'o_w': A(inputs['o_w']).astype(bf16),
        'brow_o': A(inputs['o_b']).reshape(1, D).astype(bf16),
        'ffn1': A(inputs['ffn1_w']).astype(np.float16),
        'brow_f1': A(inputs['ffn1_b']).reshape(1, 2 * D).astype(np.float16),
        'ffn2': ffn2p.astype(np.float16),
        'brow_f2': A(inputs['ffn2_b']).reshape(1, D).astype(np.float16),
        'ln2_g': np.tile(A(inputs['ln2_g'])[None, :], (128, 1)),
        'ln2_b': np.tile(A(inputs['ln2_b'])[None, :], (128, 1)),
    }
    for nm in ('gn1', 'gn2'):
        w = A(inputs[f'{nm}_w']); b = A(inputs[f'{nm}_b']); ms = A(inputs[f'{nm}_ms'])
        w_shared[f'{nm}_w16'] = np.tile(w[None, :], (GMAX, 1))
        w_shared[f'{nm}_b16'] = np.tile(b[None, :], (GMAX, 1))
        w_shared[f'{nm}_ms16'] = np.tile(ms[None, :], (GMAX, 1))
        w_shared[f'{nm}_msfac16'] = np.tile((ms * (2 - ms))[None, :], (GMAX, 1))
    for nm in ('ln1_g', 'ln1_b'):
        w_shared[f'{nm}16'] = np.tile(A(inputs[nm])[None, :], (GMAX, 1))
    for im in in_maps:
        im.update(w_shared)

    static = dict(P_NODES=P_NODES, NW=NW, NSW=NSW, NSWB=NSWB, RPP=RPP,
                  NTAB=NTAB,
                  rc_chunks=tuple(int(x) for x in rc_chunks),
                  at_chunks=tuple(int(x) for x in at_chunks),
                  rc_calls=tuple(rc_calls), at_calls=tuple(at_calls),
                  RC_CHUNKS=RC_CHUNKS, AT_CHUNKS=AT_CHUNKS)
    meta = dict(n0=n0, n1=n1, sizes=sizes)
    return static, in_maps, meta


# ----------------------------------------------------------------------------
# Bass program
# ----------------------------------------------------------------------------

_PROGRAM_CACHE = {}


def build_program(st):
    import concourse.bass as bass
    import concourse.bacc as bacc
    import concourse.mybir as mybir
    import concourse.tile as tile
    from concourse.tile import TileContext
    from concourse.masks import make_identity
    from bass_rust import add_dep_helper

    P_NODES = st['P_NODES']; NW = st['NW']; NSW = st['NSW']
    NSWB = st['NSWB']; RPP = st['RPP']; NTAB = st['NTAB']
    BASEQ = NCORES * RPP
    rc_chunks = st['rc_chunks']; at_chunks = st['at_chunks']
    rc_calls = st['rc_calls']; at_calls = st['at_calls']
    RC_CHUNKS = st['RC_CHUNKS']; AT_CHUNKS = st['AT_CHUNKS']
    f32 = mybir.dt.float32
    f16 = mybir.dt.float16
    b16 = mybir.dt.bfloat16
    i16 = mybir.dt.int16
    AO = mybir.AluOpType
    AF = mybir.ActivationFunctionType

    nc = bacc.Bacc()

    # --- I/O ---
    h_slice = nc.declare_dram_parameter('h_slice', [P_NODES, D], f16, isOutput=False)
    seg_col = nc.declare_dram_parameter('seg_col', [128, NSW], f32, isOutput=False)
    inv_cnt = nc.declare_dram_parameter('inv_cnt', [GMAX, 1], f32, isOutput=False)
    rc_idx = nc.declare_dram_parameter('rc_idx', [128, RC_CHUNKS * 8], i16, isOutput=False)
    rc_keyd = nc.declare_dram_parameter('rc_key', [128, RC_CHUNKS], f32, isOutput=False)
    at_idx = nc.declare_dram_parameter('at_idx', [128, AT_CHUNKS * 8], i16, isOutput=False)
    at_keyd = nc.declare_dram_parameter('at_key', [128, AT_CHUNKS], f32, isOutput=False)
    w_rel = nc.declare_dram_parameter('w_rel', [REL * D, 3 * D], f16, isOutput=False)
    w_loop = nc.declare_dram_parameter('w_loop', [D, 3 * D], f16, isOutput=False)
    brow_qkv = nc.declare_dram_parameter('brow_qkv', [1, 3 * D], f16, isOutput=False)
    o_w = nc.declare_dram_parameter('o_w', [D, D], b16, isOutput=False)
    brow_o = nc.declare_dram_parameter('brow_o', [1, D], b16, isOutput=False)
    ffn1 = nc.declare_dram_parameter('ffn1', [D, 2 * D], f16, isOutput=False)
    brow_f1 = nc.declare_dram_parameter('brow_f1', [1, 2 * D], f16, isOutput=False)
    ffn2 = nc.declare_dram_parameter('ffn2', [D, 2 * D], f16, isOutput=False)
    brow_f2 = nc.declare_dram_parameter('brow_f2', [1, D], f16, isOutput=False)
    cdecl = {}
    for nm in ('ln2_g', 'ln2_b'):
        cdecl[nm] = nc.declare_dram_parameter(nm, [128, D], f32, isOutput=False)
    for nm in ('gn1_w16', 'gn1_b16', 'gn1_ms16', 'gn1_msfac16',
               'gn2_w16', 'gn2_b16', 'gn2_ms16', 'gn2_msfac16',
               'ln1_g16', 'ln1_b16'):
        cdecl[nm] = nc.declare_dram_parameter(nm, [GMAX, D], f32, isOutput=False)
    out_sl = nc.declare_dram_parameter('out_slice', [P_NODES, D], f32, isOutput=True)

    # --- internal DRAM ---
    hn_local = nc.dram_tensor('hn_local', [P_NODES, D], f16)
    q_local = nc.dram_tensor('q_local', [P_NODES, D], b16)
    kv_local = nc.dram_tensor('kv_local', [P_NODES, 2 * D], b16)
    hn_full = nc.dram_tensor('hn_full', [NTAB, D], f16, addr_space='Shared')
    kv_full = nc.dram_tensor('kv_full', [NTAB, 2 * D], b16, addr_space='Shared')
    debug = os.environ.get('KERNEL_DEBUG') == '1'
    if debug:
        hn_dbg = nc.declare_dram_parameter('hn_dbg', [P_NODES, D], f16, isOutput=True)
        kv_dbg = nc.declare_dram_parameter('kv_dbg', [P_NODES, 2 * D], b16, isOutput=True)

    with TileContext(nc) as tc:
        with tc.tile_pool(name='const', bufs=1) as cpool:
            iota = cpool.tile([128, 2 * WIN], f16)
            nc.gpsimd.iota(iota[:], pattern=[[1, 2 * WIN]], base=0,
                           channel_multiplier=0, allow_small_or_imprecise_dtypes=True)
            iota_b = cpool.tile([128, 128], b16)
            nc.gpsimd.iota(iota_b[:], pattern=[[1, 128]], base=0,
                           channel_multiplier=0, allow_small_or_imprecise_dtypes=True)
            ident_f = cpool.tile([128, 128], f16)
            make_identity(nc, ident_f[:])
            ident_b = cpool.tile([128, 128], b16)
            make_identity(nc, ident_b[:])
            ones_f = cpool.tile([1, 128], f16)
            nc.gpsimd.memset(ones_f[:], 1.0)
            ones_b = cpool.tile([1, 128], b16)
            nc.gpsimd.memset(ones_b[:], 1.0)

            segs = cpool.tile([128, NSW], f32)
            nc.sync.dma_start(out=segs[:], in_=seg_col[:])
            rck = cpool.tile([128, RC_CHUNKS], f32)
            nc.sync.dma_start(out=rck[:], in_=rc_keyd[:])
            atk = cpool.tile([128, AT_CHUNKS], f32)
            nc.sync.dma_start(out=atk[:], in_=at_keyd[:])

            wrel_sb = cpool.tile([128, REL * 3 * D], f16)
            for r in range(REL):
                nc.sync.dma_start(out=wrel_sb[:, r * 3 * D:(r + 1) * 3 * D],
                                  in_=w_rel[r * D:(r + 1) * D, :])
            wloop_sb = cpool.tile([128, 3 * D], f16)
            nc.sync.dma_start(out=wloop_sb[:], in_=w_loop[:])
            brqkv_sb = cpool.tile([1, 3 * D], f16)
            nc.sync.dma_start(out=brqkv_sb[:], in_=brow_qkv[:])
            ow_sb = cpool.tile([D, D], b16)
            nc.sync.dma_start(out=ow_sb[:], in_=o_w[:])
            bro_sb = cpool.tile([1, D], b16)
            nc.sync.dma_start(out=bro_sb[:], in_=brow_o[:])
            ffn1_sb = cpool.tile([D, 2 * D], f16)
            nc.sync.dma_start(out=ffn1_sb[:], in_=ffn1[:])
            brf1_sb = cpool.tile([1, 2 * D], f16)
            nc.sync.dma_start(out=brf1_sb[:], in_=brow_f1[:])
            ffn2_sb = cpool.tile([D, 2 * D], f16)
            nc.sync.dma_start(out=ffn2_sb[:], in_=ffn2[:])
            brf2_sb = cpool.tile([1, D], f16)
            nc.sync.dma_start(out=brf2_sb[:], in_=brow_f2[:])
            csb = {}
            for nm, dd in cdecl.items():
                t = cpool.tile(list(dd.shape), f32, tag=f'c_{nm}')
                nc.sync.dma_start(out=t[:], in_=dd[:])
                csb[nm] = t
            invc_sb = cpool.tile([GMAX, 1], f32)
            nc.sync.dma_start(out=invc_sb[:], in_=inv_cnt[:])

            # batched LN stat columns
            ln1_ns = cpool.tile([128, NSW], f32, tag='ln1_ns')
            ln1_sq = cpool.tile([128, NSW], f32, tag='ln1_sq')
            ln1_rsd = cpool.tile([128, NSW], f32, tag='ln1_rsd')
            ln1_nmr = cpool.tile([128, NSW], f32, tag='ln1_nmr')
            ln2_vr = cpool.tile([128, NSW], f32, tag='ln2_vr')
            ln2_rsd = cpool.tile([128, NSW], f32, tag='ln2_rsd')

            # persistent pools
            bpool_cm = tc.tile_pool(name='bpool', bufs=1)
            bpool = bpool_cm.__enter__()
            hobp_cm = tc.tile_pool(name='hobp', bufs=1)
            hobp = hobp_cm.__enter__()
            xc2p_cm = tc.tile_pool(name='xc2p', bufs=1)
            xc2p = xc2p_cm.__enter__()
            k4 = tc.tile_pool(name='p4keep', bufs=1)
            keep4 = k4.__enter__()
            Bt = []      # [128, GMAX] f16 one-hot per subwindow
            hobt = []    # [128, D] f16 attn-out per subwindow
            xc2t = []    # [128, D] f16 LN2-centered per subwindow

            # =========== phase 1: graphNorm1 ===========
            sc1 = nc.enter_named_scope('ph1_gn1', False)
            cc_hn = []
            with (
                tc.tile_pool(name='p1keep', bufs=1) as keep1,
                tc.tile_pool(name='p1sb', bufs=3) as sb1,
                tc.tile_pool(name='p1ps', bufs=1, space='PSUM') as ps1,
                tc.tile_pool(name='p1ps2', bufs=2, space='PSUM') as ps1b,
            ):
                sum_ps = ps1.tile([GMAX, D], f32, tag='sums')
                sq_ps = ps1.tile([GMAX, D], f32, tag='sqs')
                for s in range(NSW):
                    hw = sb1.tile([128, D], f16, tag='h_in')
                    nc.sync.dma_start(out=hw[:], in_=h_slice[s * 128:(s + 1) * 128, :])
                    B = bpool.tile([128, GMAX], f16, tag=f'B_{s}', name=f'B_{s}')
                    nc.vector.tensor_scalar(out=B[:], in0=iota[:, :GMAX],
                                            scalar1=segs[:, s:s + 1], scalar2=None,
                                            op0=AO.is_equal)
                    hsq = sb1.tile([128, D], f16, tag='hsq')
                    nc.scalar.activation(out=hsq[:], in_=hw[:], func=AF.Square)
                    nc.tensor.matmul(out=sum_ps[:], lhsT=B[:], rhs=hw[:],
                                     start=(s == 0), stop=(s == NSW - 1))
                    nc.tensor.matmul(out=sq_ps[:], lhsT=B[:], rhs=hsq[:],
                                     start=(s == 0), stop=(s == NSW - 1))
                    Bt.append(B)
                # finalize -> alpha/beta [GMAX, D] f32
                mean = keep1.tile([GMAX, D], f32)
                nc.vector.tensor_tensor(out=mean[:], in0=sum_ps[:],
                                        in1=invc_sb[:].to_broadcast([GMAX, D]), op=AO.mult)
                ex2 = keep1.tile([GMAX, D], f32)
                nc.vector.tensor_tensor(out=ex2[:], in0=sq_ps[:],
                                        in1=invc_sb[:].to_broadcast([GMAX, D]), op=AO.mult)
                msq = keep1.tile([GMAX, D], f32)
                nc.vector.tensor_tensor(out=msq[:], in0=mean[:], in1=mean[:], op=AO.mult)
                nc.vector.tensor_tensor(out=msq[:], in0=msq[:], in1=csb['gn1_msfac16'][:], op=AO.mult)
                var = keep1.tile([GMAX, D], f32)
                nc.vector.tensor_tensor(out=var[:], in0=ex2[:], in1=msq[:], op=AO.subtract)
                nc.vector.tensor_scalar_add(out=var[:], in0=var[:], scalar1=1e-6)
                std = keep1.tile([GMAX, D], f32)
                nc.scalar.activation(out=std[:], in_=var[:], func=AF.Sqrt)
                rstd = keep1.tile([GMAX, D], f32)
                nc.vector.reciprocal(out=rstd[:], in_=std[:])
                alpha1 = keep1.tile([GMAX, D], f32)
                nc.vector.tensor_tensor(out=alpha1[:], in0=rstd[:], in1=csb['gn1_w16'][:], op=AO.mult)
                beta1 = keep1.tile([GMAX, D], f32)
                nc.vector.tensor_tensor(out=beta1[:], in0=mean[:], in1=csb['gn1_ms16'][:], op=AO.mult)
                nc.vector.tensor_tensor(out=beta1[:], in0=beta1[:], in1=alpha1[:], op=AO.mult)
                nc.vector.tensor_tensor(out=beta1[:], in0=csb['gn1_b16'][:], in1=beta1[:], op=AO.subtract)
                a1b1 = keep1.tile([GMAX, 2 * D], f16)
                nc.vector.tensor_copy(out=a1b1[:, :D], in_=alpha1[:])
                nc.vector.tensor_copy(out=a1b1[:, D:], in_=beta1[:])
                # apply
                piece_stores = []
                for s in range(NSW):
                    btp = ps1b.tile([GMAX, 128], f16, tag='btp')
                    nc.tensor.transpose(out=btp[:], in_=Bt[s][:], identity=ident_f[:])
                    bts = sb1.tile([GMAX, 128], f16, tag='bts')
                    nc.scalar.copy(out=bts[:], in_=btp[:])
                    ab_ps = ps1b.tile([128, 2 * D], f32, tag='ab')
                    nc.tensor.matmul(out=ab_ps[:], lhsT=bts[:], rhs=a1b1[:],
                                     start=True, stop=True)
                    hw2 = sb1.tile([128, D], f16, tag='h_in2')
                    nc.sync.dma_start(out=hw2[:], in_=h_slice[s * 128:(s + 1) * 128, :])
                    hnw = sb1.tile([128, D], f16, tag='hnw')
                    nc.vector.tensor_tensor(out=hnw[:], in0=hw2[:], in1=ab_ps[:, :D], op=AO.mult)
                    nc.vector.tensor_tensor(out=hnw[:], in0=hnw[:], in1=ab_ps[:, D:], op=AO.add)
                    stin = nc.sync.dma_start(out=hn_local[s * 128:(s + 1) * 128, :], in_=hnw[:])
                    piece_stores.append(stin)
                    if (s + 1) % (NSW // PIECES) == 0:
                        p = (s + 1) // (NSW // PIECES) - 1
                        cc = nc.gpsimd.collective_compute(
                            'AllGather', AO.bypass,
                            replica_groups=[list(range(NCORES))],
                            ins=[hn_local[p * RPP:(p + 1) * RPP, :]],
                            outs=[hn_full[p * NCORES * RPP:(p + 1) * NCORES * RPP, :]
                                  .rearrange('(c r) d -> c r d', c=NCORES)])
                        for stx in piece_stores:
                            add_dep_helper(cc.ins, stx.ins, True, 'allgather reads hn piece')
                        piece_stores = []
                        cc_hn.append(cc)
            nc.leave_named_scope('ph1_gn1', sc1[0], False)

            # =========== phase 2: relconv (fused QKV) ===========
            sc2 = nc.enter_named_scope('ph2_relconv', False)
            cc_kv = []
            RC_SLOTS = max(len(c) for c in rc_calls)
            with (
                tc.tile_pool(name='p2i', bufs=1) as ip2,
                tc.tile_pool(name='p2g', bufs=2) as gp2,
                tc.tile_pool(name='p2sb', bufs=3) as sb2,
                tc.tile_pool(name='p2S', bufs=1, space='PSUM') as psS,
                tc.tile_pool(name='p2qkv', bufs=1, space='PSUM') as psQ,
                tc.tile_pool(name='p2tr', bufs=1, space='PSUM') as psT,
            ):
                rci = ip2.tile([128, RC_CHUNKS * 8], i16)
                nc.sync.dma_start(out=rci[:], in_=rc_idx[:])
                rc_off = 0
                piece_stores = []
                for w in range(NW):
                    qkv_ps = [psQ.tile([128, 3 * D], f32, tag=f'qkv{i}', name=f'qkv{i}')
                              for i in range(2)]
                    S_ps = [psS.tile([128, 512 if p < 4 else 256], f32,
                                     tag=f'S{p}', name=f'S{p}') for p in range(NPAIR)]
                    # first/last chunk flags per pair across base regions
                    nch_wp = [sum(rc_chunks[(w * PIECES + pc) * NPAIR + p]
                                  for pc in range(PIECES)) for p in range(NPAIR)]
                    done_wp = [0] * NPAIR
                    for hh in range(PIECES):
                        base = hh * BASEQ
                        calls = rc_calls[w * PIECES + hh]
                        gtiles = {}
                        co = rc_off
                        for slot, take in enumerate(calls):
                            gt = gp2.tile([128, 2 * CALL_MAX * D], f16, tag=f'g{slot}')
                            gi = nc.gpsimd.dma_gather(
                                out_ap=gt[:, :take * D].rearrange('p (c e) -> p c e', e=D),
                                in_ap=hn_full[base:NTAB, :],
                                idxs_ap=rci[:, co * 8:(co + take) * 8],
                                num_idxs=take * 128, num_idxs_reg=take * 128,
                                elem_size=D)
                            for cc in cc_hn:
                                add_dep_helper(gi.ins, cc.ins, True, 'gather reads hn')
                            for j in range(take):
                                gtiles[co + j] = (gt, j)
                            co += take
                        # batched one-hot builds, 2 chunks per vector op
                        nch_pc = sum(rc_chunks[(w * PIECES + hh) * NPAIR + p]
                                     for p in range(NPAIR))
                        atiles = {}
                        for k2 in range(0, nch_pc, 2):
                            nb2 = min(2, nch_pc - k2)
                            c0b = rc_off + k2
                            A2 = sb2.tile([128, 2 * 512], f16, tag='A2')
                            nc.vector.tensor_tensor(
                                out=A2[:, :nb2 * 512].rearrange('p (c e) -> p c e', e=512),
                                in0=rck[:, c0b:c0b + nb2]
                                    .rearrange('p (c o) -> p c o', o=1)
                                    .to_broadcast([128, nb2, 512]),
                                in1=iota[:].rearrange('p (o e) -> p o e', o=1)
                                    .to_broadcast([128, nb2, 512]),
                                op=AO.is_equal)
                            for jj in range(nb2):
                                atiles[c0b + jj] = (A2, jj)
                        for p in range(NPAIR):
                            nch = rc_chunks[(w * PIECES + hh) * NPAIR + p]
                            wd = 512 if p < 4 else 256
                            tot_p = nch_wp[p]
                            for k in range(nch):
                                ck = rc_off
                                gt, j = gtiles[ck]
                                At, ja = atiles[ck]
                                nc.tensor.matmul(
                                    out=S_ps[p][:], lhsT=gt[:, j * D:(j + 1) * D],
                                    rhs=At[:, ja * 512:ja * 512 + wd],
                                    start=(done_wp[p] == 0),
                                    stop=(done_wp[p] == tot_p - 1))
                                done_wp[p] += 1
                                rc_off += 1
                    # copy S to sbuf (fp16)
                    stp = []
                    for p in range(NPAIR):
                        wd = 512 if p < 4 else 256
                        stx = sb2.tile([128, 512], f16, tag=f'St{p}')
                        nc.scalar.copy(out=stx[:, :wd], in_=S_ps[p][:])
                        stp.append(stx)
                    # transforms + self-loop + bias + relu
                    for sub in range(2):
                        row0 = w * WIN + sub * 128
                        for r in range(REL):
                            p, par = r // 2, r % 2
                            lhs = stp[p][:, par * 256 + sub * 128: par * 256 + sub * 128 + 128]
                            nc.tensor.matmul(out=qkv_ps[sub][:], lhsT=lhs,
                                             rhs=wrel_sb[:, r * 3 * D:(r + 1) * 3 * D],
                                             start=(r == 0), stop=False)
                        hnw2 = sb2.tile([128, D], f16, tag='hnl')
                        nc.sync.dma_start(out=hnw2[:], in_=hn_local[row0:row0 + 128, :])
                        ht_ps = psT.tile([128, 128], f16, tag='ht')
                        nc.tensor.transpose(out=ht_ps[:], in_=hnw2[:], identity=ident_f[:])
                        ht = sb2.tile([128, 128], f16, tag='htsb')
                        nc.scalar.copy(out=ht[:], in_=ht_ps[:])
                        nc.tensor.matmul(out=qkv_ps[sub][:], lhsT=ht[:], rhs=wloop_sb[:],
                                         start=False, stop=False)
                        nc.tensor.matmul(out=qkv_ps[sub][:], lhsT=ones_f[:], rhs=brqkv_sb[:],
                                         start=False, stop=True)
                        qk = sb2.tile([128, D], b16, tag='qsb')
                        nc.scalar.activation(out=qk[:], in_=qkv_ps[sub][:, :D], func=AF.Relu)
                        nc.sync.dma_start(out=q_local[row0:row0 + 128, :], in_=qk[:])
                        kvsb = sb2.tile([128, 2 * D], b16, tag='kvsb')
                        nc.scalar.activation(out=kvsb[:], in_=qkv_ps[sub][:, D:], func=AF.Relu)
                        stin = nc.sync.dma_start(out=kv_local[row0:row0 + 128, :], in_=kvsb[:])
                        piece_stores.append(stin)
                    if (w + 1) % (NW // PIECES) == 0:
                        p = (w + 1) // (NW // PIECES) - 1
                        cc = nc.gpsimd.collective_compute(
                            'AllGather', AO.bypass,
                            replica_groups=[list(range(NCORES))],
                            ins=[kv_local[p * RPP:(p + 1) * RPP, :]],
                            outs=[kv_full[p * NCORES * RPP:(p + 1) * NCORES * RPP, :]
                                  .rearrange('(c r) d -> c r d', c=NCORES)])
                        for stx in piece_stores:
                            add_dep_helper(cc.ins, stx.ins, True, 'allgather reads kv piece')
                        piece_stores = []
                        cc_kv.append(cc)
            nc.leave_named_scope('ph2_relconv', sc2[0], False)

            # =========== phase 3: attention ===========
            sc3 = nc.enter_named_scope('ph3_attn', False)
            AT_SLOTS = max(len(c) for c in at_calls)
            with (
                tc.tile_pool(name='p3i', bufs=1) as ip3,
                tc.tile_pool(name='p3g', bufs=1) as gp3,
                tc.tile_pool(name='p3sb', bufs=3) as sb3,
                tc.tile_pool(name='p3at', bufs=2, space='PSUM') as psA,
                tc.tile_pool(name='p3wv', bufs=2, space='PSUM') as psW,
                tc.tile_pool(name='p3ep', bufs=1, space='PSUM') as psE,
            ):
                ati = ip3.tile([128, AT_CHUNKS * 8], i16)
                nc.sync.dma_start(out=ati[:], in_=at_idx[:])
                at_off = 0
                for swb in range(NSWB):
                    # gather calls for both halves of this block
                    gtiles = {}
                    co = at_off
                    for hh in range(PIECES):
                        base = hh * BASEQ
                        calls = at_calls[swb * PIECES + hh]
                        for slot, take in enumerate(calls):
                            gt = gp3.tile([128, CALL_MAX * 2 * D], b16,
                                          tag=f'ag{hh}_{slot}')
                            gi = nc.gpsimd.dma_gather(
                                out_ap=gt[:, :take * 2 * D].rearrange('p (c e) -> p c e', e=2 * D),
                                in_ap=kv_full[base:NTAB, :],
                                idxs_ap=ati[:, co * 8:(co + take) * 8],
                                num_idxs=take * 128, num_idxs_reg=take * 128,
                                elem_size=2 * D)
                            for cc in cc_kv:
                                add_dep_helper(gi.ins, cc.ins, True, 'gather reads kv')
                            for j in range(take):
                                gtiles[co + j] = (gt, j)
                            co += take
                    # chunk offsets per (hh, swin)
                    offs = {}
                    o = at_off
                    for hh in range(PIECES):
                        for swin in range(4):
                            n = at_chunks[(swb * PIECES + hh) * 4 + swin]
                            offs[(hh, swin)] = (o, n)
                            o += n
                    at_off = o
                    for swin in range(4):
                        sw = swb * 4 + swin
                        tot_sw = sum(offs[(hh, swin)][1] for hh in range(PIECES))
                        done = 0
                        qwin = sb3.tile([128, D], b16, tag='qwin')
                        nc.sync.dma_start(out=qwin[:], in_=q_local[sw * 128:(sw + 1) * 128, :])
                        wvz = psW.tile([128, D + HEADS], f32, tag='wvz')
                        for hh in range(PIECES):
                            c0, nch = offs[(hh, swin)]
                            k = 0
                            while k < nch:
                                # batch within one gather tile
                                gt0, j0 = gtiles[c0 + k]
                                nb = 1
                                while (nb < NBATCH and k + nb < nch
                                       and gtiles[c0 + k + nb][0] is gt0):
                                    nb += 1
                                A4 = sb3.tile([128, NBATCH * 128], b16, tag='A4')
                                ckb = c0 + k
                                nc.vector.tensor_tensor(
                                    out=A4[:, :nb * 128].rearrange('p (c e) -> p c e', e=128),
                                    in0=atk[:, ckb:ckb + nb]
                                        .rearrange('p (c o) -> p c o', o=1)
                                        .to_broadcast([128, nb, 128]),
                                    in1=iota_b[:].rearrange('p (o e) -> p o e', o=1)
                                        .to_broadcast([128, nb, 128]),
                                    op=AO.is_equal)
                                at4 = psA.tile([128, NBATCH * 128], b16, tag='at4')
                                for i in range(nb):
                                    nc.tensor.transpose(
                                        out=at4[:, i * 128:(i + 1) * 128],
                                        in_=A4[:, i * 128:(i + 1) * 128],
                                        identity=ident_b[:])
                                at4s = sb3.tile([128, NBATCH * 128], b16, tag='at4s')
                                nc.scalar.copy(out=at4s[:, :nb * 128], in_=at4[:, :nb * 128])
                                qd4 = psA.tile([128, NBATCH * 128], f32, tag='qd4')
                                for i in range(nb):
                                    nc.tensor.matmul(
                                        out=qd4[:, i * 128:(i + 1) * 128],
                                        lhsT=at4s[:, i * 128:(i + 1) * 128],
                                        rhs=qwin[:], start=True, stop=True)
                                gtv = gt0[:].rearrange('p (c e) -> p c e', e=2 * D)
                                kq4 = sb3.tile([128, NBATCH * 128], f32, tag='kq4')
                                nc.vector.tensor_tensor(
                                    out=kq4[:, :nb * 128].rearrange('p (c e) -> p c e', e=D),
                                    in0=gtv[:, j0:j0 + nb, :D],
                                    in1=qd4[:, :nb * 128].rearrange('p (c e) -> p c e', e=D),
                                    op=AO.mult)
                                sc4 = sb3.tile([128, NBATCH * HEADS], f32, tag='sc4')
                                nc.vector.reduce_sum(
                                    out=sc4[:, :nb * HEADS],
                                    in_=kq4[:, :nb * 128].rearrange('p (h e) -> p h e', e=DH),
                                    axis=mybir.AxisListType.X)
                                nc.vector.tensor_scalar_min(
                                    out=sc4[:, :nb * HEADS], in0=sc4[:, :nb * HEADS],
                                    scalar1=10.0)
                                vse4 = sb3.tile([128, NBATCH * 136], b16, tag='vse4')
                                vsev = vse4[:].rearrange('p (c e) -> p c e', e=136)
                                nc.scalar.activation(
                                    out=vsev[:, :nb, D:],
                                    in_=sc4[:, :nb * HEADS].rearrange('p (c h) -> p c h', h=HEADS),
                                    func=AF.Exp)
                                nc.vector.tensor_tensor(
                                    out=vsev[:, :nb, :D].rearrange('p c (h e) -> p c h e', e=DH),
                                    in0=gtv[:, j0:j0 + nb, D:].rearrange('p c (h e) -> p c h e', e=DH),
                                    in1=vsev[:, :nb, D:].rearrange('p c (h o) -> p c h o', o=1)
                                        .to_broadcast([128, nb, HEADS, DH]),
                                    op=AO.mult)
                                for i in range(nb):
                                    nc.tensor.matmul(
                                        out=wvz[:], lhsT=A4[:, i * 128:(i + 1) * 128],
                                        rhs=vse4[:, i * 136:(i + 1) * 136],
                                        start=(done == 0), stop=(done == tot_sw - 1))
                                    done += 1
                                k += nb
                        # epilogue for this subwindow
                        zr = sb3.tile([128, HEADS], f32, tag='zr')
                        nc.vector.tensor_scalar_add(out=zr[:], in0=wvz[:, D:], scalar1=1e-6)
                        zrec = sb3.tile([128, HEADS], f32, tag='zrec')
                        nc.vector.reciprocal(out=zrec[:], in_=zr[:])
                        attn = sb3.tile([128, D], b16, tag='attn')
                        nc.vector.tensor_tensor(
                            out=attn[:].rearrange('p (h e) -> p h e', e=DH),
                            in0=wvz[:, :D].rearrange('p (h e) -> p h e', e=DH),
                            in1=zrec[:].rearrange('p (h o) -> p h o', o=1)
                                .to_broadcast([128, HEADS, DH]),
                            op=AO.mult)
                        atr_ps = psE.tile([128, D], b16, tag='atr')
                        nc.tensor.transpose(out=atr_ps[:], in_=attn[:], identity=ident_b[:])
                        atr = sb3.tile([128, D], b16, tag='atrsb')
                        nc.scalar.copy(out=atr[:], in_=atr_ps[:])
                        ho_ps = psE.tile([128, D], f32, tag='ho')
                        nc.tensor.matmul(out=ho_ps[:], lhsT=atr[:], rhs=ow_sb[:],
                                         start=True, stop=False)
                        nc.tensor.matmul(out=ho_ps[:], lhsT=ones_b[:], rhs=bro_sb[:],
                                         start=False, stop=True)
                        hob = hobp.tile([128, D], f16, tag=f'hob_{sw}', name=f'hob_{sw}')
                        nc.scalar.copy(out=hob[:], in_=ho_ps[:])
                        hobt.append(hob)
                        nc.vector.tensor_reduce(out=ln1_ns[:, sw:sw + 1], in_=hob[:],
                                                axis=mybir.AxisListType.X, op=AO.add,
                                                negate=True)
                        hsq2 = sb3.tile([128, D], f32, tag='hsq2')
                        nc.vector.tensor_tensor(out=hsq2[:], in0=hob[:], in1=hob[:],
                                                op=AO.mult)
                        nc.vector.tensor_reduce(out=ln1_sq[:, sw:sw + 1], in_=hsq2[:],
                                                axis=mybir.AxisListType.X, op=AO.add)
            nc.leave_named_scope('ph3_attn', sc3[0], False)

            # =========== phase 4: LN1 finalize + gn2 stats ===========
            sc4 = nc.enter_named_scope('ph4_stats', False)
            with (
                tc.tile_pool(name='p4sb', bufs=3) as sb4,
                tc.tile_pool(name='p4ps', bufs=1, space='PSUM') as ps4,
            ):
                nmu = sb4.tile([128, NSW], f32, tag='nmu')
                nc.vector.tensor_scalar_mul(out=nmu[:], in0=ln1_ns[:], scalar1=1.0 / D)
                e2 = sb4.tile([128, NSW], f32, tag='e2')
                nc.vector.tensor_scalar_mul(out=e2[:], in0=ln1_sq[:], scalar1=1.0 / D)
                msq1 = sb4.tile([128, NSW], f32, tag='msq1')
                nc.vector.tensor_tensor(out=msq1[:], in0=nmu[:], in1=nmu[:], op=AO.mult)
                nc.vector.tensor_tensor(out=msq1[:], in0=e2[:], in1=msq1[:], op=AO.subtract)
                nc.vector.tensor_scalar_add(out=msq1[:], in0=msq1[:], scalar1=1e-5)
                rv1 = sb4.tile([128, NSW], f32, tag='rv1')
                nc.vector.reciprocal(out=rv1[:], in_=msq1[:])
                nc.scalar.activation(out=ln1_rsd[:], in_=rv1[:], func=AF.Sqrt)
                nc.vector.tensor_tensor(out=ln1_nmr[:], in0=nmu[:], in1=ln1_rsd[:], op=AO.mult)
                sum2 = ps4.tile([GMAX, D], f32, tag='sum2')
                sq2 = ps4.tile([GMAX, D], f32, tag='sq2')
                for s in range(NSW):
                    y = sb4.tile([128, D], f16, tag='y4')
                    nc.vector.tensor_scalar(out=y[:], in0=hobt[s][:],
                                            scalar1=ln1_rsd[:, s:s + 1],
                                            scalar2=ln1_nmr[:, s:s + 1],
                                            op0=AO.mult, op1=AO.add)
                    ysq = sb4.tile([128, D], f16, tag='ysq')
                    nc.scalar.activation(out=ysq[:], in_=y[:], func=AF.Square)
                    nc.tensor.matmul(out=sum2[:], lhsT=Bt[s][:], rhs=y[:],
                                     start=(s == 0), stop=(s == NSW - 1))
                    nc.tensor.matmul(out=sq2[:], lhsT=Bt[s][:], rhs=ysq[:],
                                     start=(s == 0), stop=(s == NSW - 1))

                # gn2 finalize with LN1 affine folded in
                S1 = keep4.tile([GMAX, D], f32)
                nc.vector.tensor_tensor(out=S1[:], in0=sum2[:],
                                        in1=invc_sb[:].to_broadcast([GMAX, D]), op=AO.mult)
                S2 = keep4.tile([GMAX, D], f32)
                nc.vector.tensor_tensor(out=S2[:], in0=sq2[:],
                                        in1=invc_sb[:].to_broadcast([GMAX, D]), op=AO.mult)
                g1 = csb['ln1_g16']; b1 = csb['ln1_b16']
                mh = keep4.tile([GMAX, D], f32)
                nc.vector.tensor_tensor(out=mh[:], in0=S1[:], in1=g1[:], op=AO.mult)
                nc.vector.tensor_tensor(out=mh[:], in0=mh[:], in1=b1[:], op=AO.add)
                t1 = keep4.tile([GMAX, D], f32)
                nc.vector.tensor_tensor(out=t1[:], in0=S2[:], in1=g1[:], op=AO.mult)
                nc.vector.tensor_tensor(out=t1[:], in0=t1[:], in1=g1[:], op=AO.mult)
                t2 = keep4.tile([GMAX, D], f32)
                nc.vector.tensor_tensor(out=t2[:], in0=S1[:], in1=g1[:], op=AO.mult)
                nc.vector.tensor_tensor(out=t2[:], in0=t2[:], in1=b1[:], op=AO.mult)
                nc.vector.tensor_scalar_mul(out=t2[:], in0=t2[:], scalar1=2.0)
                nc.vector.tensor_tensor(out=t1[:], in0=t1[:], in1=t2[:], op=AO.add)
                nc.vector.tensor_tensor(out=t2[:], in0=b1[:], in1=b1[:], op=AO.mult)
                nc.vector.tensor_tensor(out=t1[:], in0=t1[:], in1=t2[:], op=AO.add)
                # t1 = E[h1^2]
                msq2 = keep4.tile([GMAX, D], f32)
                nc.vector.tensor_tensor(out=msq2[:], in0=mh[:], in1=mh[:], op=AO.mult)
                nc.vector.tensor_tensor(out=msq2[:], in0=msq2[:], in1=csb['gn2_msfac16'][:], op=AO.mult)
                nc.vector.tensor_tensor(out=t1[:], in0=t1[:], in1=msq2[:], op=AO.subtract)
                nc.vector.tensor_scalar_add(out=t1[:], in0=t1[:], scalar1=1e-6)
                std2 = keep4.tile([GMAX, D], f32)
                nc.scalar.activation(out=std2[:], in_=t1[:], func=AF.Sqrt)
                rstd2 = keep4.tile([GMAX, D], f32)
                nc.vector.reciprocal(out=rstd2[:], in_=std2[:])
                alpha2 = keep4.tile([GMAX, D], f32)
                nc.vector.tensor_tensor(out=alpha2[:], in0=rstd2[:], in1=csb['gn2_w16'][:], op=AO.mult)
                A2 = keep4.tile([GMAX, D], f32)
                nc.vector.tensor_tensor(out=A2[:], in0=alpha2[:], in1=g1[:], op=AO.mult)
                B2 = keep4.tile([GMAX, D], f32)
                nc.vector.tensor_tensor(out=B2[:], in0=mh[:], in1=csb['gn2_ms16'][:], op=AO.mult)
                nc.vector.tensor_tensor(out=B2[:], in0=B2[:], in1=alpha2[:], op=AO.mult)
                nc.vector.tensor_tensor(out=B2[:], in0=csb['gn2_b16'][:], in1=B2[:], op=AO.subtract)
                t3 = keep4.tile([GMAX, D], f32)
                nc.vector.tensor_tensor(out=t3[:], in0=b1[:], in1=alpha2[:], op=AO.mult)
                nc.vector.tensor_tensor(out=B2[:], in0=B2[:], in1=t3[:], op=AO.add)
                a2b2 = keep4.tile([GMAX, 2 * D], f16)
                nc.vector.tensor_copy(out=a2b2[:, :D], in_=A2[:])
                nc.vector.tensor_copy(out=a2b2[:, D:], in_=B2[:])
            nc.leave_named_scope('ph4_stats', sc4[0], False)

            # =========== phase 5: gn2 apply + FFN + LN2 center ===========
            sc5 = nc.enter_named_scope('ph5_ffn', False)
            with (
                tc.tile_pool(name='p5sb', bufs=3) as sb5,
                tc.tile_pool(name='p5ps', bufs=1, space='PSUM') as ps5,
                tc.tile_pool(name='p5ps2', bufs=2, space='PSUM') as ps5b,
            ):
                for s in range(NSW):
                    btp = ps5b.tile([GMAX, 128], f16, tag='btp2')
                    nc.tensor.transpose(out=btp[:], in_=Bt[s][:], identity=ident_f[:])
                    bts = sb5.tile([GMAX, 128], f16, tag='bts2')
                    nc.scalar.copy(out=bts[:], in_=btp[:])
                    ab_ps = ps5b.tile([128, 2 * D], f32, tag='ab2')
                    nc.tensor.matmul(out=ab_ps[:], lhsT=bts[:], rhs=a2b2[:],
                                     start=True, stop=True)
                    y5 = sb5.tile([128, D], f16, tag='y5')
                    nc.vector.tensor_scalar(out=y5[:], in0=hobt[s][:],
                                            scalar1=ln1_rsd[:, s:s + 1],
                                            scalar2=ln1_nmr[:, s:s + 1],
                                            op0=AO.mult, op1=AO.add)
                    h2 = sb5.tile([128, D], f16, tag='h2')
                    nc.vector.tensor_tensor(out=h2[:], in0=y5[:], in1=ab_ps[:, :D], op=AO.mult)
                    nc.vector.tensor_tensor(out=h2[:], in0=h2[:], in1=ab_ps[:, D:], op=AO.add)
                    h2t_ps = ps5.tile([128, D], f16, tag='h2t')
                    nc.tensor.transpose(out=h2t_ps[:], in_=h2[:], identity=ident_f[:])
                    h2tt = sb5.tile([128, D], f16, tag='h2tsb')
                    nc.scalar.copy(out=h2tt[:], in_=h2t_ps[:])
                    f1_ps = ps5.tile([128, 2 * D], f32, tag='f1')
                    nc.tensor.matmul(out=f1_ps[:], lhsT=h2tt[:], rhs=ffn1_sb[:],
                                     start=True, stop=False)
                    nc.tensor.matmul(out=f1_ps[:], lhsT=ones_f[:], rhs=brf1_sb[:],
                                     start=False, stop=True)
                    fr = sb5.tile([128, 2 * D], f16, tag='fr')
                    nc.scalar.activation(out=fr[:], in_=f1_ps[:], func=AF.Relu)
                    frt_ps = ps5.tile([128, 2 * D], f16, tag='frt')
                    nc.tensor.transpose(out=frt_ps[:, :D], in_=fr[:, :D], identity=ident_f[:])
                    nc.tensor.transpose(out=frt_ps[:, D:], in_=fr[:, D:], identity=ident_f[:])
                    frt = sb5.tile([128, 2 * D], f16, tag='frtsb')
                    nc.scalar.copy(out=frt[:], in_=frt_ps[:])
                    h3_ps = ps5.tile([128, D], f32, tag='h3')
                    nc.tensor.matmul(out=h3_ps[:], lhsT=frt[:, :D], rhs=ffn2_sb[:, :D],
                                     start=True, stop=False)
                    nc.tensor.matmul(out=h3_ps[:], lhsT=frt[:, D:], rhs=ffn2_sb[:, D:],
                                     start=False, stop=False)
                    nc.tensor.matmul(out=h3_ps[:], lhsT=ones_f[:], rhs=brf2_sb[:],
                                     start=False, stop=True)
                    # LN2 center + var column
                    nscol = sb5.tile([128, 1], f32, tag='nscol')
                    nc.vector.tensor_reduce(out=nscol[:], in_=h3_ps[:],
                                            axis=mybir.AxisListType.X, op=AO.add,
                                            negate=True)
                    nc.vector.tensor_scalar_mul(out=nscol[:], in0=nscol[:], scalar1=1.0 / D)
                    xc2 = xc2p.tile([128, D], f16, tag=f'xc2_{s}', name=f'xc2_{s}')
                    nc.vector.tensor_scalar(out=xc2[:], in0=h3_ps[:], scalar1=nscol[:],
                                            scalar2=None, op0=AO.add)
                    xc2t.append(xc2)
                    sqt = sb5.tile([128, D], f16, tag='sqt')
                    nc.vector.tensor_tensor(out=sqt[:], in0=xc2[:], in1=xc2[:], op=AO.mult)
                    nc.vector.tensor_reduce(out=ln2_vr[:, s:s + 1], in_=sqt[:],
                                            axis=mybir.AxisListType.X, op=AO.add)
            nc.leave_named_scope('ph5_ffn', sc5[0], False)

            # =========== phase 6: LN2 finalize + output ===========
            sc6 = nc.enter_named_scope('ph6_out', False)
            with tc.tile_pool(name='p6sb', bufs=3) as sb6:
                vr2 = sb6.tile([128, NSW], f32, tag='vr2')
                nc.vector.tensor_scalar(out=vr2[:], in0=ln2_vr[:], scalar1=1.0 / D,
                                        scalar2=1e-5, op0=AO.mult, op1=AO.add)
                rv2 = sb6.tile([128, NSW], f32, tag='rv2')
                nc.vector.reciprocal(out=rv2[:], in_=vr2[:])
                nc.scalar.activation(out=ln2_rsd[:], in_=rv2[:], func=AF.Sqrt)
                for s in range(NSW):
                    ov = sb6.tile([128, D], f32, tag='ov')
                    nc.vector.tensor_scalar(out=ov[:], in0=xc2t[s][:],
                                            scalar1=ln2_rsd[:, s:s + 1],
                                            scalar2=None, op0=AO.mult)
                    nc.vector.tensor_tensor(out=ov[:], in0=ov[:], in1=csb['ln2_g'][:], op=AO.mult)
                    nc.vector.tensor_tensor(out=ov[:], in0=ov[:], in1=csb['ln2_b'][:], op=AO.add)
                    nc.sync.dma_start(out=out_sl[s * 128:(s + 1) * 128, :], in_=ov[:])
            nc.leave_named_scope('ph6_out', sc6[0], False)

            if debug:
                nc.sync.dma_start(out=hn_dbg[:], in_=hn_local[:])
                nc.sync.dma_start(out=kv_dbg[:], in_=kv_local[:])

            k4.__exit__(None, None, None)
            xc2p_cm.__exit__(None, None, None)
            hobp_cm.__exit__(None, None, None)
            bpool_cm.__exit__(None, None, None)

    nc.finalize()
    return nc


def kernel(**inputs) -> np.ndarray:
    _ensure_hooks()
    from concourse.bass_utils import run_bass_kernel_spmd

    static, in_maps, meta = preprocess(inputs)
    key = tuple(sorted((k, str(v)) for k, v in static.items()))
    if key not in _PROGRAM_CACHE:
        _PROGRAM_CACHE[key] = build_program(static)
    nc = _PROGRAM_CACHE[key]

    trace = os.environ.get("KERNEL_TRACE") == "1"
    res = run_bass_kernel_spmd(nc, in_maps, list(range(NCORES)), trace=trace)
    if trace and res.exec_time_ns:
        print("HW exec time:", res.exec_time_ns, "ns")
    out = np.zeros((N_NODES, D), np.float32)
    for c in range(NCORES):
        n0, n1 = int(meta['n0'][c]), int(meta['n1'][c])
        out[n0:n1] = res.results[c]['out_slice'][:n1 - n0]
    return out


# revision 12
# speedup vs baseline: 1.5356x; 1.1755x over previous
"""GTLayer (relational graph transformer layer) on 8 Trainium2 NeuronCores.

v2 strategy (see kernel_v1_baseline.py for the original):
- Nodes partitioned across 8 cores in graph-aligned contiguous slices;
  edges live with the core owning dst. Global gather tables laid out in 4
  row-pieces so allgathers can be fired piecewise and overlapped.
- fp16 data lane for relconv (hn table, gathered rows, W_rel/W_loop, S
  sums), bf16 lane for attention (kv table, one-hots, V*exp) — exp(score)
  products can exceed fp16 range. PSUM accumulation is always f32.
- RelConv: edges per (dst-window 256, src-half, rel-PAIR); S psum tiles are
  [128, 512] covering two relations (one-hot column = dl + 256*parity), so
  all 9 relations fit one PSUM residency and gather call regions merge to
  (window, half).
- One-hots built with tensor_scalar(is_equal, scalar1=key column) against
  an iota ramp (2x DVE mode), not tensor_tensor broadcasts.
- Attention: sw-blocks of 4 subwindows share gather calls; chunks
  processed in batches of <=4 with batched vector ops.
- Scalar engine does psum->sbuf copies (Copy), Relu/Exp/Square, arranged
  so activation tables almost never reload. LN1/LN2 sqrt is batched
  (columns collected across subwindows, one Sqrt instruction each).
- Biases applied as K=1 matmuls (ones-row lhsT) accumulating into PSUM.
- LN1 affine folded into graphNorm2's alpha/beta algebra.
"""
import os
import sys
import types
import numpy as np

NCORES = 8
N_NODES = 100000
N_EDGES = 600000
D = 128
REL = 9
NPAIR = 5
NG = 64
HEADS = 8
DH = 16
WIN = 256          # relconv dst window
GMAX = 16          # max graphs per core
CALL_MAX = 8       # max chunks (of 128 slots) per dma_gather call
PIECES = 4         # allgather pieces; also gather base regions (idx >= 0 always)
NBATCH = 4         # attention chunk batch


def _ensure_hooks():
    if "antenv.axon_hooks" not in sys.modules:
        hooks = types.ModuleType("antenv.axon_hooks")
        h = [None]
        hooks.set_axon_ntff_profile_hook = lambda v: h.__setitem__(0, v)
        hooks.get_axon_ntff_profile_hook = lambda: h[0]
        sys.modules["antenv.axon_hooks"] = hooks
        try:
            from trn_agent_boot.trn_boot import _ntff_profile_via_ctypes
            hooks.set_axon_ntff_profile_hook(
                _ntff_profile_via_ctypes("/opt/axon/libaxon_pjrt.so"))
        except Exception:
            pass


# ----------------------------------------------------------------------------
# Host preprocessing
# ----------------------------------------------------------------------------

def _pack_idx16(idx):
    """int16 index array -> [128, n/16] wrapped+replicated layout."""
    n = len(idx)
    assert n % 16 == 0
    blk = idx.reshape(n // 16, 16).T
    return np.tile(blk, (8, 1)).astype(np.int16)


def _layout_slots(order_edges, idx_vals, key_vals, n_chunks):
    """Place edges into n_chunks*128 slots (full 128 per chunk), pads get
    idx 0, key -1. Returns (idx int32, key f32)."""
    tot = n_chunks * 128
    idx = np.zeros(tot, np.int32)
    key = np.full(tot, -1.0, np.float32)
    ne = len(order_edges)
    assert ne <= tot, (ne, n_chunks)
    idx[:ne] = idx_vals[order_edges]
    key[:ne] = key_vals[order_edges]
    return idx, key


def _calls_for(n, cap):
    calls = []
    n = int(n)
    while n > 0:
        take = min(n, cap)
        calls.append(take)
        n -= take
    return calls


def preprocess(inputs):
    import ml_dtypes
    bf16 = ml_dtypes.bfloat16
    h = np.asarray(inputs['h'], np.float32)
    src = np.asarray(inputs['src']).astype(np.int64)
    dst = np.asarray(inputs['dst']).astype(np.int64)
    et = np.asarray(inputs['etypes']).astype(np.int64)
    seg = np.asarray(inputs['seg']).astype(np.int64)

    # --- graph-aligned node partition ---
    gstart = np.searchsorted(seg, np.arange(NG + 1))
    bounds = [0]
    for c in range(1, NCORES):
        target = c * N_NODES / NCORES
        g = int(np.argmin(np.abs(gstart - target)))
        bounds.append(int(gstart[g]))
    bounds.append(N_NODES)
    n0 = np.array(bounds[:-1]); n1 = np.array(bounds[1:])
    sizes = n1 - n0
    ROUND = max(512, WIN * PIECES)
    P_NODES = int(np.ceil(sizes.max() / ROUND) * ROUND)
    RPP = P_NODES // PIECES
    NW = P_NODES // WIN
    NSW = NW * 2
    assert NSW % 4 == 0
    NSWB = NSW // 4
    NTAB = NCORES * P_NODES
    assert NCORES * RPP <= 32768  # per-piece base region fits int16 idx

    owner = np.searchsorted(n1, np.arange(N_NODES), side='right')
    rloc = np.arange(N_NODES) - n0[owner]
    piece = rloc // RPP
    gpos = piece * (NCORES * RPP) + owner * RPP + (rloc - piece * RPP)

    g0 = np.searchsorted(gstart, n0, side='right') - 1
    counts_g = np.diff(gstart).astype(np.float32)

    BASEQ = NCORES * RPP
    srcp = gpos[src]
    half = srcp // BASEQ          # source piece = gather base region (0..3)
    ecore = owner[dst]
    dst_off = dst - n0[ecore]
    w_e = dst_off // WIN
    pair_e = et // 2
    par_e = et % 2
    key512 = (dst_off % WIN + 256 * par_e).astype(np.float32)
    sw_e = dst_off // 128
    dl128 = (dst_off % 128).astype(np.float32)
    swb_e = sw_e // 4
    swin_e = sw_e % 4
    idx_rel = (srcp - half * BASEQ).astype(np.int32)
    assert idx_rel.min() >= 0 and idx_rel.max() < 32768

    # --- relconv chunk structure: groups q = (w, pc, pair) ---
    NQ = NW * PIECES * NPAIR
    rkey = (w_e * PIECES + half) * NPAIR + pair_e
    rc_counts = np.zeros((NCORES, NQ), np.int64)
    for c in range(NCORES):
        rc_counts[c] = np.bincount(rkey[ecore == c], minlength=NQ)
    rc_chunks = np.ceil(rc_counts / 128.0).max(0).astype(np.int64)
    # ensure >=1 chunk per (w, pair) so S psum gets initialized
    for w in range(NW):
        for p in range(NPAIR):
            qs = [(w * PIECES + pc) * NPAIR + p for pc in range(PIECES)]
            if sum(rc_chunks[q] for q in qs) == 0:
                rc_chunks[qs[0]] = 1
    RC_CHUNKS = int(rc_chunks.sum())
    # call list per (w, pc)
    rc_calls = []
    for w in range(NW):
        for pc in range(PIECES):
            tot = int(sum(rc_chunks[(w * PIECES + pc) * NPAIR + p] for p in range(NPAIR)))
            rc_calls.append(tuple(_calls_for(tot, 2 * CALL_MAX)))

    # --- attention chunk structure: groups aq = (swb, pc, swin) ---
    NAQ = NSWB * PIECES * 4
    akey = (swb_e * PIECES + half) * 4 + swin_e
    at_counts = np.zeros((NCORES, NAQ), np.int64)
    for c in range(NCORES):
        at_counts[c] = np.bincount(akey[ecore == c], minlength=NAQ)
    at_chunks = np.ceil(at_counts / 128.0).max(0).astype(np.int64)
    # ensure every sw has >=1 chunk overall (wvz psum init)
    for sw in range(NSW):
        swb, swin = sw // 4, sw % 4
        aqs = [(swb * PIECES + pc) * 4 + swin for pc in range(PIECES)]
        if sum(at_chunks[a] for a in aqs) == 0:
            at_chunks[aqs[0]] = 1
    AT_CHUNKS = int(at_chunks.sum())
    at_calls = []
    for swb in range(NSWB):
        for pc in range(PIECES):
            tot = int(sum(at_chunks[(swb * PIECES + pc) * 4 + s] for s in range(4)))
            at_calls.append(tuple(_calls_for(tot, CALL_MAX)))

    # --- per-core data arrays ---
    in_maps = []
    for c in range(NCORES):
        m = np.nonzero(ecore == c)[0]
        # order by (group, srcp) for gather locality
        order = np.lexsort((srcp[m], rkey[m]))
        es = m[order]
        rk = rkey[m][order]
        run_s = np.searchsorted(rk, np.arange(NQ))
        run_e = np.searchsorted(rk, np.arange(NQ) + 1)
        rc_idx = np.zeros(RC_CHUNKS * 128, np.int32)
        rc_key = np.full(RC_CHUNKS * 128, -1.0, np.float32)
        coff = 0
        for q in range(NQ):
            nch = int(rc_chunks[q])
            if nch == 0:
                assert run_e[q] == run_s[q]
                continue
            ii, kk = _layout_slots(es[run_s[q]:run_e[q]], idx_rel, key512, nch)
            rc_idx[coff * 128:(coff + nch) * 128] = ii
            rc_key[coff * 128:(coff + nch) * 128] = kk
            coff += nch
        assert coff == RC_CHUNKS

        aorder = np.lexsort((srcp[m], akey[m]))
        aes = m[aorder]
        ak = akey[m][aorder]
        arun_s = np.searchsorted(ak, np.arange(NAQ))
        arun_e = np.searchsorted(ak, np.arange(NAQ) + 1)
        at_idx = np.zeros(AT_CHUNKS * 128, np.int32)
        at_key = np.full(AT_CHUNKS * 128, -1.0, np.float32)
        coff = 0
        for q in range(NAQ):
            nch = int(at_chunks[q])
            if nch == 0:
                assert arun_e[q] == arun_s[q]
                continue
            ii, kk = _layout_slots(aes[arun_s[q]:arun_e[q]], idx_rel, dl128, nch)
            at_idx[coff * 128:(coff + nch) * 128] = ii
            at_key[coff * 128:(coff + nch) * 128] = kk
            coff += nch
        assert coff == AT_CHUNKS

        hs = np.zeros((P_NODES, D), np.float32)
        hs[:sizes[c]] = h[n0[c]:n1[c]]
        segl = np.full(P_NODES, -1.0, np.float32)
        segl[:sizes[c]] = (seg[n0[c]:n1[c]] - g0[c]).astype(np.float32)
        ginc = np.zeros((GMAX, 1), np.float32)
        ng_c = int(seg[n1[c] - 1] - g0[c]) + 1
        assert ng_c <= GMAX
        ginc[:ng_c, 0] = 1.0 / counts_g[g0[c]:g0[c] + ng_c]

        im = {
            'h_slice': hs.astype(np.float16),
            'seg_col': segl.reshape(NSW, 128).T.copy(),
            'inv_cnt': ginc,
            'rc_idx': _pack_idx16(rc_idx.astype(np.int16)),
            'rc_key': rc_key.reshape(RC_CHUNKS, 128).T.copy(),
            'at_idx': _pack_idx16(at_idx.astype(np.int16)),
            'at_key': at_key.reshape(AT_CHUNKS, 128).T.copy(),
        }
        in_maps.append(im)

    # --- shared weights ---
    def A(x):
        return np.ascontiguousarray(np.asarray(x, np.float32))
    Wrel = np.concatenate([
        np.einsum('rb,bio->rio', A(inputs[f'{nm}_coeff']), A(inputs[f'{nm}_basis']))
        for nm in ('q', 'k', 'v')], axis=2)            # [9, 128, 384]
    Wrel[:, :, :D] *= 0.25  # fold score/sqrt(dh) into Q
    wloop = np.concatenate([A(inputs[f'{nm}_loop']) for nm in ('q', 'k', 'v')], 1)
    wloop[:, :D] *= 0.25
    bqkv = np.concatenate([A(inputs[f'{nm}_bias']) for nm in ('q', 'k', 'v')])
    bqkv[:D] *= 0.25
    ffn2p = np.zeros((D, 2 * D), np.float32)           # two K-chunks side by side
    ffn2p[:, :D] = A(inputs['ffn2_w'])[:D, :]
    ffn2p[:, D:] = A(inputs['ffn2_w'])[D:, :]
    w_shared = {
        'w_rel': A(Wrel.reshape(REL * D, 3 * D)).astype(np.float16),
        'w_loop': wloop.astype(np.float16),
        'brow_qkv': bqkv.reshape(1, 3 * D).astype(np.float16),
        'o_w': A(inputs['o_w']).astype(bf16),
        'brow_o': A(inputs['o_b']).reshape(1, D).astype(bf16),
        'ffn1': A(inputs['ffn1_w']).astype(np.float16),
        'brow_f1': A(inputs['ffn1_b']).reshape(1, 2 * D).astype(np.float16),
        'ffn2': ffn2p.astype(np.float16),
        'brow_f2': A(inputs['ffn2_b']).reshape(1, D).astype(np.float16),
        'ln2_g': np.tile(A(inputs['ln2_g'])[None, :], (128, 1)),
        'ln2_b': np.tile(A(inputs['ln2_b'])[None, :], (128, 1)),
    }
    for nm in ('gn1', 'gn2'):
        w = A(inputs[f'{nm}_w']); b = A(inputs[f'{nm}_b']); ms = A(inputs[f'{nm}_ms'])
        w_shared[f'{nm}_w16'] = np.tile(w[None, :], (GMAX, 1))
        w_shared[f'{nm}_b16'] = np.tile(b[None, :], (GMAX, 1))
        w_shared[f'{nm}_ms16'] = np.tile(ms[None, :], (GMAX, 1))
        w_shared[f'{nm}_msfac16'] = np.tile((ms * (2 - ms))[None, :], (GMAX, 1))
    for nm in ('ln1_g', 'ln1_b'):
        w_shared[f'{nm}16'] = np.tile(A(inputs[nm])[None, :], (GMAX, 1))
    for im in in_maps:
        im.update(w_shared)

    static = dict(P_NODES=P_NODES, NW=NW, NSW=NSW, NSWB=NSWB, RPP=RPP,
                  NTAB=NTAB,
                  rc_chunks=tuple(int(x) for x in rc_chunks),
                  at_chunks=tuple(int(x) for x in at_chunks),
                  rc_calls=tuple(rc_calls), at_calls=tuple(at_calls),
                  RC_CHUNKS=RC_CHUNKS, AT_CHUNKS=AT_CHUNKS)
    meta = dict(n0=n0, n1=n1, sizes=sizes)
    return static, in_maps, meta


# ----------------------------------------------------------------------------
# Bass program
# ----------------------------------------------------------------------------

_PROGRAM_CACHE = {}


def build_program(st):
    import concourse.bass as bass
    import concourse.bacc as bacc
    import concourse.mybir as mybir
    import concourse.tile as tile
    from concourse.tile import TileContext
    from concourse.masks import make_identity
    from bass_rust import add_dep_helper

    P_NODES = st['P_NODES']; NW = st['NW']; NSW = st['NSW']
    NSWB = st['NSWB']; RPP = st['RPP']; NTAB = st['NTAB']
    BASEQ = NCORES * RPP
    rc_chunks = st['rc_chunks']; at_chunks = st['at_chunks']
    rc_calls = st['rc_calls']; at_calls = st['at_calls']
    RC_CHUNKS = st['RC_CHUNKS']; AT_CHUNKS = st['AT_CHUNKS']
    f32 = mybir.dt.float32
    f16 = mybir.dt.float16
    b16 = mybir.dt.bfloat16
    i16 = mybir.dt.int16
    AO = mybir.AluOpType
    AF = mybir.ActivationFunctionType

    nc = bacc.Bacc()

    # --- I/O ---
    h_slice = nc.declare_dram_parameter('h_slice', [P_NODES, D], f16, isOutput=False)
    seg_col = nc.declare_dram_parameter('seg_col', [128, NSW], f32, isOutput=False)
    inv_cnt = nc.declare_dram_parameter('inv_cnt', [GMAX, 1], f32, isOutput=False)
    rc_idx = nc.declare_dram_parameter('rc_idx', [128, RC_CHUNKS * 8], i16, isOutput=False)
    rc_keyd = nc.declare_dram_parameter('rc_key', [128, RC_CHUNKS], f32, isOutput=False)
    at_idx = nc.declare_dram_parameter('at_idx', [128, AT_CHUNKS * 8], i16, isOutput=False)
    at_keyd = nc.declare_dram_parameter('at_key', [128, AT_CHUNKS], f32, isOutput=False)
    w_rel = nc.declare_dram_parameter('w_rel', [REL * D, 3 * D], f16, isOutput=False)
    w_loop = nc.declare_dram_parameter('w_loop', [D, 3 * D], f16, isOutput=False)
    brow_qkv = nc.declare_dram_parameter('brow_qkv', [1, 3 * D], f16, isOutput=False)
    o_w = nc.declare_dram_parameter('o_w', [D, D], b16, isOutput=False)
    brow_o = nc.declare_dram_parameter('brow_o', [1, D], b16, isOutput=False)
    ffn1 = nc.declare_dram_parameter('ffn1', [D, 2 * D], f16, isOutput=False)
    brow_f1 = nc.declare_dram_parameter('brow_f1', [1, 2 * D], f16, isOutput=False)
    ffn2 = nc.declare_dram_parameter('ffn2', [D, 2 * D], f16, isOutput=False)
    brow_f2 = nc.declare_dram_parameter('brow_f2', [1, D], f16, isOutput=False)
    cdecl = {}
    for nm in ('ln2_g', 'ln2_b'):
        cdecl[nm] = nc.declare_dram_parameter(nm, [128, D], f32, isOutput=False)
    for nm in ('gn1_w16', 'gn1_b16', 'gn1_ms16', 'gn1_msfac16',
               'gn2_w16', 'gn2_b16', 'gn2_ms16', 'gn2_msfac16',
               'ln1_g16', 'ln1_b16'):
        cdecl[nm] = nc.declare_dram_parameter(nm, [GMAX, D], f32, isOutput=False)
    out_sl = nc.declare_dram_parameter('out_slice', [P_NODES, D], f32, isOutput=True)

    # --- internal DRAM ---
    hn_local = nc.dram_tensor('hn_local', [P_NODES, D], f16)
    q_local = nc.dram_tensor('q_local', [P_NODES, D], b16)
    kv_local = nc.dram_tensor('kv_local', [P_NODES, 2 * D], b16)
    hn_full = nc.dram_tensor('hn_full', [NTAB, D], f16, addr_space='Shared')
    kv_full = nc.dram_tensor('kv_full', [NTAB, 2 * D], b16, addr_space='Shared')
    debug = os.environ.get('KERNEL_DEBUG') == '1'
    if debug:
        hn_dbg = nc.declare_dram_parameter('hn_dbg', [P_NODES, D], f16, isOutput=True)
        kv_dbg = nc.declare_dram_parameter('kv_dbg', [P_NODES, 2 * D], b16, isOutput=True)

    with TileContext(nc) as tc:
        with tc.tile_pool(name='const', bufs=1) as cpool:
            iota = cpool.tile([128, 2 * WIN], f16)
            nc.gpsimd.iota(iota[:], pattern=[[1, 2 * WIN]], base=0,
                           channel_multiplier=0, allow_small_or_imprecise_dtypes=True)
            iota_b = cpool.tile([128, 128], b16)
            nc.gpsimd.iota(iota_b[:], pattern=[[1, 128]], base=0,
                           channel_multiplier=0, allow_small_or_imprecise_dtypes=True)
            ident_f = cpool.tile([128, 128], f16)
            make_identity(nc, ident_f[:])
            ident_b = cpool.tile([128, 128], b16)
            make_identity(nc, ident_b[:])
            ones_f = cpool.tile([1, 128], f16)
            nc.gpsimd.memset(ones_f[:], 1.0)
            ones_b = cpool.tile([1, 128], b16)
            nc.gpsimd.memset(ones_b[:], 1.0)

            segs = cpool.tile([128, NSW], f32)
            nc.sync.dma_start(out=segs[:], in_=seg_col[:])
            rck = cpool.tile([128, RC_CHUNKS], f32)
            nc.sync.dma_start(out=rck[:], in_=rc_keyd[:])
            atk = cpool.tile([128, AT_CHUNKS], f32)
            nc.sync.dma_start(out=atk[:], in_=at_keyd[:])

            wrel_sb = cpool.tile([128, REL * 3 * D], f16)
            for r in range(REL):
                nc.sync.dma_start(out=wrel_sb[:, r * 3 * D:(r + 1) * 3 * D],
                                  in_=w_rel[r * D:(r + 1) * D, :])
            wloop_sb = cpool.tile([128, 3 * D], f16)
            nc.sync.dma_start(out=wloop_sb[:], in_=w_loop[:])
            brqkv_sb = cpool.tile([1, 3 * D], f16)
            nc.sync.dma_start(out=brqkv_sb[:], in_=brow_qkv[:])
            ow_sb = cpool.tile([D, D], b16)
            nc.sync.dma_start(out=ow_sb[:], in_=o_w[:])
            bro_sb = cpool.tile([1, D], b16)
            nc.sync.dma_start(out=bro_sb[:], in_=brow_o[:])
            ffn1_sb = cpool.tile([D, 2 * D], f16)
            nc.sync.dma_start(out=ffn1_sb[:], in_=ffn1[:])
            brf1_sb = cpool.tile([1, 2 * D], f16)
            nc.sync.dma_start(out=brf1_sb[:], in_=brow_f1[:])
            ffn2_sb = cpool.tile([D, 2 * D], f16)
            nc.sync.dma_start(out=ffn2_sb[:], in_=ffn2[:])
            brf2_sb = cpool.tile([1, D], f16)
            nc.sync.dma_start(out=brf2_sb[:], in_=brow_f2[:])
            csb = {}
            for nm, dd in cdecl.items():
                t = cpool.tile(list(dd.shape), f32, tag=f'c_{nm}')
                nc.sync.dma_start(out=t[:], in_=dd[:])
                csb[nm] = t
            invc_sb = cpool.tile([GMAX, 1], f32)
            nc.sync.dma_start(out=invc_sb[:], in_=inv_cnt[:])

            # batched LN stat columns
            ln1_ns = cpool.tile([128, NSW], f32, tag='ln1_ns')
            ln1_sq = cpool.tile([128, NSW], f32, tag='ln1_sq')
            ln1_rsd = cpool.tile([128, NSW], f32, tag='ln1_rsd')
            ln1_nmr = cpool.tile([128, NSW], f32, tag='ln1_nmr')
            ln2_vr = cpool.tile([128, NSW], f32, tag='ln2_vr')
            ln2_rsd = cpool.tile([128, NSW], f32, tag='ln2_rsd')

            # persistent pools
            bpool_cm = tc.tile_pool(name='bpool', bufs=1)
            bpool = bpool_cm.__enter__()
            hobp_cm = tc.tile_pool(name='hobp', bufs=1)
            hobp = hobp_cm.__enter__()
            xc2p_cm = tc.tile_pool(name='xc2p', bufs=1)
            xc2p = xc2p_cm.__enter__()
            k4 = tc.tile_pool(name='p4keep', bufs=1)
            keep4 = k4.__enter__()
            Bt = []      # [128, GMAX] f16 one-hot per subwindow
            hobt = []    # [128, D] f16 attn-out per subwindow
            xc2t = []    # [128, D] f16 LN2-centered per subwindow

            # =========== phase 1: graphNorm1 ===========
            sc1 = nc.enter_named_scope('ph1_gn1', False)
            cc_hn = []
            with (
                tc.tile_pool(name='p1keep', bufs=1) as keep1,
                tc.tile_pool(name='p1sb', bufs=3) as sb1,
                tc.tile_pool(name='p1ps', bufs=1, space='PSUM') as ps1,
                tc.tile_pool(name='p1ps2', bufs=2, space='PSUM') as ps1b,
            ):
                sum_ps = ps1.tile([GMAX, D], f32, tag='sums')
                sq_ps = ps1.tile([GMAX, D], f32, tag='sqs')
                for s in range(NSW):
                    hw = sb1.tile([128, D], f16, tag='h_in')
                    nc.sync.dma_start(out=hw[:], in_=h_slice[s * 128:(s + 1) * 128, :])
                    B = bpool.tile([128, GMAX], f16, tag=f'B_{s}', name=f'B_{s}')
                    nc.vector.tensor_scalar(out=B[:], in0=iota[:, :GMAX],
                                            scalar1=segs[:, s:s + 1], scalar2=None,
                                            op0=AO.is_equal)
                    hsq = sb1.tile([128, D], f16, tag='hsq')
                    nc.scalar.activation(out=hsq[:], in_=hw[:], func=AF.Square)
                    nc.tensor.matmul(out=sum_ps[:], lhsT=B[:], rhs=hw[:],
                                     start=(s == 0), stop=(s == NSW - 1))
                    nc.tensor.matmul(out=sq_ps[:], lhsT=B[:], rhs=hsq[:],
                                     start=(s == 0), stop=(s == NSW - 1))
                    Bt.append(B)
                # finalize -> alpha/beta [GMAX, D] f32
                mean = keep1.tile([GMAX, D], f32)
                nc.vector.tensor_tensor(out=mean[:], in0=sum_ps[:],
                                        in1=invc_sb[:].to_broadcast([GMAX, D]), op=AO.mult)
                ex2 = keep1.tile([GMAX, D], f32)
                nc.vector.tensor_tensor(out=ex2[:], in0=sq_ps[:],
                                        in1=invc_sb[:].to_broadcast([GMAX, D]), op=AO.mult)
                msq = keep1.tile([GMAX, D], f32)
                nc.vector.tensor_tensor(out=msq[:], in0=mean[:], in1=mean[:], op=AO.mult)
                nc.vector.tensor_tensor(out=msq[:], in0=msq[:], in1=csb['gn1_msfac16'][:], op=AO.mult)
                var = keep1.tile([GMAX, D], f32)
                nc.vector.tensor_tensor(out=var[:], in0=ex2[:], in1=msq[:], op=AO.subtract)
                nc.vector.tensor_scalar_add(out=var[:], in0=var[:], scalar1=1e-6)
                std = keep1.tile([GMAX, D], f32)
                nc.scalar.activation(out=std[:], in_=var[:], func=AF.Sqrt)
                rstd = keep1.tile([GMAX, D], f32)
                nc.vector.reciprocal(out=rstd[:], in_=std[:])
                alpha1 = keep1.tile([GMAX, D], f32)
                nc.vector.tensor_tensor(out=alpha1[:], in0=rstd[:], in1=csb['gn1_w16'][:], op=AO.mult)
                beta1 = keep1.tile([GMAX, D], f32)
                nc.vector.tensor_tensor(out=beta1[:], in0=mean[:], in1=csb['gn1_ms16'][:], op=AO.mult)
                nc.vector.tensor_tensor(out=beta1[:], in0=beta1[:], in1=alpha1[:], op=AO.mult)
                nc.vector.tensor_tensor(out=beta1[:], in0=csb['gn1_b16'][:], in1=beta1[:], op=AO.subtract)
                a1b1 = keep1.tile([GMAX, 2 * D], f16)
                nc.vector.tensor_copy(out=a1b1[:, :D], in_=alpha1[:])
                nc.vector.tensor_copy(out=a1b1[:, D:], in_=beta1[:])
                # apply
                piece_stores = []
                for s in range(NSW):
                    btp = ps1b.tile([GMAX, 128], f16, tag='btp')
                    nc.tensor.transpose(out=btp[:], in_=Bt[s][:], identity=ident_f[:])
                    bts = sb1.tile([GMAX, 128], f16, tag='bts')
                    nc.scalar.copy(out=bts[:], in_=btp[:])
                    ab_ps = ps1b.tile([128, 2 * D], f32, tag='ab')
                    nc.tensor.matmul(out=ab_ps[:], lhsT=bts[:], rhs=a1b1[:],
                                     start=True, stop=True)
                    hw2 = sb1.tile([128, D], f16, tag='h_in2')
                    nc.sync.dma_start(out=hw2[:], in_=h_slice[s * 128:(s + 1) * 128, :])
                    hnw = sb1.tile([128, D], f16, tag='hnw')
                    nc.vector.tensor_tensor(out=hnw[:], in0=hw2[:], in1=ab_ps[:, :D], op=AO.mult)
                    nc.vector.tensor_tensor(out=hnw[:], in0=hnw[:], in1=ab_ps[:, D:], op=AO.add)
                    stin = nc.sync.dma_start(out=hn_local[s * 128:(s + 1) * 128, :], in_=hnw[:])
                    piece_stores.append(stin)
                    if (s + 1) % (NSW // PIECES) == 0:
                        p = (s + 1) // (NSW // PIECES) - 1
                        cc = nc.gpsimd.collective_compute(
                            'AllGather', AO.bypass,
                            replica_groups=[list(range(NCORES))],
                            ins=[hn_local[p * RPP:(p + 1) * RPP, :]],
                            outs=[hn_full[p * NCORES * RPP:(p + 1) * NCORES * RPP, :]
                                  .rearrange('(c r) d -> c r d', c=NCORES)])
                        for stx in piece_stores:
                            add_dep_helper(cc.ins, stx.ins, True, 'allgather reads hn piece')
                        piece_stores = []
                        cc_hn.append(cc)
            nc.leave_named_scope('ph1_gn1', sc1[0], False)

            # =========== phase 2: relconv (fused QKV) ===========
            sc2 = nc.enter_named_scope('ph2_relconv', False)
            cc_kv = []
            RC_SLOTS = max(len(c) for c in rc_calls)
            with (
                tc.tile_pool(name='p2i', bufs=1) as ip2,
                tc.tile_pool(name='p2g', bufs=3) as gp2,
                tc.tile_pool(name='p2sb', bufs=3) as sb2,
                tc.tile_pool(name='p2S', bufs=1, space='PSUM') as psS,
                tc.tile_pool(name='p2qkv', bufs=1, space='PSUM') as psQ,
                tc.tile_pool(name='p2tr', bufs=1, space='PSUM') as psT,
            ):
                rci = ip2.tile([128, RC_CHUNKS * 8], i16)
                nc.sync.dma_start(out=rci[:], in_=rc_idx[:])
                rc_off = 0
                piece_stores = []
                for w in range(NW):
                    qkv_ps = [psQ.tile([128, 3 * D], f32, tag=f'qkv{i}', name=f'qkv{i}')
                              for i in range(2)]
                    S_ps = [psS.tile([128, 512 if p < 4 else 256], f32,
                                     tag=f'S{p}', name=f'S{p}') for p in range(NPAIR)]
                    # first/last chunk flags per pair across base regions
                    nch_wp = [sum(rc_chunks[(w * PIECES + pc) * NPAIR + p]
                                  for pc in range(PIECES)) for p in range(NPAIR)]
                    done_wp = [0] * NPAIR
                    for hh in range(PIECES):
                        base = hh * BASEQ
                        calls = rc_calls[w * PIECES + hh]
                        gtiles = {}
                        co = rc_off
                        for slot, take in enumerate(calls):
                            gt = gp2.tile([128, 2 * CALL_MAX * D], f16, tag=f'g{slot}')
                            gi = nc.gpsimd.dma_gather(
                                out_ap=gt[:, :take * D].rearrange('p (c e) -> p c e', e=D),
                                in_ap=hn_full[base:NTAB, :],
                                idxs_ap=rci[:, co * 8:(co + take) * 8],
                                num_idxs=take * 128, num_idxs_reg=take * 128,
                                elem_size=D)
                            for cc in cc_hn:
                                add_dep_helper(gi.ins, cc.ins, True, 'gather reads hn')
                            for j in range(take):
                                gtiles[co + j] = (gt, j)
                            co += take
                        # batched one-hot builds, 2 chunks per vector op
                        nch_pc = sum(rc_chunks[(w * PIECES + hh) * NPAIR + p]
                                     for p in range(NPAIR))
                        atiles = {}
                        for k2 in range(0, nch_pc, 2):
                            nb2 = min(2, nch_pc - k2)
                            c0b = rc_off + k2
                            A2 = sb2.tile([128, 2 * 512], f16, tag='A2')
                            nc.vector.tensor_tensor(
                                out=A2[:, :nb2 * 512].rearrange('p (c e) -> p c e', e=512),
                                in0=rck[:, c0b:c0b + nb2]
                                    .rearrange('p (c o) -> p c o', o=1)
                                    .to_broadcast([128, nb2, 512]),
                                in1=iota[:].rearrange('p (o e) -> p o e', o=1)
                                    .to_broadcast([128, nb2, 512]),
                                op=AO.is_equal)
                            for jj in range(nb2):
                                atiles[c0b + jj] = (A2, jj)
                        for p in range(NPAIR):
                            nch = rc_chunks[(w * PIECES + hh) * NPAIR + p]
                            wd = 512 if p < 4 else 256
                            tot_p = nch_wp[p]
                            for k in range(nch):
                                ck = rc_off
                                gt, j = gtiles[ck]
                                At, ja = atiles[ck]
                                nc.tensor.matmul(
                                    out=S_ps[p][:], lhsT=gt[:, j * D:(j + 1) * D],
                                    rhs=At[:, ja * 512:ja * 512 + wd],
                                    start=(done_wp[p] == 0),
                                    stop=(done_wp[p] == tot_p - 1))
                                done_wp[p] += 1
                                rc_off += 1
                    # copy S to sbuf (fp16)
                    stp = []
                    for p in range(NPAIR):
                        wd = 512 if p < 4 else 256
                        stx = sb2.tile([128, 512], f16, tag=f'St{p}')
                        nc.scalar.copy(out=stx[:, :wd], in_=S_ps[p][:])
                        stp.append(stx)
                    # transforms + self-loop + bias + relu
                    for sub in range(2):
                        row0 = w * WIN + sub * 128
                        for r in range(REL):
                            p, par = r // 2, r % 2
                            lhs = stp[p][:, par * 256 + sub * 128: par * 256 + sub * 128 + 128]
                            nc.tensor.matmul(out=qkv_ps[sub][:], lhsT=lhs,
                                             rhs=wrel_sb[:, r * 3 * D:(r + 1) * 3 * D],
                                             start=(r == 0), stop=False)
                        hnw2 = sb2.tile([128, D], f16, tag='hnl')
                        nc.sync.dma_start(out=hnw2[:], in_=hn_local[row0:row0 + 128, :])
                        ht_ps = psT.tile([128, 128], f16, tag='ht')
                        nc.tensor.transpose(out=ht_ps[:], in_=hnw2[:], identity=ident_f[:])
                        ht = sb2.tile([128, 128], f16, tag='htsb')
                        nc.scalar.copy(out=ht[:], in_=ht_ps[:])
                        nc.tensor.matmul(out=qkv_ps[sub][:], lhsT=ht[:], rhs=wloop_sb[:],
                                         start=False, stop=False)
                        nc.tensor.matmul(out=qkv_ps[sub][:], lhsT=ones_f[:], rhs=brqkv_sb[:],
                                         start=False, stop=True)
                        qk = sb2.tile([128, D], b16, tag='qsb')
                        nc.scalar.activation(out=qk[:], in_=qkv_ps[sub][:, :D], func=AF.Relu)
                        nc.sync.dma_start(out=q_local[row0:row0 + 128, :], in_=qk[:])
                        kvsb = sb2.tile([128, 2 * D], b16, tag='kvsb')
                        nc.scalar.activation(out=kvsb[:], in_=qkv_ps[sub][:, D:], func=AF.Relu)
                        stin = nc.sync.dma_start(out=kv_local[row0:row0 + 128, :], in_=kvsb[:])
                        piece_stores.append(stin)
                    if (w + 1) % (NW // PIECES) == 0:
                        p = (w + 1) // (NW // PIECES) - 1
                        cc = nc.gpsimd.collective_compute(
                            'AllGather', AO.bypass,
                            replica_groups=[list(range(NCORES))],
                            ins=[kv_local[p * RPP:(p + 1) * RPP, :]],
                            outs=[kv_full[p * NCORES * RPP:(p + 1) * NCORES * RPP, :]
                                  .rearrange('(c r) d -> c r d', c=NCORES)])
                        for stx in piece_stores:
                            add_dep_helper(cc.ins, stx.ins, True, 'allgather reads kv piece')
                        piece_stores = []
                        cc_kv.append(cc)
            nc.leave_named_scope('ph2_relconv', sc2[0], False)

            # =========== phase 3: attention ===========
            sc3 = nc.enter_named_scope('ph3_attn', False)
            AT_SLOTS = max(len(c) for c in at_calls)
            with (
                tc.tile_pool(name='p3i', bufs=1) as ip3,
                tc.tile_pool(name='p3g', bufs=2) as gp3,
                tc.tile_pool(name='p3sb', bufs=3) as sb3,
                tc.tile_pool(name='p3at', bufs=2, space='PSUM') as psA,
                tc.tile_pool(name='p3wv', bufs=2, space='PSUM') as psW,
                tc.tile_pool(name='p3ep', bufs=1, space='PSUM') as psE,
            ):
                ati = ip3.tile([128, AT_CHUNKS * 8], i16)
                nc.sync.dma_start(out=ati[:], in_=at_idx[:])
                at_off = 0
                for swb in range(NSWB):
                    # gather calls for both halves of this block
                    gtiles = {}
                    co = at_off
                    for hh in range(PIECES):
                        base = hh * BASEQ
                        calls = at_calls[swb * PIECES + hh]
                        for slot, take in enumerate(calls):
                            gt = gp3.tile([128, CALL_MAX * 2 * D], b16,
                                          tag=f'ag{hh}_{slot}')
                            gi = nc.gpsimd.dma_gather(
                                out_ap=gt[:, :take * 2 * D].rearrange('p (c e) -> p c e', e=2 * D),
                                in_ap=kv_full[base:NTAB, :],
                                idxs_ap=ati[:, co * 8:(co + take) * 8],
                                num_idxs=take * 128, num_idxs_reg=take * 128,
                                elem_size=2 * D)
                            for cc in cc_kv:
                                add_dep_helper(gi.ins, cc.ins, True, 'gather reads kv')
                            for j in range(take):
                                gtiles[co + j] = (gt, j)
                            co += take
                    # chunk offsets per (hh, swin)
                    offs = {}
                    o = at_off
                    for hh in range(PIECES):
                        for swin in range(4):
                            n = at_chunks[(swb * PIECES + hh) * 4 + swin]
                            offs[(hh, swin)] = (o, n)
                            o += n
                    at_off = o
                    for swin in range(4):
                        sw = swb * 4 + swin
                        tot_sw = sum(offs[(hh, swin)][1] for hh in range(PIECES))
                        done = 0
                        qwin = sb3.tile([128, D], b16, tag='qwin')
                        nc.sync.dma_start(out=qwin[:], in_=q_local[sw * 128:(sw + 1) * 128, :])
                        wvz = psW.tile([128, D + HEADS], f32, tag='wvz')
                        for hh in range(PIECES):
                            c0, nch = offs[(hh, swin)]
                            k = 0
                            while k < nch:
                                # batch within one gather tile
                                gt0, j0 = gtiles[c0 + k]
                                nb = 1
                                while (nb < NBATCH and k + nb < nch
                                       and gtiles[c0 + k + nb][0] is gt0):
                                    nb += 1
                                A4 = sb3.tile([128, NBATCH * 128], b16, tag='A4')
                                ckb = c0 + k
                                nc.vector.tensor_tensor(
                                    out=A4[:, :nb * 128].rearrange('p (c e) -> p c e', e=128),
                                    in0=atk[:, ckb:ckb + nb]
                                        .rearrange('p (c o) -> p c o', o=1)
                                        .to_broadcast([128, nb, 128]),
                                    in1=iota_b[:].rearrange('p (o e) -> p o e', o=1)
                                        .to_broadcast([128, nb, 128]),
                                    op=AO.is_equal)
                                at4 = psA.tile([128, NBATCH * 128], b16, tag='at4')
                                for i in range(nb):
                                    nc.tensor.transpose(
                                        out=at4[:, i * 128:(i + 1) * 128],
                                        in_=A4[:, i * 128:(i + 1) * 128],
                                        identity=ident_b[:])
                                at4s = sb3.tile([128, NBATCH * 128], b16, tag='at4s')
                                nc.scalar.copy(out=at4s[:, :nb * 128], in_=at4[:, :nb * 128])
                                qd4 = psA.tile([128, NBATCH * 128], f32, tag='qd4')
                                for i in range(nb):
                                    nc.tensor.matmul(
                                        out=qd4[:, i * 128:(i + 1) * 128],
                                        lhsT=at4s[:, i * 128:(i + 1) * 128],
                                        rhs=qwin[:], start=True, stop=True)
                                gtv = gt0[:].rearrange('p (c e) -> p c e', e=2 * D)
                                kq4 = sb3.tile([128, NBATCH * 128], f32, tag='kq4')
                                nc.vector.tensor_tensor(
                                    out=kq4[:, :nb * 128].rearrange('p (c e) -> p c e', e=D),
                                    in0=gtv[:, j0:j0 + nb, :D],
                                    in1=qd4[:, :nb * 128].rearrange('p (c e) -> p c e', e=D),
                                    op=AO.mult)
                                sc4 = sb3.tile([128, NBATCH * HEADS], f32, tag='sc4')
                                nc.vector.reduce_sum(
                                    out=sc4[:, :nb * HEADS],
                                    in_=kq4[:, :nb * 128].rearrange('p (h e) -> p h e', e=DH),
                                    axis=mybir.AxisListType.X)
                                nc.vector.tensor_scalar_min(
                                    out=sc4[:, :nb * HEADS], in0=sc4[:, :nb * HEADS],
                                    scalar1=10.0)
                                vse4 = sb3.tile([128, NBATCH * 136], b16, tag='vse4')
                                vsev = vse4[:].rearrange('p (c e) -> p c e', e=136)
                                nc.scalar.activation(
                                    out=vsev[:, :nb, D:],
                                    in_=sc4[:, :nb * HEADS].rearrange('p (c h) -> p c h', h=HEADS),
                                    func=AF.Exp)
                                nc.vector.tensor_tensor(
                                    out=vsev[:, :nb, :D].rearrange('p c (h e) -> p c h e', e=DH),
                                    in0=gtv[:, j0:j0 + nb, D:].rearrange('p c (h e) -> p c h e', e=DH),
                                    in1=vsev[:, :nb, D:].rearrange('p c (h o) -> p c h o', o=1)
                                        .to_broadcast([128, nb, HEADS, DH]),
                                    op=AO.mult)
                                for i in range(nb):
                                    nc.tensor.matmul(
                                        out=wvz[:], lhsT=A4[:, i * 128:(i + 1) * 128],
                                        rhs=vse4[:, i * 136:(i + 1) * 136],
                                        start=(done == 0), stop=(done == tot_sw - 1))
                                    done += 1
                                k += nb
                        # epilogue for this subwindow
                        zr = sb3.tile([128, HEADS], f32, tag='zr')
                        nc.vector.tensor_scalar_add(out=zr[:], in0=wvz[:, D:], scalar1=1e-6)
                        zrec = sb3.tile([128, HEADS], f32, tag='zrec')
                        nc.vector.reciprocal(out=zrec[:], in_=zr[:])
                        attn = sb3.tile([128, D], b16, tag='attn')
                        nc.vector.tensor_tensor(
                            out=attn[:].rearrange('p (h e) -> p h e', e=DH),
                            in0=wvz[:, :D].rearrange('p (h e) -> p h e', e=DH),
                            in1=zrec[:].rearrange('p (h o) -> p h o', o=1)
                                .to_broadcast([128, HEADS, DH]),
                            op=AO.mult)
                        atr_ps = psE.tile([128, D], b16, tag='atr')
                        nc.tensor.transpose(out=atr_ps[:], in_=attn[:], identity=ident_b[:])
                        atr = sb3.tile([128, D], b16, tag='atrsb')
                        nc.scalar.copy(out=atr[:], in_=atr_ps[:])
                        ho_ps = psE.tile([128, D], f32, tag='ho')
                        nc.tensor.matmul(out=ho_ps[:], lhsT=atr[:], rhs=ow_sb[:],
                                         start=True, stop=False)
                        nc.tensor.matmul(out=ho_ps[:], lhsT=ones_b[:], rhs=bro_sb[:],
                                         start=False, stop=True)
                        hob = hobp.tile([128, D], f16, tag=f'hob_{sw}', name=f'hob_{sw}')
                        nc.scalar.copy(out=hob[:], in_=ho_ps[:])
                        hobt.append(hob)
                        nc.vector.tensor_reduce(out=ln1_ns[:, sw:sw + 1], in_=hob[:],
                                                axis=mybir.AxisListType.X, op=AO.add,
                                                negate=True)
                        hsq2 = sb3.tile([128, D], f32, tag='hsq2')
                        nc.vector.tensor_tensor(out=hsq2[:], in0=hob[:], in1=hob[:],
                                                op=AO.mult)
                        nc.vector.tensor_reduce(out=ln1_sq[:, sw:sw + 1], in_=hsq2[:],
                                                axis=mybir.AxisListType.X, op=AO.add)
            nc.leave_named_scope('ph3_attn', sc3[0], False)

            # =========== phase 4: LN1 finalize + gn2 stats ===========
            sc4 = nc.enter_named_scope('ph4_stats', False)
            with (
                tc.tile_pool(name='p4sb', bufs=3) as sb4,
                tc.tile_pool(name='p4ps', bufs=1, space='PSUM') as ps4,
            ):
                nmu = sb4.tile([128, NSW], f32, tag='nmu')
                nc.vector.tensor_scalar_mul(out=nmu[:], in0=ln1_ns[:], scalar1=1.0 / D)
                e2 = sb4.tile([128, NSW], f32, tag='e2')
                nc.vector.tensor_scalar_mul(out=e2[:], in0=ln1_sq[:], scalar1=1.0 / D)
                msq1 = sb4.tile([128, NSW], f32, tag='msq1')
                nc.vector.tensor_tensor(out=msq1[:], in0=nmu[:], in1=nmu[:], op=AO.mult)
                nc.vector.tensor_tensor(out=msq1[:], in0=e2[:], in1=msq1[:], op=AO.subtract)
                nc.vector.tensor_scalar_add(out=msq1[:], in0=msq1[:], scalar1=1e-5)
                rv1 = sb4.tile([128, NSW], f32, tag='rv1')
                nc.vector.reciprocal(out=rv1[:], in_=msq1[:])
                nc.scalar.activation(out=ln1_rsd[:], in_=rv1[:], func=AF.Sqrt)
                nc.vector.tensor_tensor(out=ln1_nmr[:], in0=nmu[:], in1=ln1_rsd[:], op=AO.mult)
                sum2 = ps4.tile([GMAX, D], f32, tag='sum2')
                sq2 = ps4.tile([GMAX, D], f32, tag='sq2')
                for s in range(NSW):
                    y = sb4.tile([128, D], f16, tag='y4')
                    nc.vector.tensor_scalar(out=y[:], in0=hobt[s][:],
                                            scalar1=ln1_rsd[:, s:s + 1],
                                            scalar2=ln1_nmr[:, s:s + 1],
                                            op0=AO.mult, op1=AO.add)
                    ysq = sb4.tile([128, D], f16, tag='ysq')
                    nc.scalar.activation(out=ysq[:], in_=y[:], func=AF.Square)
                    nc.tensor.matmul(out=sum2[:], lhsT=Bt[s][:], rhs=y[:],
                                     start=(s == 0), stop=(s == NSW - 1))
                    nc.tensor.matmul(out=sq2[:], lhsT=Bt[s][:], rhs=ysq[:],
                                     start=(s == 0), stop=(s == NSW - 1))

                # gn2 finalize with LN1 affine folded in
                S1 = keep4.tile([GMAX, D], f32)
                nc.vector.tensor_tensor(out=S1[:], in0=sum2[:],
                                        in1=invc_sb[:].to_broadcast([GMAX, D]), op=AO.mult)
                S2 = keep4.tile([GMAX, D], f32)
                nc.vector.tensor_tensor(out=S2[:], in0=sq2[:],
                                        in1=invc_sb[:].to_broadcast([GMAX, D]), op=AO.mult)
                g1 = csb['ln1_g16']; b1 = csb['ln1_b16']
                mh = keep4.tile([GMAX, D], f32)
                nc.vector.tensor_tensor(out=mh[:], in0=S1[:], in1=g1[:], op=AO.mult)
                nc.vector.tensor_tensor(out=mh[:], in0=mh[:], in1=b1[:], op=AO.add)
                t1 = keep4.tile([GMAX, D], f32)
                nc.vector.tensor_tensor(out=t1[:], in0=S2[:], in1=g1[:], op=AO.mult)
                nc.vector.tensor_tensor(out=t1[:], in0=t1[:], in1=g1[:], op=AO.mult)
                t2 = keep4.tile([GMAX, D], f32)
                nc.vector.tensor_tensor(out=t2[:], in0=S1[:], in1=g1[:], op=AO.mult)
                nc.vector.tensor_tensor(out=t2[:], in0=t2[:], in1=b1[:], op=AO.mult)
                nc.vector.tensor_scalar_mul(out=t2[:], in0=t2[:], scalar1=2.0)
                nc.vector.tensor_tensor(out=t1[:], in0=t1[:], in1=t2[:], op=AO.add)
                nc.vector.tensor_tensor(out=t2[:], in0=b1[:], in1=b1[:], op=AO.mult)
                nc.vector.tensor_tensor(out=t1[:], in0=t1[:], in1=t2[:], op=AO.add)
                # t1 = E[h1^2]
                msq2 = keep4.tile([GMAX, D], f32)
                nc.vector.tensor_tensor(out=msq2[:], in0=mh[:], in1=mh[:], op=AO.mult)
                nc.vector.tensor_tensor(out=msq2[:], in0=msq2[:], in1=csb['gn2_msfac16'][:], op=AO.mult)
                nc.vector.tensor_tensor(out=t1[:], in0=t1[:], in1=msq2[:], op=AO.subtract)
                nc.vector.tensor_scalar_add(out=t1[:], in0=t1[:], scalar1=1e-6)
                std2 = keep4.tile([GMAX, D], f32)
                nc.scalar.activation(out=std2[:], in_=t1[:], func=AF.Sqrt)
                rstd2 = keep4.tile([GMAX, D], f32)
                nc.vector.reciprocal(out=rstd2[:], in_=std2[:])
                alpha2 = keep4.tile([GMAX, D], f32)
                nc.vector.tensor_tensor(out=alpha2[:], in0=rstd2[:], in1=csb['gn2_w16'][:], op=AO.mult)
                A2 = keep4.tile([GMAX, D], f32)
                nc.vector.tensor_tensor(out=A2[:], in0=alpha2[:], in1=g1[:], op=AO.mult)
                B2 = keep4.tile([GMAX, D], f32)
                nc.vector.tensor_tensor(out=B2[:], in0=mh[:], in1=csb['gn2_ms16'][:], op=AO.mult)
                nc.vector.tensor_tensor(out=B2[:], in0=B2[:], in1=alpha2[:], op=AO.mult)
                nc.vector.tensor_tensor(out=B2[:], in0=csb['gn2_b16'][:], in1=B2[:], op=AO.subtract)
                t3 = keep4.tile([GMAX, D], f32)
                nc.vector.tensor_tensor(out=t3[:], in0=b1[:], in1=alpha2[:], op=AO.mult)
                nc.vector.tensor_tensor(out=B2[:], in0=B2[:], in1=t3[:], op=AO.add)
                a2b2 = keep4.tile([GMAX, 2 * D], f16)
                nc.vector.tensor_copy(out=a2b2[:, :D], in_=A2[:])
                nc.vector.tensor_copy(out=a2b2[:, D:], in_=B2[:])
            nc.leave_named_scope('ph4_stats', sc4[0], False)

            # =========== phase 5: gn2 apply + FFN + LN2 center ===========
            sc5 = nc.enter_named_scope('ph5_ffn', False)
            with (
                tc.tile_pool(name='p5sb', bufs=3) as sb5,
                tc.tile_pool(name='p5ps', bufs=1, space='PSUM') as ps5,
                tc.tile_pool(name='p5ps2', bufs=2, space='PSUM') as ps5b,
            ):
                for s in range(NSW):
                    btp = ps5b.tile([GMAX, 128], f16, tag='btp2')
                    nc.tensor.transpose(out=btp[:], in_=Bt[s][:], identity=ident_f[:])
                    bts = sb5.tile([GMAX, 128], f16, tag='bts2')
                    nc.scalar.copy(out=bts[:], in_=btp[:])
                    ab_ps = ps5b.tile([128, 2 * D], f32, tag='ab2')
                    nc.tensor.matmul(out=ab_ps[:], lhsT=bts[:], rhs=a2b2[:],
                                     start=True, stop=True)
                    y5 = sb5.tile([128, D], f16, tag='y5')
                    nc.vector.tensor_scalar(out=y5[:], in0=hobt[s][:],
                                            scalar1=ln1_rsd[:, s:s + 1],
                                            scalar2=ln1_nmr[:, s:s + 1],
                                            op0=AO.mult, op1=AO.add)
                    h2 = sb5.tile([128, D], f16, tag='h2')
                    nc.vector.tensor_tensor(out=h2[:], in0=y5[:], in1=ab_ps[:, :D], op=AO.mult)
                    nc.vector.tensor_tensor(out=h2[:], in0=h2[:], in1=ab_ps[:, D:], op=AO.add)
                    h2t_ps = ps5.tile([128, D], f16, tag='h2t')
                    nc.tensor.transpose(out=h2t_ps[:], in_=h2[:], identity=ident_f[:])
                    h2tt = sb5.tile([128, D], f16, tag='h2tsb')
                    nc.scalar.copy(out=h2tt[:], in_=h2t_ps[:])
                    f1_ps = ps5.tile([128, 2 * D], f32, tag='f1')
                    nc.tensor.matmul(out=f1_ps[:], lhsT=h2tt[:], rhs=ffn1_sb[:],
                                     start=True, stop=False)
                    nc.tensor.matmul(out=f1_ps[:], lhsT=ones_f[:], rhs=brf1_sb[:],
                                     start=False, stop=True)
                    fr = sb5.tile([128, 2 * D], f16, tag='fr')
                    nc.scalar.activation(out=fr[:], in_=f1_ps[:], func=AF.Relu)
                    frt_ps = ps5.tile([128, 2 * D], f16, tag='frt')
                    nc.tensor.transpose(out=frt_ps[:, :D], in_=fr[:, :D], identity=ident_f[:])
                    nc.tensor.transpose(out=frt_ps[:, D:], in_=fr[:, D:], identity=ident_f[:])
                    frt = sb5.tile([128, 2 * D], f16, tag='frtsb')
                    nc.scalar.copy(out=frt[:], in_=frt_ps[:])
                    h3_ps = ps5.tile([128, D], f32, tag='h3')
                    nc.tensor.matmul(out=h3_ps[:], lhsT=frt[:, :D], rhs=ffn2_sb[:, :D],
                                     start=True, stop=False)
                    nc.tensor.matmul(out=h3_ps[:], lhsT=frt[:, D:], rhs=ffn2_sb[:, D:],
                                     start=False, stop=False)
                    nc.tensor.matmul(out=h3_ps[:], lhsT=ones_f[:], rhs=brf2_sb[:],
                                     start=False, stop=True)
                    # LN2 center + var column
                    nscol = sb5.tile([128, 1], f32, tag='nscol')
                    nc.vector.tensor_reduce(out=nscol[:], in_=h3_ps[:],
                                            axis=mybir.AxisListType.X, op=AO.add,
                                            negate=True)
                    nc.vector.tensor_scalar_mul(out=nscol[:], in0=nscol[:], scalar1=1.0 / D)
                    xc2 = xc2p.tile([128, D], f16, tag=f'xc2_{s}', name=f'xc2_{s}')
                    nc.vector.tensor_scalar(out=xc2[:], in0=h3_ps[:], scalar1=nscol[:],
                                            scalar2=None, op0=AO.add)
                    xc2t.append(xc2)
                    sqt = sb5.tile([128, D], f16, tag='sqt')
                    nc.vector.tensor_tensor(out=sqt[:], in0=xc2[:], in1=xc2[:], op=AO.mult)
                    nc.vector.tensor_reduce(out=ln2_vr[:, s:s + 1], in_=sqt[:],
                                            axis=mybir.AxisListType.X, op=AO.add)
            nc.leave_named_scope('ph5_ffn', sc5[0], False)

            # =========== phase 6: LN2 finalize + output ===========
            sc6 = nc.enter_named_scope('ph6_out', False)
            with tc.tile_pool(name='p6sb', bufs=3) as sb6:
                vr2 = sb6.tile([128, NSW], f32, tag='vr2')
                nc.vector.tensor_scalar(out=vr2[:], in0=ln2_vr[:], scalar1=1.0 / D,
                                        scalar2=1e-5, op0=AO.mult, op1=AO.add)
                rv2 = sb6.tile([128, NSW], f32, tag='rv2')
                nc.vector.reciprocal(out=rv2[:], in_=vr2[:])
                nc.scalar.activation(out=ln2_rsd[:], in_=rv2[:], func=AF.Sqrt)
                for s in range(NSW):
                    ov = sb6.tile([128, D], f32, tag='ov')
                    nc.vector.tensor_scalar(out=ov[:], in0=xc2t[s][:],
                                            scalar1=ln2_rsd[:, s:s + 1],
                                            scalar2=None, op0=AO.mult)
                    nc.vector.tensor_tensor(out=ov[:], in0=ov[:], in1=csb['ln2_g'][:], op=AO.mult)
                    nc.vector.tensor_tensor(out=ov[:], in0=ov[:], in1=csb['ln2_b'][:], op=AO.add)
                    nc.sync.dma_start(out=out_sl[s * 128:(s + 1) * 128, :], in_=ov[:])
            nc.leave_named_scope('ph6_out', sc6[0], False)

            if debug:
                nc.sync.dma_start(out=hn_dbg[:], in_=hn_local[:])
                nc.sync.dma_start(out=kv_dbg[:], in_=kv_local[:])

            k4.__exit__(None, None, None)
            xc2p_cm.__exit__(None, None, None)
            hobp_cm.__exit__(None, None, None)
            bpool_cm.__exit__(None, None, None)

    nc.finalize()
    return nc


def kernel(**inputs) -> np.ndarray:
    _ensure_hooks()
    from concourse.bass_utils import run_bass_kernel_spmd

    static, in_maps, meta = preprocess(inputs)
    key = tuple(sorted((k, str(v)) for k, v in static.items()))
    if key not in _PROGRAM_CACHE:
        _PROGRAM_CACHE[key] = build_program(static)
    nc = _PROGRAM_CACHE[key]

    trace = os.environ.get("KERNEL_TRACE") == "1"
    res = run_bass_kernel_spmd(nc, in_maps, list(range(NCORES)), trace=trace)
    if trace and res.exec_time_ns:
        print("HW exec time:", res.exec_time_ns, "ns")
    out = np.zeros((N_NODES, D), np.float32)
    for c in range(NCORES):
        n0, n1 = int(meta['n0'][c]), int(meta['n1'][c])
        out[n0:n1] = res.results[c]['out_slice'][:n1 - n0]
    return out
